# revision 70
# baseline (speedup 1.0000x reference)
"""DepthLSSTransform Trainium kernel: 3 SPMD launches over 8 NeuronCores.

Launch A: per-camera conv pipeline (dtransform + depthnet + softmax) on
          24-row bands (one 16-row + one 8-row segment per core).
Launch B: bev_pool segment-sum via one-hot matmuls over a host-built
          virtual-window schedule (sorted-by-voxel points).
Launch C: BEV downsample convs, spatially sharded.
Host: geometry/voxel indices, scheduling, gathers, folds (orchestration).
"""
import numpy as np
import ml_dtypes

import concourse.bass as bass
import concourse.tile as tile
from concourse import bacc, mybir
from concourse.bass_utils import run_bass_kernel_spmd

dt = mybir.dt
bf16 = ml_dtypes.bfloat16

# ---- problem constants (hardcoded per contract) ----
B, N = 1, 6
CIN, CIMG, DD = 256, 80, 59
FH, FW, IH, IW = 32, 88, 256, 704
XY0, DXY, NX = -54.0, 0.3, 360
Z0, DZ, NZ = -10.0, 20.0, 1
NPTS = N * DD * FH * FW
NPIX = N * FH * FW
NCORES = 8
QV = 4                      # chunks of 128 points per virtual window

# per-core segments: (camera, h0) for seg A (16 rows) and seg B (8 rows)
SEG_A = [(0, 0), (1, 0), (1, 16), (2, 16), (3, 0), (4, 0), (4, 16), (5, 16)]
SEG_B = [(0, 16), (0, 24), (2, 0), (2, 8), (3, 16), (3, 24), (5, 0), (5, 8)]
# band pixel ranges in global row order (row = n*32 + h)
ROWS_OF_CORE = [[(SEG_A[c][0] * FH + SEG_A[c][1] + r) for r in range(16)] +
                [(SEG_B[c][0] * FH + SEG_B[c][1] + r) for r in range(8)]
                for c in range(NCORES)]

# segment geometry: rows16 segment: d rows [8h0-34, 8h0+158) (192), dt2 out
# rows [2h0-8, 2h0+39) (47), dt3 [h0-3, h0+19) (22), dn1 [h0-1, h0+17) (18)
SEGS = [dict(nout=16, nd=192, nq=48, nt2=47, nt3=22, nn1=18),
        dict(nout=8, nd=128, nq=32, nt2=31, nt3=14, nn1=10)]


def _seg_ranges(h0, S):
    return dict(d0=8 * h0 - 34, q0=2 * h0 - 8, t0=h0 - 3, r0=h0 - 1, o0=h0)


# ---------------------------------------------------------------- launch A
def build_launch_a(debug=False, psum_bufs=3, work_bufs=3, stages=9):
    nc = bacc.Bacc("TRN2", target_bir_lowering=False, debug=False,
                   num_devices=NCORES)
    AP = {}

    def inp(name, shape, dtype=dt.bfloat16):
        AP[name] = nc.dram_tensor(name, shape, dtype, kind="ExternalInput").ap()
        return AP[name]

    # per segment inputs (s = 0: 16-row, 1: 8-row); flat free dims so DMAs
    # are single-descriptor-per-partition and tile deps stay precise
    for s, S in enumerate(SEGS):
        inp(f"dph{s}", [128, S["nq"] * 177])
        inp(f"masks{s}", [128, S["nt2"] + S["nt3"] + S["nn1"]])
        inp(f"xseg{s}", [2, 128, S["nt3"] * 92])        # x_img slice (padded)
    # packed f32 constants: [alpha, beta, s_dt2, t_dt2, s_dt3, t_dt3,
    #  s_dn1(2), t_dn1(2), s_dn2(2), t_dn2(2), b_dn3(139)] -> [128, 153]
    inp("consts", [128, 153], dt.float32)
    # conv weights (host-prepped layouts)
    inp("w_dt2", [4, 128, 32])                          # groups (dky,dmx)
    inp("w_dt3", [9, 128, 64])
    inp("w_dn1", [9, 3, 128, 256])                      # tap, icchunk(128,128,64pad) -> 256
    inp("w_dn2", [9, 2, 128, 256])
    inp("w_dn3", [2, 128, 139])

    DBG = {}
    dbg_specs = [] if not debug else [("dbg_t1", [128, SEGS[0]["nq"], 177], dt.bfloat16),
                        ("dbg_dt2o", [32, SEGS[0]["nt2"] + 1, 180], dt.bfloat16),
                        ("dbg_dtc", [64, SEGS[0]["nt3"], 92], dt.bfloat16),
                        ("dbg_n1o", [128, SEGS[0]["nn1"], 92], dt.bfloat16),
                        ("dbg_n2o", [128, SEGS[0]["nout"], 88], dt.bfloat16)]
    for nm, sh, dty in dbg_specs:
        DBG[nm] = nc.dram_tensor(nm, sh, dty, kind="ExternalOutput").ap()
    # chunk-major outputs: pixel (a*128+p) of segment s at [p, a0_s + a, :]
    out_depth = nc.dram_tensor("out_depth", [128, 17, DD], dt.float32,
                               kind="ExternalOutput").ap()
    out_feat = nc.dram_tensor("out_feat", [128, 17, CIMG], dt.bfloat16,
                              kind="ExternalOutput").ap()

    # HBM scratch, phase-major: [c32, a2, b2, q', x90] (q' = dt2-row // 2)
    scr = {}
    for s, S in enumerate(SEGS):
        scr[f"dt2o{s}"] = nc.dram_tensor(
            f"dt2o{s}", [32, 2, 2, (S["nt2"] + 1) // 2, 90], dt.bfloat16).ap()

    RELU = mybir.ActivationFunctionType.Relu
    with tile.TileContext(nc) as tc:
        with tc.tile_pool(name="const", bufs=1) as cpool, \
             tc.tile_pool(name="work", bufs=work_bufs) as wpool, \
             tc.tile_pool(name="big", bufs=1) as bpool, \
             tc.tile_pool(name="psum", bufs=2, space="PSUM") as ppool, \
             tc.tile_pool(name="psum2", bufs=4, space="PSUM") as ppool2:
            # ---- DMA issue order = consumption order (the SP queue and the
            # modeled DMA engines serialize; early-stage inputs must land first)
            cts = cpool.tile([128, 153], dt.float32, name="cts")
            nc.sync.dma_start(out=cts[:], in_=AP["consts"])
            # tiny activation right away so the act-table load happens while
            # the first dph chunk is still in flight
            warm = wpool.tile([128, 1], dt.float32, tag="warm", name="warm")
            nc.scalar.activation(warm[:], cts[:, 0:1], RELU)
            ct = {"dt1_alpha": cts[:, 0:1], "dt1_beta": cts[:, 1:2],
                  "s_dt2": cts[:, 2:3], "t_dt2": cts[:, 3:4],
                  "s_dt3": cts[:, 4:5], "t_dt3": cts[:, 5:6],
                  "s_dn1": cts[:, 6:8], "t_dn1": cts[:, 8:10],
                  "s_dn2": cts[:, 10:12], "t_dn2": cts[:, 12:14],
                  "b_dn3": cts[:, 14:153]}
            wt = {}

            def load_w(nm, pat):
                sh = list(AP[nm].shape)
                wt[nm] = cpool.tile([sh[-2], int(np.prod(sh[:-2])), sh[-1]],
                                    dt.bfloat16, tag=nm, name=f'wt_{nm}')
                nc.sync.dma_start(out=wt[nm][:], in_=AP[nm].rearrange(pat))

            # first dph chunk small so dt2 starts ASAP; host has already
            # applied dt1 (relu(alpha*d+beta), pads zeroed) into dph.
            # The big dn-weights are issued later (stage_wload) so they don't
            # sit ahead of the dt2->dt3 scratch roundtrip in the serial DMA
            # stream.
            QCHUNKS = {0: [6, 14, 14, 14], 1: [6, 13, 13]}
            dphs, malls = {}, {}

            def load_dph(s):
                S = SEGS[s]
                nq = S["nq"]
                dphs[s] = bpool.tile([128, nq * 177], dt.bfloat16,
                                     tag=f"dph{s}", name=f"dph{s}")
                qq = 0
                for nqq in QCHUNKS[s]:
                    nc.sync.dma_start(
                        out=dphs[s][:, qq * 177:(qq + nqq) * 177],
                        in_=AP[f"dph{s}"][:, qq * 177:(qq + nqq) * 177])
                    qq += nqq
                malls[s] = wpool.tile([128, S["nt2"] + S["nt3"] + S["nn1"]],
                                      dt.bfloat16, tag=f"msk{s}", name="mall")
                nc.sync.dma_start(out=malls[s][:], in_=AP[f"masks{s}"])

            load_w("w_dt2", "g p o -> p g o")
            load_dph(0)
            load_dph(1)
            load_w("w_dt3", "g p o -> p g o")

            def stage_wload():
                load_w("w_dn1", "t i p o -> p (t i) o")
                load_w("w_dn2", "t i p o -> p (t i) o")
                load_w("w_dn3", "g p o -> p g o")

            feat_sb = {}
            depth_sb = {}
            st = {s: {} for s in range(len(SEGS))}

            def stage_dt2(s):
                S = SEGS[s]
                nt2, t1, mall = S["nt2"], dphs[s], malls[s]
                Q2 = (nt2 + 1) // 2
                # phase-major layout [c32, a2, b2, q', x90]: row q=(2q'+a),
                # col c at (b=c%2, x=c//2+1); makes scr write + ph3 reads
                # fully contiguous per partition
                o2 = bpool.tile([32, 2, 2, Q2, 90], dt.bfloat16, tag=f"o2{s}",
                                name=f"o2{s}")
                st[s]["o2"] = o2
                o2f = o2.rearrange("p a b q x -> p (a b q) x")
                nc.vector.memset(o2f[:, :, 0:1], 0.0)          # x pad left
                nc.vector.memset(o2f[:, :, 89:90], 0.0)        # x pad right
                nc.vector.memset(o2[:, 1, :, Q2 - 1, :], 0.0)  # pad row q=nt2
                m2 = bass.AP(mall.tensor, mall.offset, [mall.ap[0], [1, nt2]])
                RPP2 = 2
                for q0 in range(0, nt2, RPP2):
                    nr = min(RPP2, nt2 - q0)
                    ps = ppool2.tile([32, nr, 176], dt.float32, tag="ps2",
                                     name="ps2")
                    gi = 0
                    for dky in range(2):
                        for dmx in range(2):
                            g = dky * 2 + dmx
                            rhs = bass.AP(
                                t1.tensor, t1.offset + (q0 + dky) * 177 + dmx,
                                [t1.ap[0], [177, nr], [1, 176]])
                            nc.tensor.matmul(ps[:], wt["w_dt2"][:, g, :], rhs,
                                             start=(gi == 0), stop=(gi == 3))
                            gi += 1
                    ev = wpool.tile([32, nr, 176], dt.bfloat16, tag=f"ev2{s}")
                    nc.scalar.activation(ev[:], ps[:], RELU,
                                         bias=ct["t_dt2"][0:32, 0:1],
                                         scale=ct["s_dt2"][0:32, 0:1])
                    mbb = bass.AP(m2.tensor, m2.offset + q0,
                                  [[m2.ap[0][0], 32], [1, nr], [0, 176]])
                    # rows (q0, q0+1) -> a=(0,1) at q'=q0//2; c -> (x, b)
                    o2dst = bass.AP(o2.tensor,
                                    o2.offset + (q0 // 2) * 90 + 1,
                                    [[o2.ap[0][0], 32], [2 * Q2 * 90, nr],
                                     [1, 88], [Q2 * 90, 2]])
                    nc.vector.tensor_tensor(out=o2dst, in0=ev[:], in1=mbb,
                                            op=mybir.AluOpType.mult)

            def scr_write(s):
                nc.sync.dma_start(out=scr[f"dt2o{s}"], in_=st[s]["o2"][:])

            def stage_dt3(s):
                S = SEGS[s]
                nt2, nt3, mall = S["nt2"], S["nt3"], malls[s]
                Q2 = (nt2 + 1) // 2
                nry3 = nt3 + 2
                ph3 = bpool.tile([128, nry3 * 90], dt.bfloat16, tag=f"ph3{s}",
                                 name=f"ph3{s}")
                sd2 = scr[f"dt2o{s}"]
                for g in range(4):
                    pap3 = bass.AP(sd2.tensor, sd2.offset + g * Q2 * 90,
                                   [[4 * Q2 * 90, 32], [1, nry3 * 90]])
                    nc.sync.dma_start(out=ph3[g * 32:(g + 1) * 32, :],
                                      in_=pap3)
                # concat input tile: [64 dt3 | pad] plus x_img tiles
                dtc = bpool.tile([64, nt3, 92], dt.bfloat16, tag=f"dtc{s}",
                                 name=f"dtc{s}")
                st[s]["dtc"] = dtc
                nc.vector.memset(dtc[:, :, 0:2], 0.0)
                nc.vector.memset(dtc[:, :, 90:92], 0.0)
                m3 = bass.AP(mall.tensor, mall.offset + nt2,
                             [mall.ap[0], [1, nt3]])
                RPP3 = 4
                for t0 in range(0, nt3, RPP3):
                    nr = min(RPP3, nt3 - t0)
                    ps = ppool.tile([64, nr, 88], dt.float32, tag=f"ps{s}")
                    gi = 0
                    for dky in range(3):
                        for dmx in range(3):
                            g = dky * 3 + dmx
                            rhs = bass.AP(ph3.tensor,
                                          ph3.offset + (t0 + dky) * 90 + dmx,
                                          [ph3.ap[0], [90, nr], [1, 88]])
                            nc.tensor.matmul(ps[:], wt["w_dt3"][:, g, :], rhs,
                                             start=(gi == 0), stop=(gi == 8))
                            gi += 1
                    ev = wpool.tile([64, nr, 88], dt.bfloat16, tag=f"ev3{s}")
                    nc.scalar.activation(ev[:], ps[:], RELU,
                                         bias=ct["t_dt3"][0:64, 0:1],
                                         scale=ct["s_dt3"][0:64, 0:1])
                    mbb = bass.AP(m3.tensor, m3.offset + t0,
                                  [m3.ap[0], [1, nr], [0, 88]])
                    nc.vector.tensor_tensor(out=dtc[:, t0:t0 + nr, 2:90],
                                            in0=ev[:], in1=mbb[0:64],
                                            op=mybir.AluOpType.mult)

            def stage_xload(s):
                S = SEGS[s]
                xs = []
                for g in range(2):
                    xt = bpool.tile([128, S["nt3"] * 92], dt.bfloat16,
                                    tag=f"x{g}_{s}", name=f"xseg_t{g}")
                    nc.sync.dma_start(out=xt[:], in_=AP[f"xseg{s}"][g])
                    xs.append(xt)
                st[s]["xs"] = xs

            def stage_dn1(s):
                S = SEGS[s]
                nt2, nt3, nn1 = S["nt2"], S["nt3"], S["nn1"]
                mall, dtc, xs = malls[s], st[s]["dtc"], st[s]["xs"]
                mn1 = bass.AP(mall.tensor, mall.offset + nt2 + nt3,
                              [mall.ap[0], [1, nn1]])
                n1o = []
                for g in range(2):
                    t = bpool.tile([128, nn1, 92], dt.bfloat16,
                                   tag=f"n1o{g}_{s}", name=f"n1o{g}_{s}")
                    nc.vector.memset(t[:, :, 0:2], 0.0)
                    nc.vector.memset(t[:, :, 90:92], 0.0)
                    n1o.append(t)
                st[s]["n1o"] = n1o
                RPP = 5
                for ocg in range(2):
                    for r0 in range(0, nn1, RPP):
                        nr = min(RPP, nn1 - r0)
                        ps = ppool.tile([128, nr, 88], dt.float32, tag=f"ps{s}")
                        gi = 0
                        for ky in range(3):
                            for kx in range(3):
                                tap = ky * 3 + kx
                                for icc, srcT in enumerate((xs[0], xs[1], dtc)):
                                    kk = 128 if icc < 2 else 64
                                    rhs = bass.AP(
                                        srcT.tensor,
                                        srcT.offset + (r0 + ky + 1) * 92 + kx + 1,
                                        [srcT.ap[0], [92, nr], [1, 88]])
                                    lhs = wt["w_dn1"][0:kk, tap * 3 + icc,
                                                      ocg * 128:(ocg + 1) * 128]
                                    nc.tensor.matmul(ps[:], lhs, rhs,
                                                     start=(gi == 0),
                                                     stop=(gi == 26))
                                    gi += 1
                        ev = wpool.tile([128, nr, 88], dt.bfloat16, tag=f"evn1{s}")
                        nc.scalar.activation(ev[:], ps[:], RELU,
                                             bias=ct["t_dn1"][:, ocg:ocg + 1],
                                             scale=ct["s_dn1"][:, ocg:ocg + 1])
                        mbb = bass.AP(mn1.tensor, mn1.offset + r0,
                                      [mn1.ap[0], [1, nr], [0, 88]])
                        nc.vector.tensor_tensor(
                            out=n1o[ocg][:, r0:r0 + nr, 2:90],
                            in0=ev[:], in1=mbb, op=mybir.AluOpType.mult)

            def stage_dn2(s):
                S = SEGS[s]
                nout, n1o = S["nout"], st[s]["n1o"]
                RPP = 5
                n2o = []
                for g in range(2):
                    n2o.append(bpool.tile([128, nout, 88], dt.bfloat16,
                                          tag=f"n2o{g}_{s}", name=f"n2o{g}_{s}"))
                st[s]["n2o"] = n2o
                dn3 = stage_dn3(s)
                next(dn3)                        # prime: allocates out tiles
                for r0 in range(0, nout, RPP):
                    nr = min(RPP, nout - r0)
                    for ocg in range(2):
                        ps = ppool.tile([128, nr, 88], dt.float32, tag=f"ps{s}")
                        gi = 0
                        for ky in range(3):
                            for kx in range(3):
                                tap = ky * 3 + kx
                                for icc in range(2):
                                    rhs = bass.AP(
                                        n1o[icc].tensor,
                                        n1o[icc].offset + (r0 + ky) * 92 + kx + 1,
                                        [n1o[icc].ap[0], [92, nr], [1, 88]])
                                    lhs = wt["w_dn2"][:, tap * 2 + icc,
                                                      ocg * 128:(ocg + 1) * 128]
                                    nc.tensor.matmul(ps[:], lhs, rhs,
                                                     start=(gi == 0),
                                                     stop=(gi == 17))
                                    gi += 1
                        ev = wpool.tile([128, nr, 88], dt.bfloat16, tag=f"evn2{s}")
                        nc.scalar.activation(ev[:], ps[:], RELU,
                                             bias=ct["t_dn2"][:, ocg:ocg + 1],
                                             scale=ct["s_dn2"][:, ocg:ocg + 1])
                        nc.vector.tensor_copy(n2o[ocg][:, r0:r0 + nr, :], ev[:])
                    try:
                        dn3.send(r0 + nr)        # emit dn3 chunks now ready
                    except StopIteration:
                        pass

            def stage_dn3(s):
                """Generator: receives the count of completed dn2 rows and
                emits dn3+softmax for pixel chunks whose rows are ready."""
                S = SEGS[s]
                nout, n2o = S["nout"], st[s]["n2o"]
                npix = nout * FW
                feat_sb[s] = bpool.tile([128, ((npix + 127) // 128) * CIMG],
                                        dt.bfloat16, tag=f"feat{s}", name=f"feat_sb{s}")
                depth_sb[s] = bpool.tile([128, ((npix + 127) // 128) * DD],
                                         dt.float32, tag=f"depth{s}", name=f"depth_sb{s}")
                n2f = [t.rearrange("p a b -> p (a b)") for t in n2o]
                a0 = 0 if s == 0 else 11
                pcs = (npix + 127) // 128
                rows_done = yield
                for pc in range(pcs):
                    if pc == pcs - 1:
                        # flush all-but-last chunk now so only the final
                        # chunk's output DMA sits in the tail
                        dsl = bass.AP(out_depth.tensor,
                                      out_depth.offset + a0 * DD,
                                      [[17 * DD, 128], [1, (pcs - 1) * DD]])
                        nc.sync.dma_start(
                            out=dsl, in_=depth_sb[s][:, 0:(pcs - 1) * DD])
                        fsl = bass.AP(out_feat.tensor,
                                      out_feat.offset + a0 * CIMG,
                                      [[17 * CIMG, 128], [1, (pcs - 1) * CIMG]])
                        nc.sync.dma_start(
                            out=fsl, in_=feat_sb[s][:, 0:(pcs - 1) * CIMG])
                    m = min(128, npix - pc * 128)
                    # rows needed by pixels [pc*128, pc*128+m)
                    need = (pc * 128 + m - 1) // FW + 1
                    while rows_done < need:
                        rows_done = yield
                    ps = ppool.tile([m, 139], dt.float32, tag=f"ps{s}")
                    for icc in range(2):
                        nc.tensor.matmul(ps[:], n2f[icc][:, pc * 128:pc * 128 + m],
                                         wt["w_dn3"][:, icc, :],
                                         start=(icc == 0), stop=(icc == 1))
                    # add bias via vector then softmax over first 59
                    lg = wpool.tile([m, 139], dt.float32, tag=f"lg{s}")
                    nc.vector.tensor_tensor(out=lg[:], in0=ps[:],
                                            in1=ct["b_dn3"][0:m],
                                            op=mybir.AluOpType.add)
                    mx = wpool.tile([m, 1], dt.float32, tag=f"mx{s}")
                    nc.vector.reduce_max(mx[:], lg[:, 0:DD],
                                         axis=mybir.AxisListType.X, negate=True)
                    ex = wpool.tile([m, DD], dt.float32, tag=f"ex{s}")
                    nc.scalar.activation(ex[:], lg[:, 0:DD],
                                         mybir.ActivationFunctionType.Exp,
                                         bias=mx[:, 0:1], scale=1.0)
                    sm = wpool.tile([m, 1], dt.float32, tag=f"sm{s}")
                    nc.vector.reduce_sum(sm[:], ex[:], axis=mybir.AxisListType.X)
                    rc = wpool.tile([m, 1], dt.float32, tag=f"rc{s}")
                    nc.vector.reciprocal(rc[:], sm[:])
                    nc.vector.tensor_scalar(
                        out=depth_sb[s][0:m, pc * DD:(pc + 1) * DD], in0=ex[:],
                        scalar1=rc[:, 0:1], scalar2=None,
                        op0=mybir.AluOpType.mult)
                    nc.vector.tensor_copy(
                        feat_sb[s][0:m, pc * CIMG:(pc + 1) * CIMG],
                        lg[:, DD:DD + CIMG])

                # final chunk's outputs
                dsl = bass.AP(out_depth.tensor,
                              out_depth.offset + (a0 + pcs - 1) * DD,
                              [[17 * DD, 128], [1, DD]])
                nc.sync.dma_start(out=dsl,
                                  in_=depth_sb[s][:, (pcs - 1) * DD:pcs * DD])
                fsl = bass.AP(out_feat.tensor,
                              out_feat.offset + (a0 + pcs - 1) * CIMG,
                              [[17 * CIMG, 128], [1, CIMG]])
                nc.sync.dma_start(out=fsl,
                                  in_=feat_sb[s][:, (pcs - 1) * CIMG:pcs * CIMG])

            # schedule: dt1 is folded into the host's dph prep; dt2(1)/dt3(0)
            # hide the scr roundtrips; dn3 is fused into dn2 so softmax
            # pipelines under matmuls
            stage_dt2(0)
            scr_write(0)
            stage_dt2(1)
            stage_dt3(0)
            stage_xload(0)
            scr_write(1)
            stage_dt3(1)
            stage_xload(1)
            stage_wload()
            stage_dn1(0)
            stage_dn1(1)
            stage_dn2(0)
            stage_dn2(1)
    nc.compile()
    return nc


# ------------------------------------------------------------ host helpers
def _host_geometry(rots, trans, intr, post_rots, post_trans):
    import jax
    import jax.numpy as jnp
    with jax.default_device(jax.devices("cpu")[0]):
        f32 = jnp.float32
        ds = jnp.arange(1.0, 60.0, 1.0, dtype=f32)
        xs = jnp.linspace(0.0, IW - 1.0, FW, dtype=f32)
        ys = jnp.linspace(0.0, IH - 1.0, FH, dtype=f32)
        dm = jnp.broadcast_to(ds[:, None, None], (DD, FH, FW))
        xm = jnp.broadcast_to(xs[None, None, :], (DD, FH, FW))
        ym = jnp.broadcast_to(ys[None, :, None], (DD, FH, FW))
        fr = jnp.stack([xm, ym, dm], -1)
        pts = fr[None, None] - jnp.asarray(post_trans)[:, :, None, None, None, :]
        pts = jnp.einsum("bnij,bndhwj->bndhwi",
                         jnp.linalg.inv(jnp.asarray(post_rots)), pts)
        pts = jnp.concatenate([pts[..., :2] * pts[..., 2:3], pts[..., 2:3]], -1)
        comb = jnp.einsum("bnij,bnjk->bnik", jnp.asarray(rots),
                          jnp.linalg.inv(jnp.asarray(intr)))
        pts = jnp.einsum("bnij,bndhwj->bndhwi", comb, pts) \
            + jnp.asarray(trans)[:, :, None, None, None, :]
        lo = jnp.array([XY0, XY0, Z0], dtype=f32)
        dxv = jnp.array([DXY, DXY, DZ], dtype=f32)
        g = ((pts - lo) / dxv).astype(jnp.int32).reshape(-1, 3)
        kept = ((g[:, 0] >= 0) & (g[:, 0] < NX) & (g[:, 1] >= 0) & (g[:, 1] < NX)
                & (g[:, 2] >= 0) & (g[:, 2] < NZ))
        flat = (g[:, 2] * NX + g[:, 0]) * NX + g[:, 1]
        return np.asarray(flat, np.int64), np.asarray(kept)


def _prep_a_inputs(inputs):
    """Build per-core input maps for launch A."""
    d = np.asarray(inputs["d"], np.float32).reshape(N, IH, IW)
    x_img = np.asarray(inputs["x_img"], np.float32)

    # dt1 folded affine: relu(alpha*d + beta), alpha = s*w, beta = s*b + t
    a1 = (inputs["dt1_s"] * inputs["dt1_w"][:, 0, 0, 0]).astype(np.float32)
    b1 = (inputs["dt1_s"] * inputs["dt1_b"] + inputs["dt1_t"]).astype(np.float32)
    cab = np.arange(128)
    dt1_alpha = a1[cab // 16][:, None]
    dt1_beta = b1[cab // 16][:, None]

    def wprep_dt2():
        w = np.asarray(inputs["dt2_w"], np.float32)      # [32,8,5,5]
        out = np.zeros((4, 128, 32), np.float32)
        for ky in range(5):
            for kx in range(5):
                a, dky = ky % 4, ky // 4
                bph, dmx = (kx + 2) % 4, (kx + 2) // 4
                g = dky * 2 + dmx
                rows = (np.arange(8)) * 16 + a * 4 + bph
                out[g, rows, :] = w[:, :, ky, kx].T
        return out.astype(bf16)

    def wprep_dt3():
        w = np.asarray(inputs["dt3_w"], np.float32)      # [64,32,5,5]
        out = np.zeros((9, 128, 64), np.float32)
        for ky in range(5):
            for kx in range(5):
                a, dky = ky % 2, ky // 2
                bph, dmx = kx % 2, (kx + 2) // 2 - 1
                g = dky * 3 + dmx
                rows = (a * 2 + bph) * 32 + np.arange(32)
                out[g, rows, :] = w[:, :, ky, kx].T
        return out.astype(bf16)

    def wprep_3x3(w, icc_sizes):
        O, I = w.shape[0], w.shape[1]
        nic = len(icc_sizes)
        out = np.zeros((9, nic, 128, O), np.float32)
        for ky in range(3):
            for kx in range(3):
                tap = ky * 3 + kx
                ic0 = 0
                for icc, sz in enumerate(icc_sizes):
                    out[tap, icc, 0:sz, :] = w[:, ic0:ic0 + sz, ky, kx].T
                    ic0 += sz
        return out.astype(bf16)

    # NOTE: dn1 input concat order is [dt3(64) | x_img(256)] in the reference;
    # our matmul chunks are (x0:128, x1:128, dt3:64) -> weight cols must match:
    w_dn1_full = np.asarray(inputs["dn1_w"], np.float32)
    w_dn1 = np.zeros((9, 3, 128, 256), np.float32)
    for ky in range(3):
        for kx in range(3):
            tap = ky * 3 + kx
            w_dn1[tap, 0, :, :] = w_dn1_full[:, 64:192, ky, kx].T
            w_dn1[tap, 1, :, :] = w_dn1_full[:, 192:320, ky, kx].T
            w_dn1[tap, 2, 0:64, :] = w_dn1_full[:, 0:64, ky, kx].T
    w_dn1 = w_dn1.astype(bf16)
    w_dn2 = wprep_3x3(np.asarray(inputs["dn2_w"], np.float32), [128, 128])
    w_dn3 = np.asarray(inputs["dn3_w"], np.float32)[:, :, 0, 0]  # [139, 256]
    w_dn3p = np.zeros((2, 128, 139), np.float32)
    w_dn3p[0] = w_dn3[:, 0:128].T
    w_dn3p[1] = w_dn3[:, 128:256].T

    def fold_bias(b, s, t):
        # conv bias b then bn scale/shift: relu(s*(x+b) + t) = relu(s*x + (s*b+t))
        return np.asarray(s, np.float32), np.asarray(s * b + t, np.float32)

    s2, t2 = fold_bias(inputs["dt2_b"], inputs["dt2_s"], inputs["dt2_t"])
    s3, t3 = fold_bias(inputs["dt3_b"], inputs["dt3_s"], inputs["dt3_t"])
    sn1, tn1 = fold_bias(inputs["dn1_b"], inputs["dn1_s"], inputs["dn1_t"])
    sn2, tn2 = fold_bias(inputs["dn2_b"], inputs["dn2_s"], inputs["dn2_t"])
    b_dn3 = np.broadcast_to(np.asarray(inputs["dn3_b"], np.float32)[None, :],
                            (128, 139)).copy()

    consts = np.zeros((128, 153), np.float32)
    consts[:, 0] = dt1_alpha[:, 0]
    consts[:, 1] = dt1_beta[:, 0]
    consts[:, 2] = np.tile(s2, 4)
    consts[:, 3] = np.tile(t2, 4)
    consts[:, 4] = np.tile(s3, 2)
    consts[:, 5] = np.tile(t3, 2)
    consts[:, 6:8] = sn1.reshape(2, 128).T
    consts[:, 8:10] = tn1.reshape(2, 128).T
    consts[:, 10:12] = sn2.reshape(2, 128).T
    consts[:, 12:14] = tn2.reshape(2, 128).T
    consts[:, 14:153] = b_dn3
    shared = dict(
        consts=consts,
        w_dt2=wprep_dt2(), w_dt3=wprep_dt3(), w_dn1=w_dn1, w_dn2=w_dn2,
        w_dn3=w_dn3p.astype(bf16),
    )

    maps = []
    for c in range(NCORES):
        m = dict(shared)
        for s, (cam, h0) in enumerate([SEG_A[c], SEG_B[c]]):
            S = SEGS[s]
            d0 = 8 * h0 - 34
            dseg = np.zeros((S["nd"], 712), np.float32)
            vseg = np.zeros((S["nd"], 712), bool)
            lo, hi = max(0, d0), min(IH, d0 + S["nd"])
            if hi > lo:
                dseg[lo - d0:hi - d0, 4:708] = d[cam, lo:hi]
                vseg[lo - d0:hi - d0, 4:708] = True
            nq = S["nq"]
            ph = dseg.reshape(nq, 4, 178, 4)[:, :, :177, :]     # ry a rx b
            ph = ph.transpose(1, 3, 0, 2)                        # a b ry rx
            vph = vseg.reshape(nq, 4, 178, 4)[:, :, :177, :].transpose(1, 3, 0, 2)
            # dt1 applied on host: relu(alpha*d + beta), zero at pads
            dphc = np.where(vph[None],
                            np.maximum(a1[:, None, None, None, None] * ph[None]
                                       + b1[:, None, None, None, None], 0.0),
                            0.0)                                 # [8,4,4,nq,177]
            m[f"dph{s}"] = dphc.reshape(128, nq * 177).astype(bf16)
            q0, t0, r0 = 2 * h0 - 8, h0 - 3, h0 - 1
            qr = np.arange(S["nt2"]) + q0
            m2m = np.broadcast_to(((qr >= 0) & (qr < 64))[None, :],
                                  (128, S["nt2"]))
            tr = np.arange(S["nt3"]) + t0
            m3m = np.broadcast_to(((tr >= 0) & (tr < FH))[None, :],
                                  (128, S["nt3"]))
            rr = np.arange(S["nn1"]) + r0
            mn1m = np.broadcast_to(((rr >= 0) & (rr < FH))[None, :],
                                   (128, S["nn1"]))
            m[f"masks{s}"] = np.concatenate(
                [m2m, m3m, mn1m], axis=1).astype(bf16)
            xseg = np.zeros((2, 128, S["nt3"], 92), np.float32)
            lo2, hi2 = max(0, t0), min(FH, t0 + S["nt3"])
            if hi2 > lo2:
                xseg[:, :, lo2 - t0:hi2 - t0, 2:90] = \
                    x_img[cam, :, lo2:hi2, :].reshape(2, 128, hi2 - lo2, FW)
            m[f"xseg{s}"] = xseg.reshape(2, 128, S["nt3"] * 92).astype(bf16)
        maps.append(m)
    return maps


# ---------------------------------------------------------------- launch B
def build_launch_b(sizes):
    """Per chunk k: [128pix x 80ch] stationary feat tile x host-built
    [128pix x sizes[k] voxel-slot] depth-weight matrix -> [80, nv] window
    sums. W and out use packed (variable-size) layouts; W loads in a few
    batched DMAs, out in one."""
    nc = bacc.Bacc("TRN2", target_bir_lowering=False, debug=False,
                   num_devices=NCORES)
    NCH = len(sizes)
    offs = np.concatenate([[0], np.cumsum(sizes)]).astype(int)
    S = int(offs[-1])
    wmat = nc.dram_tensor("wmat", [128, S], dt.bfloat16,
                          kind="ExternalInput").ap()
    feats = nc.dram_tensor("feats", [128, NCH, CIMG], dt.bfloat16,
                           kind="ExternalInput").ap()
    owin = nc.dram_tensor("owin", [CIMG, S], dt.bfloat16,
                          kind="ExternalOutput").ap()
    NB = 4                                   # W DMA batches
    bnd = [int(round(NCH * i / NB)) for i in range(NB + 1)]
    with tile.TileContext(nc) as tc:
        with tc.tile_pool(name="const", bufs=1) as cpool, \
             tc.tile_pool(name="ps", bufs=4, space="PSUM") as pp:
            ft = cpool.tile([128, NCH, CIMG], dt.bfloat16, name="ft")
            nc.sync.dma_start(out=ft[:], in_=feats)
            wt = cpool.tile([128, S], dt.bfloat16, name="wt")
            for b in range(NB):
                lo, hi = offs[bnd[b]], offs[bnd[b + 1]]
                if hi > lo:
                    nc.sync.dma_start(out=wt[:, lo:hi], in_=wmat[:, lo:hi])
            ot = cpool.tile([CIMG, S], dt.bfloat16, name="ot")
            for k in range(NCH):
                nv, o0 = int(sizes[k]), int(offs[k])
                ps = pp.tile([CIMG, 512], dt.float32, tag="ps", name="ps")
                nc.tensor.matmul(ps[:, 0:nv], ft[:, k, :], wt[:, o0:o0 + nv],
                                 start=True, stop=True)
                if k % 2 == 0:
                    nc.scalar.activation(ot[:, o0:o0 + nv], ps[:, 0:nv],
                                         mybir.ActivationFunctionType.Copy)
                else:
                    nc.vector.tensor_copy(ot[:, o0:o0 + nv], ps[:, 0:nv])
            nc.sync.dma_start(out=owin, in_=ot[:])
    nc.compile()
    return nc


# ---------------------------------------------------------------- launch C
C_OUT_ROWS = 23              # ds2-out rows per core (8*23 = 184 >= 180)


def build_launch_c():
    nc = bacc.Bacc("TRN2", target_bir_lowering=False, debug=False,
                   num_devices=NCORES)
    NR1 = C_OUT_ROWS + 2                         # ds1-out rows incl halo (25)
    NRP = 2 * NR1 + 1                            # pooled rows needed (51)
    slab = nc.dram_tensor("slab", [CIMG, NRP, 362], dt.bfloat16,
                          kind="ExternalInput").ap()
    m1 = nc.dram_tensor("m1", [128, NR1], dt.bfloat16, kind="ExternalInput").ap()
    wd1 = nc.dram_tensor("wd1", [9, CIMG, CIMG], dt.bfloat16,
                         kind="ExternalInput").ap()
    wd2 = nc.dram_tensor("wd2", [9, CIMG, CIMG], dt.bfloat16,
                         kind="ExternalInput").ap()
    sb1 = nc.dram_tensor("sb1", [CIMG, 2], dt.float32, kind="ExternalInput").ap()
    sb2 = nc.dram_tensor("sb2", [CIMG, 2], dt.float32, kind="ExternalInput").ap()
    yout = nc.dram_tensor("yout", [CIMG, C_OUT_ROWS, 180], dt.float32,
                          kind="ExternalOutput").ap()
    RELU = mybir.ActivationFunctionType.Relu
    with tile.TileContext(nc) as tc:
        with tc.tile_pool(name="const", bufs=1) as cpool,              tc.tile_pool(name="work", bufs=2) as wp,              tc.tile_pool(name="big", bufs=1) as bp,              tc.tile_pool(name="ps", bufs=3, space="PSUM") as pp:
            # weights/consts first so ds1 can start on the first slab chunk
            w1 = cpool.tile([CIMG, 9, CIMG], dt.bfloat16, name="w1")
            nc.sync.dma_start(out=w1[:], in_=wd1.rearrange("t p o -> p t o"))
            sb1t = cpool.tile([CIMG, 2], dt.float32, name="sb1t")
            nc.sync.dma_start(out=sb1t[:], in_=sb1)
            m1t = wp.tile([128, NR1], dt.bfloat16, name="m1t")
            nc.sync.dma_start(out=m1t[:], in_=m1)
            slabt = bp.tile([CIMG, NRP, 362], dt.bfloat16, name="slabt")
            for rr in range(0, NRP, 9):
                nrr = min(9, NRP - rr)
                nc.sync.dma_start(out=slabt[:, rr:rr + nrr, :],
                                  in_=slab[:, rr:rr + nrr, :])
            w2 = cpool.tile([CIMG, 9, CIMG], dt.bfloat16, name="w2")
            nc.sync.dma_start(out=w2[:], in_=wd2.rearrange("t p o -> p t o"))
            sb2t = cpool.tile([CIMG, 2], dt.float32, name="sb2t")
            nc.sync.dma_start(out=sb2t[:], in_=sb2)
            h1 = bp.tile([CIMG, NR1, 182], dt.bfloat16, name="h1")
            nc.vector.memset(h1[:, :, 0:1], 0.0)
            nc.vector.memset(h1[:, :, 181:182], 0.0)
            # ds1: stride-2 3x3; out row t reads slab rows 2t..2t+2 (slab row 0
            # = pooled row 2o0-3, so out row t (global o0-1+t) reads
            # 2(o0-1+t)-1..+1 - (2o0-3) = 2t..2t+2); col c reads 2c..2c+2
            RP = 2
            for t0 in range(0, NR1, RP):
                nr = min(RP, NR1 - t0)
                ps = pp.tile([CIMG, nr, 180], dt.float32, tag="ps1", name="ps")
                gi = 0
                for ky in range(3):
                    for kx in range(3):
                        rhs = bass.AP(slabt.tensor,
                                      slabt.offset + (2 * t0 + ky) * 362 + kx,
                                      [slabt.ap[0], [2 * 362, nr], [2, 180]])
                        nc.tensor.matmul(ps[:], w1[:, ky * 3 + kx, :], rhs,
                                         start=(gi == 0), stop=(gi == 8))
                        gi += 1
                ev = wp.tile([CIMG, nr, 180], dt.bfloat16, tag="ev", name="ev")
                nc.scalar.activation(ev[:], ps[:], RELU, bias=sb1t[:, 1:2],
                                     scale=sb1t[:, 0:1])
                mbb = bass.AP(m1t.tensor, m1t.offset + t0,
                              [[m1t.ap[0][0], CIMG], [1, nr], [0, 180]])
                nc.vector.tensor_tensor(out=h1[:, t0:t0 + nr, 1:181],
                                        in0=ev[:], in1=mbb,
                                        op=mybir.AluOpType.mult)
            # ds2: 3x3 pad 1: out row o reads h1 rows o..o+2, col c: c..c+2
            yo = bp.tile([CIMG, C_OUT_ROWS, 180], dt.float32, name="yo")
            for o0 in range(0, C_OUT_ROWS, RP):
                nr = min(RP, C_OUT_ROWS - o0)
                ps = pp.tile([CIMG, nr, 180], dt.float32, tag="ps2", name="ps")
                gi = 0
                for ky in range(3):
                    for kx in range(3):
                        rhs = bass.AP(h1.tensor,
                                      h1.offset + (o0 + ky) * 182 + kx,
                                      [h1.ap[0], [182, nr], [1, 180]])
                        nc.tensor.matmul(ps[:], w2[:, ky * 3 + kx, :], rhs,
                                         start=(gi == 0), stop=(gi == 8))
                        gi += 1
                nc.scalar.activation(yo[:, o0:o0 + nr, :], ps[:], RELU,
                                     bias=sb2t[:, 1:2], scale=sb2t[:, 0:1])
                if (o0 // RP) % 3 == 2 or o0 + nr >= C_OUT_ROWS:
                    lo = (o0 // (3 * RP)) * 3 * RP
                    nc.sync.dma_start(out=yout[:, lo:o0 + nr, :],
                                      in_=yo[:, lo:o0 + nr, :])
    nc.compile()
    return nc


_CACHE = {}


def run_launch_a(inputs):
    if "A" not in _CACHE:
        _CACHE["A"] = build_launch_a()
    nc = _CACHE["A"]
    maps = _prep_a_inputs(inputs)
    res = run_bass_kernel_spmd(nc, maps, list(range(NCORES)))
    depth = np.zeros((NPIX, DD), np.float32)
    feat = np.zeros((NPIX, CIMG), np.float32)
    for c in range(NCORES):
        r = res.results[c]
        for s, (cam, h0) in enumerate([SEG_A[c], SEG_B[c]]):
            S = SEGS[s]
            npix = S["nout"] * FW
            base = (cam * FH + h0) * FW
            a0, pcs = (0, 11) if s == 0 else (11, 6)
            dsg = r["out_depth"][:, a0:a0 + pcs].transpose(1, 0, 2)
            depth[base:base + npix] = dsg.reshape(pcs * 128, DD)[:npix]
            fsg = r["out_feat"][:, a0:a0 + pcs].transpose(1, 0, 2)
            feat[base:base + npix] = fsg.reshape(pcs * 128, CIMG)[:npix]
    return depth, feat


def _build_chunks(flat, kept, depth_rows):
    """Group points by (camera, column-block); per group build the
    [pix, voxel-slot] depth-weight matrix over the group's voxel union.
    Splits column blocks whose union exceeds the PSUM window (512)."""
    fl = flat.reshape(N, DD, FH, FW)
    kp = kept.reshape(N, DD, FH, FW)
    chunks = []                      # (pix_ids, Wdense[npix, nv], vox_ids)

    def add_group(n, w0, w1):
        nw = w1 - w0
        f = fl[n, :, :, w0:w1]                       # [DD, FH, nw]
        k = kp[n, :, :, w0:w1]
        vids = np.unique(f[k])
        if len(vids) > 512 and nw > 1:
            mid = w0 + nw // 2
            add_group(n, w0, mid)
            add_group(n, mid, w1)
            return
        nv = max(len(vids), 1)
        # pixel local idx = (w - w0) * FH + h; point (d, h, w)
        slot = np.searchsorted(vids, f[k]) if len(vids) else np.zeros(0, np.int64)
        dd, hh, ww = np.nonzero(k)
        pix_loc = ww * FH + hh
        pixcol = n * FH * FW + hh * FW + (ww + w0)
        dep = depth_rows[pixcol, dd]
        Wd = np.bincount(pix_loc * nv + slot, weights=dep,
                         minlength=nw * FH * nv).reshape(nw * FH, nv)
        pix_ids = (n * FH * FW + np.arange(FH)[None, :] * FW
                   + (w0 + np.arange(nw))[:, None]).reshape(-1)
        chunks.append((pix_ids, Wd, vids))

    for n in range(N):
        for w0 in range(0, FW, 4):
            add_group(n, w0, w0 + 4)
    return chunks


def _prep_b_inputs(chunks, featflat_bf):
    """Balance chunks across cores by window size; build per-core maps with
    the packed per-slot layout (chunk k size = max over cores, desc-sorted)."""
    order = sorted(range(len(chunks)), key=lambda i: -chunks[i][1].shape[1])
    load = np.zeros(NCORES, np.int64)
    per_core = [[] for _ in range(NCORES)]
    for i in order:
        c = int(np.argmin(load))
        per_core[c].append(i)
        load[c] += chunks[i][1].shape[1]
    NCH = max(len(p) for p in per_core)
    sizes = np.zeros(NCH, np.int64)
    for p in per_core:
        for k, i in enumerate(p):
            sizes[k] = max(sizes[k], chunks[i][1].shape[1])
    sizes = (sizes + 15) // 16 * 16
    offs = np.concatenate([[0], np.cumsum(sizes)]).astype(int)
    S = int(offs[-1])
    maps, scatter = [], []
    for c in range(NCORES):
        wm = np.zeros((128, S), bf16)
        ft = np.zeros((128, NCH, CIMG), bf16)
        sc = []
        for k, i in enumerate(per_core[c]):
            pix_ids, Wd, vids = chunks[i]
            npix, nv = Wd.shape
            wm[0:npix, offs[k]:offs[k] + nv] = Wd
            ft[0:npix, k, :] = featflat_bf[pix_ids]
            sc.append((int(offs[k]), vids))
        maps.append(dict(wmat=wm, feats=ft))
        scatter.append(sc)
    return maps, scatter, tuple(int(s) for s in sizes)


def _prep_c_inputs(inputs, pooled_t):
    """pooled_t: [CIMG, 360, 360] f32 -> per-core slabs + masks + weights."""
    NR1 = C_OUT_ROWS + 2
    NRP = 2 * NR1 + 1
    w1 = np.asarray(inputs["ds1_w"], np.float32)
    w2 = np.asarray(inputs["ds2_w"], np.float32)
    wd1 = np.stack([w1[:, :, ky, kx].T for ky in range(3) for kx in range(3)])
    wd2 = np.stack([w2[:, :, ky, kx].T for ky in range(3) for kx in range(3)])
    sb1 = np.stack([np.asarray(inputs["ds1_s"], np.float32),
                    np.asarray(inputs["ds1_t"], np.float32)], 1)
    sb2 = np.stack([np.asarray(inputs["ds2_s"], np.float32),
                    np.asarray(inputs["ds2_t"], np.float32)], 1)
    shared = dict(wd1=wd1.astype(bf16), wd2=wd2.astype(bf16), sb1=sb1, sb2=sb2)
    maps = []
    pt_bf = pooled_t.astype(bf16)
    for c in range(NCORES):
        o0g = C_OUT_ROWS * c
        p0 = 2 * o0g - 3
        slab = np.zeros((CIMG, NRP, 362), bf16)
        lo, hi = max(0, p0), min(NX, p0 + NRP)
        if hi > lo:
            slab[:, lo - p0:hi - p0, 1:361] = pt_bf[:, lo:hi, :]
        t1g = np.arange(NR1) + (o0g - 1)
        m1 = np.broadcast_to(((t1g >= 0) & (t1g < 180))[None, :],
                             (128, NR1)).astype(bf16)
        maps.append(dict(shared, slab=slab, m1=np.ascontiguousarray(m1)))
    return maps


def kernel(**inputs):
    inputs = {k: np.asarray(v) for k, v in inputs.items()}
    flat, kept = _host_geometry(inputs["cam2lidar_rots"],
                                inputs["cam2lidar_trans"], inputs["intrins"],
                                inputs["post_rots"], inputs["post_trans"])
    depth_rows, feat_rows = run_launch_a(inputs)
    featflat_bf = feat_rows.astype(bf16)

    chunks = _build_chunks(flat, kept, depth_rows)
    bmaps, scatter, sizes = _prep_b_inputs(chunks, featflat_bf)
    key = ("B", sizes)
    if key not in _CACHE:
        _CACHE[key] = build_launch_b(sizes)
    res_b = run_bass_kernel_spmd(_CACHE[key], bmaps, list(range(NCORES)))

    allvox = np.concatenate([vids for c in range(NCORES)
                             for _, vids in scatter[c]])
    allval = np.concatenate(
        [res_b.results[c]["owin"][:, o0:o0 + len(vids)].T.astype(np.float32)
         for c in range(NCORES) for o0, vids in scatter[c]])
    o = np.argsort(allvox, kind="stable")
    allvox, allval = allvox[o], allval[o]
    starts = np.flatnonzero(np.r_[True, allvox[1:] != allvox[:-1]])
    pooled = np.zeros((NX * NX, CIMG), np.float32)
    pooled[allvox[starts]] = np.add.reduceat(allval, starts, axis=0)
    pooled_t = np.ascontiguousarray(
        pooled.reshape(NX, NX, CIMG).transpose(2, 0, 1))

    if "C" not in _CACHE:
        _CACHE["C"] = build_launch_c()
    cmaps = _prep_c_inputs(inputs, pooled_t)
    res_c = run_bass_kernel_spmd(_CACHE["C"], cmaps, list(range(NCORES)))
    out = np.zeros((1, CIMG, 180, 180), np.float32)
    for c in range(NCORES):
        o0g = C_OUT_ROWS * c
        nr = min(C_OUT_ROWS, 180 - o0g)
        if nr > 0:
            out[0, :, o0g:o0g + nr, :] = res_c.results[c]["yout"][:, 0:nr, :]
    return out



# revision 72
# speedup vs baseline: 1.0049x; 1.0049x over previous
"""DepthLSSTransform Trainium kernel: 3 SPMD launches over 8 NeuronCores.

Launch A: per-camera conv pipeline (dtransform + depthnet + softmax) on
          24-row bands (one 16-row + one 8-row segment per core).
Launch B: bev_pool segment-sum via one-hot matmuls over a host-built
          virtual-window schedule (sorted-by-voxel points).
Launch C: BEV downsample convs, spatially sharded.
Host: geometry/voxel indices, scheduling, gathers, folds (orchestration).
"""
import numpy as np
import ml_dtypes

import concourse.bass as bass
import concourse.tile as tile
from concourse import bacc, mybir
from concourse.bass_utils import run_bass_kernel_spmd

dt = mybir.dt
bf16 = ml_dtypes.bfloat16

# ---- problem constants (hardcoded per contract) ----
B, N = 1, 6
CIN, CIMG, DD = 256, 80, 59
FH, FW, IH, IW = 32, 88, 256, 704
XY0, DXY, NX = -54.0, 0.3, 360
Z0, DZ, NZ = -10.0, 20.0, 1
NPTS = N * DD * FH * FW
NPIX = N * FH * FW
NCORES = 8
QV = 4                      # chunks of 128 points per virtual window

# per-core segments: (camera, h0) for seg A (16 rows) and seg B (8 rows)
SEG_A = [(0, 0), (1, 0), (1, 16), (2, 16), (3, 0), (4, 0), (4, 16), (5, 16)]
SEG_B = [(0, 16), (0, 24), (2, 0), (2, 8), (3, 16), (3, 24), (5, 0), (5, 8)]
# band pixel ranges in global row order (row = n*32 + h)
ROWS_OF_CORE = [[(SEG_A[c][0] * FH + SEG_A[c][1] + r) for r in range(16)] +
                [(SEG_B[c][0] * FH + SEG_B[c][1] + r) for r in range(8)]
                for c in range(NCORES)]

# segment geometry: rows16 segment: d rows [8h0-34, 8h0+158) (192), dt2 out
# rows [2h0-8, 2h0+39) (47), dt3 [h0-3, h0+19) (22), dn1 [h0-1, h0+17) (18)
SEGS = [dict(nout=16, nd=192, nq=48, nt2=47, nt3=22, nn1=18),
        dict(nout=8, nd=128, nq=32, nt2=31, nt3=14, nn1=10)]


def _seg_ranges(h0, S):
    return dict(d0=8 * h0 - 34, q0=2 * h0 - 8, t0=h0 - 3, r0=h0 - 1, o0=h0)


# ---------------------------------------------------------------- launch A
def build_launch_a(debug=False, psum_bufs=3, work_bufs=3, stages=9):
    nc = bacc.Bacc("TRN2", target_bir_lowering=False, debug=False,
                   num_devices=NCORES)
    AP = {}

    def inp(name, shape, dtype=dt.bfloat16):
        AP[name] = nc.dram_tensor(name, shape, dtype, kind="ExternalInput").ap()
        return AP[name]

    # per segment inputs (s = 0: 16-row, 1: 8-row); flat free dims so DMAs
    # are single-descriptor-per-partition and tile deps stay precise
    for s, S in enumerate(SEGS):
        inp(f"dph{s}", [128, S["nq"] * 177])
        inp(f"masks{s}", [128, S["nt2"] + S["nt3"] + S["nn1"]])
        inp(f"xseg{s}", [2, 128, S["nt3"] * 92])        # x_img slice (padded)
    # packed f32 constants: [alpha, beta, s_dt2, t_dt2, s_dt3, t_dt3,
    #  s_dn1(2), t_dn1(2), s_dn2(2), t_dn2(2), b_dn3(139)] -> [128, 153]
    inp("consts", [128, 153], dt.float32)
    # conv weights (host-prepped layouts)
    inp("w_dt2", [4, 128, 32])                          # groups (dky,dmx)
    inp("w_dt3", [9, 128, 64])
    inp("w_dn1", [9, 3, 128, 256])                      # tap, icchunk(128,128,64pad) -> 256
    inp("w_dn2", [9, 2, 128, 256])
    inp("w_dn3", [2, 128, 139])

    DBG = {}
    dbg_specs = [] if not debug else [("dbg_t1", [128, SEGS[0]["nq"], 177], dt.bfloat16),
                        ("dbg_dt2o", [32, SEGS[0]["nt2"] + 1, 180], dt.bfloat16),
                        ("dbg_dtc", [64, SEGS[0]["nt3"], 92], dt.bfloat16),
                        ("dbg_n1o", [128, SEGS[0]["nn1"], 92], dt.bfloat16),
                        ("dbg_n2o", [128, SEGS[0]["nout"], 88], dt.bfloat16)]
    for nm, sh, dty in dbg_specs:
        DBG[nm] = nc.dram_tensor(nm, sh, dty, kind="ExternalOutput").ap()
    # chunk-major outputs: pixel (a*128+p) of segment s at [p, a0_s + a, :]
    out_depth = nc.dram_tensor("out_depth", [128, 17, DD], dt.float32,
                               kind="ExternalOutput").ap()
    out_feat = nc.dram_tensor("out_feat", [128, 17, CIMG], dt.bfloat16,
                              kind="ExternalOutput").ap()

    # HBM scratch, phase-major: [c32, a2, b2, q', x90] (q' = dt2-row // 2)
    scr = {}
    for s, S in enumerate(SEGS):
        scr[f"dt2o{s}"] = nc.dram_tensor(
            f"dt2o{s}", [32, 2, 2, (S["nt2"] + 1) // 2, 90], dt.bfloat16).ap()

    RELU = mybir.ActivationFunctionType.Relu
    with tile.TileContext(nc) as tc:
        with tc.tile_pool(name="const", bufs=1) as cpool, \
             tc.tile_pool(name="work", bufs=work_bufs) as wpool, \
             tc.tile_pool(name="big", bufs=1) as bpool, \
             tc.tile_pool(name="psum", bufs=2, space="PSUM") as ppool, \
             tc.tile_pool(name="psum2", bufs=4, space="PSUM") as ppool2:
            # ---- DMA issue order = consumption order (the SP queue and the
            # modeled DMA engines serialize; early-stage inputs must land first)
            cts = cpool.tile([128, 153], dt.float32, name="cts")
            nc.sync.dma_start(out=cts[:], in_=AP["consts"])
            # tiny activation right away so the act-table load happens while
            # the first dph chunk is still in flight
            warm = wpool.tile([128, 1], dt.float32, tag="warm", name="warm")
            nc.scalar.activation(warm[:], cts[:, 0:1], RELU)
            ct = {"dt1_alpha": cts[:, 0:1], "dt1_beta": cts[:, 1:2],
                  "s_dt2": cts[:, 2:3], "t_dt2": cts[:, 3:4],
                  "s_dt3": cts[:, 4:5], "t_dt3": cts[:, 5:6],
                  "s_dn1": cts[:, 6:8], "t_dn1": cts[:, 8:10],
                  "s_dn2": cts[:, 10:12], "t_dn2": cts[:, 12:14],
                  "b_dn3": cts[:, 14:153]}
            wt = {}

            def load_w(nm, pat):
                sh = list(AP[nm].shape)
                wt[nm] = cpool.tile([sh[-2], int(np.prod(sh[:-2])), sh[-1]],
                                    dt.bfloat16, tag=nm, name=f'wt_{nm}')
                nc.sync.dma_start(out=wt[nm][:], in_=AP[nm].rearrange(pat))

            # first dph chunk small so dt2 starts ASAP; host has already
            # applied dt1 (relu(alpha*d+beta), pads zeroed) into dph.
            # The big dn-weights are issued later (stage_wload) so they don't
            # sit ahead of the dt2->dt3 scratch roundtrip in the serial DMA
            # stream.
            QCHUNKS = {0: [6, 14, 14, 14], 1: [6, 13, 13]}
            dphs, malls = {}, {}

            def load_dph(s):
                S = SEGS[s]
                nq = S["nq"]
                dphs[s] = bpool.tile([128, nq * 177], dt.bfloat16,
                                     tag=f"dph{s}", name=f"dph{s}")
                qq = 0
                for nqq in QCHUNKS[s]:
                    nc.sync.dma_start(
                        out=dphs[s][:, qq * 177:(qq + nqq) * 177],
                        in_=AP[f"dph{s}"][:, qq * 177:(qq + nqq) * 177])
                    qq += nqq
                malls[s] = wpool.tile([128, S["nt2"] + S["nt3"] + S["nn1"]],
                                      dt.bfloat16, tag=f"msk{s}", name="mall")
                nc.sync.dma_start(out=malls[s][:], in_=AP[f"masks{s}"])

            load_w("w_dt2", "g p o -> p g o")
            load_dph(0)
            load_dph(1)
            load_w("w_dt3", "g p o -> p g o")

            def stage_wload():
                load_w("w_dn1", "t i p o -> p (t i) o")
                load_w("w_dn2", "t i p o -> p (t i) o")
                load_w("w_dn3", "g p o -> p g o")

            feat_sb = {}
            depth_sb = {}
            st = {s: {} for s in range(len(SEGS))}

            def stage_dt2(s):
                S = SEGS[s]
                nt2, t1, mall = S["nt2"], dphs[s], malls[s]
                Q2 = (nt2 + 1) // 2
                # phase-major layout [c32, a2, b2, q', x90]: row q=(2q'+a),
                # col c at (b=c%2, x=c//2+1); makes scr write + ph3 reads
                # fully contiguous per partition
                o2 = bpool.tile([32, 2, 2, Q2, 90], dt.bfloat16, tag=f"o2{s}",
                                name=f"o2{s}")
                st[s]["o2"] = o2
                o2f = o2.rearrange("p a b q x -> p (a b q) x")
                nc.vector.memset(o2f[:, :, 0:1], 0.0)          # x pad left
                nc.vector.memset(o2f[:, :, 89:90], 0.0)        # x pad right
                nc.vector.memset(o2[:, 1, :, Q2 - 1, :], 0.0)  # pad row q=nt2
                m2 = bass.AP(mall.tensor, mall.offset, [mall.ap[0], [1, nt2]])
                RPP2 = 2
                for q0 in range(0, nt2, RPP2):
                    nr = min(RPP2, nt2 - q0)
                    ps = ppool2.tile([32, nr, 176], dt.float32, tag="ps2",
                                     name="ps2")
                    gi = 0
                    for dky in range(2):
                        for dmx in range(2):
                            g = dky * 2 + dmx
                            rhs = bass.AP(
                                t1.tensor, t1.offset + (q0 + dky) * 177 + dmx,
                                [t1.ap[0], [177, nr], [1, 176]])
                            nc.tensor.matmul(ps[:], wt["w_dt2"][:, g, :], rhs,
                                             start=(gi == 0), stop=(gi == 3))
                            gi += 1
                    ev = wpool.tile([32, nr, 176], dt.bfloat16, tag=f"ev2{s}")
                    nc.scalar.activation(ev[:], ps[:], RELU,
                                         bias=ct["t_dt2"][0:32, 0:1],
                                         scale=ct["s_dt2"][0:32, 0:1])
                    mbb = bass.AP(m2.tensor, m2.offset + q0,
                                  [[m2.ap[0][0], 32], [1, nr], [0, 176]])
                    # rows (q0, q0+1) -> a=(0,1) at q'=q0//2; c -> (x, b)
                    o2dst = bass.AP(o2.tensor,
                                    o2.offset + (q0 // 2) * 90 + 1,
                                    [[o2.ap[0][0], 32], [2 * Q2 * 90, nr],
                                     [1, 88], [Q2 * 90, 2]])
                    nc.vector.tensor_tensor(out=o2dst, in0=ev[:], in1=mbb,
                                            op=mybir.AluOpType.mult)

            def scr_write(s):
                # on the idle GPSIMD (SWDGE) queue: its sem wait must not
                # head-of-line-block the streaming SP DMA queue
                nc.gpsimd.dma_start(out=scr[f"dt2o{s}"], in_=st[s]["o2"][:])

            def stage_dt3(s):
                S = SEGS[s]
                nt2, nt3, mall = S["nt2"], S["nt3"], malls[s]
                Q2 = (nt2 + 1) // 2
                nry3 = nt3 + 2
                ph3 = bpool.tile([128, nry3 * 90], dt.bfloat16, tag=f"ph3{s}",
                                 name=f"ph3{s}")
                sd2 = scr[f"dt2o{s}"]
                for g in range(4):
                    pap3 = bass.AP(sd2.tensor, sd2.offset + g * Q2 * 90,
                                   [[4 * Q2 * 90, 32], [1, nry3 * 90]])
                    nc.gpsimd.dma_start(out=ph3[g * 32:(g + 1) * 32, :],
                                        in_=pap3)
                # concat input tile: [64 dt3 | pad] plus x_img tiles
                dtc = bpool.tile([64, nt3, 92], dt.bfloat16, tag=f"dtc{s}",
                                 name=f"dtc{s}")
                st[s]["dtc"] = dtc
                nc.vector.memset(dtc[:, :, 0:2], 0.0)
                nc.vector.memset(dtc[:, :, 90:92], 0.0)
                m3 = bass.AP(mall.tensor, mall.offset + nt2,
                             [mall.ap[0], [1, nt3]])
                RPP3 = 4
                for t0 in range(0, nt3, RPP3):
                    nr = min(RPP3, nt3 - t0)
                    ps = ppool.tile([64, nr, 88], dt.float32, tag=f"ps{s}")
                    gi = 0
                    for dky in range(3):
                        for dmx in range(3):
                            g = dky * 3 + dmx
                            rhs = bass.AP(ph3.tensor,
                                          ph3.offset + (t0 + dky) * 90 + dmx,
                                          [ph3.ap[0], [90, nr], [1, 88]])
                            nc.tensor.matmul(ps[:], wt["w_dt3"][:, g, :], rhs,
                                             start=(gi == 0), stop=(gi == 8))
                            gi += 1
                    ev = wpool.tile([64, nr, 88], dt.bfloat16, tag=f"ev3{s}")
                    nc.scalar.activation(ev[:], ps[:], RELU,
                                         bias=ct["t_dt3"][0:64, 0:1],
                                         scale=ct["s_dt3"][0:64, 0:1])
                    mbb = bass.AP(m3.tensor, m3.offset + t0,
                                  [m3.ap[0], [1, nr], [0, 88]])
                    nc.vector.tensor_tensor(out=dtc[:, t0:t0 + nr, 2:90],
                                            in0=ev[:], in1=mbb[0:64],
                                            op=mybir.AluOpType.mult)

            def stage_xload(s):
                S = SEGS[s]
                xs = []
                for g in range(2):
                    xt = bpool.tile([128, S["nt3"] * 92], dt.bfloat16,
                                    tag=f"x{g}_{s}", name=f"xseg_t{g}")
                    nc.sync.dma_start(out=xt[:], in_=AP[f"xseg{s}"][g])
                    xs.append(xt)
                st[s]["xs"] = xs

            def stage_dn1(s):
                S = SEGS[s]
                nt2, nt3, nn1 = S["nt2"], S["nt3"], S["nn1"]
                mall, dtc, xs = malls[s], st[s]["dtc"], st[s]["xs"]
                mn1 = bass.AP(mall.tensor, mall.offset + nt2 + nt3,
                              [mall.ap[0], [1, nn1]])
                n1o = []
                for g in range(2):
                    t = bpool.tile([128, nn1, 92], dt.bfloat16,
                                   tag=f"n1o{g}_{s}", name=f"n1o{g}_{s}")
                    nc.vector.memset(t[:, :, 0:2], 0.0)
                    nc.vector.memset(t[:, :, 90:92], 0.0)
                    n1o.append(t)
                st[s]["n1o"] = n1o
                RPP = 5
                for ocg in range(2):
                    for r0 in range(0, nn1, RPP):
                        nr = min(RPP, nn1 - r0)
                        ps = ppool.tile([128, nr, 88], dt.float32, tag=f"ps{s}")
                        gi = 0
                        for ky in range(3):
                            for kx in range(3):
                                tap = ky * 3 + kx
                                for icc, srcT in enumerate((xs[0], xs[1], dtc)):
                                    kk = 128 if icc < 2 else 64
                                    rhs = bass.AP(
                                        srcT.tensor,
                                        srcT.offset + (r0 + ky + 1) * 92 + kx + 1,
                                        [srcT.ap[0], [92, nr], [1, 88]])
                                    lhs = wt["w_dn1"][0:kk, tap * 3 + icc,
                                                      ocg * 128:(ocg + 1) * 128]
                                    nc.tensor.matmul(ps[:], lhs, rhs,
                                                     start=(gi == 0),
                                                     stop=(gi == 26))
                                    gi += 1
                        ev = wpool.tile([128, nr, 88], dt.bfloat16, tag=f"evn1{s}")
                        nc.scalar.activation(ev[:], ps[:], RELU,
                                             bias=ct["t_dn1"][:, ocg:ocg + 1],
                                             scale=ct["s_dn1"][:, ocg:ocg + 1])
                        mbb = bass.AP(mn1.tensor, mn1.offset + r0,
                                      [mn1.ap[0], [1, nr], [0, 88]])
                        nc.vector.tensor_tensor(
                            out=n1o[ocg][:, r0:r0 + nr, 2:90],
                            in0=ev[:], in1=mbb, op=mybir.AluOpType.mult)

            def stage_dn2(s):
                S = SEGS[s]
                nout, n1o = S["nout"], st[s]["n1o"]
                RPP = 5
                n2o = []
                for g in range(2):
                    n2o.append(bpool.tile([128, nout, 88], dt.bfloat16,
                                          tag=f"n2o{g}_{s}", name=f"n2o{g}_{s}"))
                st[s]["n2o"] = n2o
                dn3 = stage_dn3(s)
                next(dn3)                        # prime: allocates out tiles
                for r0 in range(0, nout, RPP):
                    nr = min(RPP, nout - r0)
                    for ocg in range(2):
                        ps = ppool.tile([128, nr, 88], dt.float32, tag=f"ps{s}")
                        gi = 0
                        for ky in range(3):
                            for kx in range(3):
                                tap = ky * 3 + kx
                                for icc in range(2):
                                    rhs = bass.AP(
                                        n1o[icc].tensor,
                                        n1o[icc].offset + (r0 + ky) * 92 + kx + 1,
                                        [n1o[icc].ap[0], [92, nr], [1, 88]])
                                    lhs = wt["w_dn2"][:, tap * 2 + icc,
                                                      ocg * 128:(ocg + 1) * 128]
                                    nc.tensor.matmul(ps[:], lhs, rhs,
                                                     start=(gi == 0),
                                                     stop=(gi == 17))
                                    gi += 1
                        ev = wpool.tile([128, nr, 88], dt.bfloat16, tag=f"evn2{s}")
                        nc.scalar.activation(ev[:], ps[:], RELU,
                                             bias=ct["t_dn2"][:, ocg:ocg + 1],
                                             scale=ct["s_dn2"][:, ocg:ocg + 1])
                        nc.vector.tensor_copy(n2o[ocg][:, r0:r0 + nr, :], ev[:])
                    try:
                        dn3.send(r0 + nr)        # emit dn3 chunks now ready
                    except StopIteration:
                        pass

            def stage_dn3(s):
                """Generator: receives the count of completed dn2 rows and
                emits dn3+softmax for pixel chunks whose rows are ready."""
                S = SEGS[s]
                nout, n2o = S["nout"], st[s]["n2o"]
                npix = nout * FW
                feat_sb[s] = bpool.tile([128, ((npix + 127) // 128) * CIMG],
                                        dt.bfloat16, tag=f"feat{s}", name=f"feat_sb{s}")
                depth_sb[s] = bpool.tile([128, ((npix + 127) // 128) * DD],
                                         dt.float32, tag=f"depth{s}", name=f"depth_sb{s}")
                n2f = [t.rearrange("p a b -> p (a b)") for t in n2o]
                a0 = 0 if s == 0 else 11
                pcs = (npix + 127) // 128
                rows_done = yield
                for pc in range(pcs):
                    if pc == pcs - 1:
                        # flush all-but-last chunk now so only the final
                        # chunk's output DMA sits in the tail
                        dsl = bass.AP(out_depth.tensor,
                                      out_depth.offset + a0 * DD,
                                      [[17 * DD, 128], [1, (pcs - 1) * DD]])
                        nc.sync.dma_start(
                            out=dsl, in_=depth_sb[s][:, 0:(pcs - 1) * DD])
                        fsl = bass.AP(out_feat.tensor,
                                      out_feat.offset + a0 * CIMG,
                                      [[17 * CIMG, 128], [1, (pcs - 1) * CIMG]])
                        nc.sync.dma_start(
                            out=fsl, in_=feat_sb[s][:, 0:(pcs - 1) * CIMG])
                    m = min(128, npix - pc * 128)
                    # rows needed by pixels [pc*128, pc*128+m)
                    need = (pc * 128 + m - 1) // FW + 1
                    while rows_done < need:
                        rows_done = yield
                    ps = ppool.tile([m, 139], dt.float32, tag=f"ps{s}")
                    for icc in range(2):
                        nc.tensor.matmul(ps[:], n2f[icc][:, pc * 128:pc * 128 + m],
                                         wt["w_dn3"][:, icc, :],
                                         start=(icc == 0), stop=(icc == 1))
                    # add bias via vector then softmax over first 59
                    lg = wpool.tile([m, 139], dt.float32, tag=f"lg{s}")
                    nc.vector.tensor_tensor(out=lg[:], in0=ps[:],
                                            in1=ct["b_dn3"][0:m],
                                            op=mybir.AluOpType.add)
                    mx = wpool.tile([m, 1], dt.float32, tag=f"mx{s}")
                    nc.vector.reduce_max(mx[:], lg[:, 0:DD],
                                         axis=mybir.AxisListType.X, negate=True)
                    ex = wpool.tile([m, DD], dt.float32, tag=f"ex{s}")
                    nc.scalar.activation(ex[:], lg[:, 0:DD],
                                         mybir.ActivationFunctionType.Exp,
                                         bias=mx[:, 0:1], scale=1.0)
                    sm = wpool.tile([m, 1], dt.float32, tag=f"sm{s}")
                    nc.vector.reduce_sum(sm[:], ex[:], axis=mybir.AxisListType.X)
                    rc = wpool.tile([m, 1], dt.float32, tag=f"rc{s}")
                    nc.vector.reciprocal(rc[:], sm[:])
                    nc.vector.tensor_scalar(
                        out=depth_sb[s][0:m, pc * DD:(pc + 1) * DD], in0=ex[:],
                        scalar1=rc[:, 0:1], scalar2=None,
                        op0=mybir.AluOpType.mult)
                    nc.vector.tensor_copy(
                        feat_sb[s][0:m, pc * CIMG:(pc + 1) * CIMG],
                        lg[:, DD:DD + CIMG])

                # final chunk's outputs
                dsl = bass.AP(out_depth.tensor,
                              out_depth.offset + (a0 + pcs - 1) * DD,
                              [[17 * DD, 128], [1, DD]])
                nc.sync.dma_start(out=dsl,
                                  in_=depth_sb[s][:, (pcs - 1) * DD:pcs * DD])
                fsl = bass.AP(out_feat.tensor,
                              out_feat.offset + (a0 + pcs - 1) * CIMG,
                              [[17 * CIMG, 128], [1, CIMG]])
                nc.sync.dma_start(out=fsl,
                                  in_=feat_sb[s][:, (pcs - 1) * CIMG:pcs * CIMG])

            # schedule: dt1 is folded into the host's dph prep; dt2(1)/dt3(0)
            # hide the scr roundtrips; dn3 is fused into dn2 so softmax
            # pipelines under matmuls
            stage_dt2(0)
            scr_write(0)
            stage_dt2(1)
            stage_dt3(0)
            stage_xload(0)
            scr_write(1)
            stage_dt3(1)
            stage_xload(1)
            stage_wload()
            stage_dn1(0)
            stage_dn1(1)
            stage_dn2(0)
            stage_dn2(1)
    nc.compile()
    return nc


# ------------------------------------------------------------ host helpers
def _host_geometry(rots, trans, intr, post_rots, post_trans):
    import jax
    import jax.numpy as jnp
    with jax.default_device(jax.devices("cpu")[0]):
        f32 = jnp.float32
        ds = jnp.arange(1.0, 60.0, 1.0, dtype=f32)
        xs = jnp.linspace(0.0, IW - 1.0, FW, dtype=f32)
        ys = jnp.linspace(0.0, IH - 1.0, FH, dtype=f32)
        dm = jnp.broadcast_to(ds[:, None, None], (DD, FH, FW))
        xm = jnp.broadcast_to(xs[None, None, :], (DD, FH, FW))
        ym = jnp.broadcast_to(ys[None, :, None], (DD, FH, FW))
        fr = jnp.stack([xm, ym, dm], -1)
        pts = fr[None, None] - jnp.asarray(post_trans)[:, :, None, None, None, :]
        pts = jnp.einsum("bnij,bndhwj->bndhwi",
                         jnp.linalg.inv(jnp.asarray(post_rots)), pts)
        pts = jnp.concatenate([pts[..., :2] * pts[..., 2:3], pts[..., 2:3]], -1)
        comb = jnp.einsum("bnij,bnjk->bnik", jnp.asarray(rots),
                          jnp.linalg.inv(jnp.asarray(intr)))
        pts = jnp.einsum("bnij,bndhwj->bndhwi", comb, pts) \
            + jnp.asarray(trans)[:, :, None, None, None, :]
        lo = jnp.array([XY0, XY0, Z0], dtype=f32)
        dxv = jnp.array([DXY, DXY, DZ], dtype=f32)
        g = ((pts - lo) / dxv).astype(jnp.int32).reshape(-1, 3)
        kept = ((g[:, 0] >= 0) & (g[:, 0] < NX) & (g[:, 1] >= 0) & (g[:, 1] < NX)
                & (g[:, 2] >= 0) & (g[:, 2] < NZ))
        flat = (g[:, 2] * NX + g[:, 0]) * NX + g[:, 1]
        return np.asarray(flat, np.int64), np.asarray(kept)


def _prep_a_inputs(inputs):
    """Build per-core input maps for launch A."""
    d = np.asarray(inputs["d"], np.float32).reshape(N, IH, IW)
    x_img = np.asarray(inputs["x_img"], np.float32)

    # dt1 folded affine: relu(alpha*d + beta), alpha = s*w, beta = s*b + t
    a1 = (inputs["dt1_s"] * inputs["dt1_w"][:, 0, 0, 0]).astype(np.float32)
    b1 = (inputs["dt1_s"] * inputs["dt1_b"] + inputs["dt1_t"]).astype(np.float32)
    cab = np.arange(128)
    dt1_alpha = a1[cab // 16][:, None]
    dt1_beta = b1[cab // 16][:, None]

    def wprep_dt2():
        w = np.asarray(inputs["dt2_w"], np.float32)      # [32,8,5,5]
        out = np.zeros((4, 128, 32), np.float32)
        for ky in range(5):
            for kx in range(5):
                a, dky = ky % 4, ky // 4
                bph, dmx = (kx + 2) % 4, (kx + 2) // 4
                g = dky * 2 + dmx
                rows = (np.arange(8)) * 16 + a * 4 + bph
                out[g, rows, :] = w[:, :, ky, kx].T
        return out.astype(bf16)

    def wprep_dt3():
        w = np.asarray(inputs["dt3_w"], np.float32)      # [64,32,5,5]
        out = np.zeros((9, 128, 64), np.float32)
        for ky in range(5):
            for kx in range(5):
                a, dky = ky % 2, ky // 2
                bph, dmx = kx % 2, (kx + 2) // 2 - 1
                g = dky * 3 + dmx
                rows = (a * 2 + bph) * 32 + np.arange(32)
                out[g, rows, :] = w[:, :, ky, kx].T
        return out.astype(bf16)

    def wprep_3x3(w, icc_sizes):
        O, I = w.shape[0], w.shape[1]
        nic = len(icc_sizes)
        out = np.zeros((9, nic, 128, O), np.float32)
        for ky in range(3):
            for kx in range(3):
                tap = ky * 3 + kx
                ic0 = 0
                for icc, sz in enumerate(icc_sizes):
                    out[tap, icc, 0:sz, :] = w[:, ic0:ic0 + sz, ky, kx].T
                    ic0 += sz
        return out.astype(bf16)

    # NOTE: dn1 input concat order is [dt3(64) | x_img(256)] in the reference;
    # our matmul chunks are (x0:128, x1:128, dt3:64) -> weight cols must match:
    w_dn1_full = np.asarray(inputs["dn1_w"], np.float32)
    w_dn1 = np.zeros((9, 3, 128, 256), np.float32)
    for ky in range(3):
        for kx in range(3):
            tap = ky * 3 + kx
            w_dn1[tap, 0, :, :] = w_dn1_full[:, 64:192, ky, kx].T
            w_dn1[tap, 1, :, :] = w_dn1_full[:, 192:320, ky, kx].T
            w_dn1[tap, 2, 0:64, :] = w_dn1_full[:, 0:64, ky, kx].T
    w_dn1 = w_dn1.astype(bf16)
    w_dn2 = wprep_3x3(np.asarray(inputs["dn2_w"], np.float32), [128, 128])
    w_dn3 = np.asarray(inputs["dn3_w"], np.float32)[:, :, 0, 0]  # [139, 256]
    w_dn3p = np.zeros((2, 128, 139), np.float32)
    w_dn3p[0] = w_dn3[:, 0:128].T
    w_dn3p[1] = w_dn3[:, 128:256].T

    def fold_bias(b, s, t):
        # conv bias b then bn scale/shift: relu(s*(x+b) + t) = relu(s*x + (s*b+t))
        return np.asarray(s, np.float32), np.asarray(s * b + t, np.float32)

    s2, t2 = fold_bias(inputs["dt2_b"], inputs["dt2_s"], inputs["dt2_t"])
    s3, t3 = fold_bias(inputs["dt3_b"], inputs["dt3_s"], inputs["dt3_t"])
    sn1, tn1 = fold_bias(inputs["dn1_b"], inputs["dn1_s"], inputs["dn1_t"])
    sn2, tn2 = fold_bias(inputs["dn2_b"], inputs["dn2_s"], inputs["dn2_t"])
    b_dn3 = np.broadcast_to(np.asarray(inputs["dn3_b"], np.float32)[None, :],
                            (128, 139)).copy()

    consts = np.zeros((128, 153), np.float32)
    consts[:, 0] = dt1_alpha[:, 0]
    consts[:, 1] = dt1_beta[:, 0]
    consts[:, 2] = np.tile(s2, 4)
    consts[:, 3] = np.tile(t2, 4)
    consts[:, 4] = np.tile(s3, 2)
    consts[:, 5] = np.tile(t3, 2)
    consts[:, 6:8] = sn1.reshape(2, 128).T
    consts[:, 8:10] = tn1.reshape(2, 128).T
    consts[:, 10:12] = sn2.reshape(2, 128).T
    consts[:, 12:14] = tn2.reshape(2, 128).T
    consts[:, 14:153] = b_dn3
    shared = dict(
        consts=consts,
        w_dt2=wprep_dt2(), w_dt3=wprep_dt3(), w_dn1=w_dn1, w_dn2=w_dn2,
        w_dn3=w_dn3p.astype(bf16),
    )

    maps = []
    for c in range(NCORES):
        m = dict(shared)
        for s, (cam, h0) in enumerate([SEG_A[c], SEG_B[c]]):
            S = SEGS[s]
            d0 = 8 * h0 - 34
            dseg = np.zeros((S["nd"], 712), np.float32)
            vseg = np.zeros((S["nd"], 712), bool)
            lo, hi = max(0, d0), min(IH, d0 + S["nd"])
            if hi > lo:
                dseg[lo - d0:hi - d0, 4:708] = d[cam, lo:hi]
                vseg[lo - d0:hi - d0, 4:708] = True
            nq = S["nq"]
            ph = dseg.reshape(nq, 4, 178, 4)[:, :, :177, :]     # ry a rx b
            ph = ph.transpose(1, 3, 0, 2)                        # a b ry rx
            vph = vseg.reshape(nq, 4, 178, 4)[:, :, :177, :].transpose(1, 3, 0, 2)
            # dt1 applied on host: relu(alpha*d + beta), zero at pads
            dphc = np.where(vph[None],
                            np.maximum(a1[:, None, None, None, None] * ph[None]
                                       + b1[:, None, None, None, None], 0.0),
                            0.0)                                 # [8,4,4,nq,177]
            m[f"dph{s}"] = dphc.reshape(128, nq * 177).astype(bf16)
            q0, t0, r0 = 2 * h0 - 8, h0 - 3, h0 - 1
            qr = np.arange(S["nt2"]) + q0
            m2m = np.broadcast_to(((qr >= 0) & (qr < 64))[None, :],
                                  (128, S["nt2"]))
            tr = np.arange(S["nt3"]) + t0
            m3m = np.broadcast_to(((tr >= 0) & (tr < FH))[None, :],
                                  (128, S["nt3"]))
            rr = np.arange(S["nn1"]) + r0
            mn1m = np.broadcast_to(((rr >= 0) & (rr < FH))[None, :],
                                   (128, S["nn1"]))
            m[f"masks{s}"] = np.concatenate(
                [m2m, m3m, mn1m], axis=1).astype(bf16)
            xseg = np.zeros((2, 128, S["nt3"], 92), np.float32)
            lo2, hi2 = max(0, t0), min(FH, t0 + S["nt3"])
            if hi2 > lo2:
                xseg[:, :, lo2 - t0:hi2 - t0, 2:90] = \
                    x_img[cam, :, lo2:hi2, :].reshape(2, 128, hi2 - lo2, FW)
            m[f"xseg{s}"] = xseg.reshape(2, 128, S["nt3"] * 92).astype(bf16)
        maps.append(m)
    return maps


# ---------------------------------------------------------------- launch B
def build_launch_b(sizes):
    """Per chunk k: [128pix x 80ch] stationary feat tile x host-built
    [128pix x sizes[k] voxel-slot] depth-weight matrix -> [80, nv] window
    sums. W and out use packed (variable-size) layouts; W loads in a few
    batched DMAs, out in one."""
    nc = bacc.Bacc("TRN2", target_bir_lowering=False, debug=False,
                   num_devices=NCORES)
    NCH = len(sizes)
    offs = np.concatenate([[0], np.cumsum(sizes)]).astype(int)
    S = int(offs[-1])
    wmat = nc.dram_tensor("wmat", [128, S], dt.bfloat16,
                          kind="ExternalInput").ap()
    feats = nc.dram_tensor("feats", [128, NCH, CIMG], dt.bfloat16,
                           kind="ExternalInput").ap()
    owin = nc.dram_tensor("owin", [CIMG, S], dt.bfloat16,
                          kind="ExternalOutput").ap()
    NB = 4                                   # W DMA batches
    bnd = [int(round(NCH * i / NB)) for i in range(NB + 1)]
    with tile.TileContext(nc) as tc:
        with tc.tile_pool(name="const", bufs=1) as cpool, \
             tc.tile_pool(name="ps", bufs=4, space="PSUM") as pp:
            ft = cpool.tile([128, NCH, CIMG], dt.bfloat16, name="ft")
            nc.sync.dma_start(out=ft[:], in_=feats)
            wt = cpool.tile([128, S], dt.bfloat16, name="wt")
            for b in range(NB):
                lo, hi = offs[bnd[b]], offs[bnd[b + 1]]
                if hi > lo:
                    nc.sync.dma_start(out=wt[:, lo:hi], in_=wmat[:, lo:hi])
            ot = cpool.tile([CIMG, S], dt.bfloat16, name="ot")
            for k in range(NCH):
                nv, o0 = int(sizes[k]), int(offs[k])
                ps = pp.tile([CIMG, 512], dt.float32, tag="ps", name="ps")
                nc.tensor.matmul(ps[:, 0:nv], ft[:, k, :], wt[:, o0:o0 + nv],
                                 start=True, stop=True)
                if k % 2 == 0:
                    nc.scalar.activation(ot[:, o0:o0 + nv], ps[:, 0:nv],
                                         mybir.ActivationFunctionType.Copy)
                else:
                    nc.vector.tensor_copy(ot[:, o0:o0 + nv], ps[:, 0:nv])
            nc.sync.dma_start(out=owin, in_=ot[:])
    nc.compile()
    return nc


# ---------------------------------------------------------------- launch C
C_OUT_ROWS = 23              # ds2-out rows per core (8*23 = 184 >= 180)


def build_launch_c():
    nc = bacc.Bacc("TRN2", target_bir_lowering=False, debug=False,
                   num_devices=NCORES)
    NR1 = C_OUT_ROWS + 2                         # ds1-out rows incl halo (25)
    NRP = 2 * NR1 + 1                            # pooled rows needed (51)
    slab = nc.dram_tensor("slab", [CIMG, NRP, 362], dt.bfloat16,
                          kind="ExternalInput").ap()
    m1 = nc.dram_tensor("m1", [128, NR1], dt.bfloat16, kind="ExternalInput").ap()
    wd1 = nc.dram_tensor("wd1", [9, CIMG, CIMG], dt.bfloat16,
                         kind="ExternalInput").ap()
    wd2 = nc.dram_tensor("wd2", [9, CIMG, CIMG], dt.bfloat16,
                         kind="ExternalInput").ap()
    sb1 = nc.dram_tensor("sb1", [CIMG, 2], dt.float32, kind="ExternalInput").ap()
    sb2 = nc.dram_tensor("sb2", [CIMG, 2], dt.float32, kind="ExternalInput").ap()
    yout = nc.dram_tensor("yout", [CIMG, C_OUT_ROWS, 180], dt.float32,
                          kind="ExternalOutput").ap()
    RELU = mybir.ActivationFunctionType.Relu
    with tile.TileContext(nc) as tc:
        with tc.tile_pool(name="const", bufs=1) as cpool,              tc.tile_pool(name="work", bufs=2) as wp,              tc.tile_pool(name="big", bufs=1) as bp,              tc.tile_pool(name="ps", bufs=3, space="PSUM") as pp:
            # weights/consts first so ds1 can start on the first slab chunk
            w1 = cpool.tile([CIMG, 9, CIMG], dt.bfloat16, name="w1")
            nc.sync.dma_start(out=w1[:], in_=wd1.rearrange("t p o -> p t o"))
            sb1t = cpool.tile([CIMG, 2], dt.float32, name="sb1t")
            nc.sync.dma_start(out=sb1t[:], in_=sb1)
            m1t = wp.tile([128, NR1], dt.bfloat16, name="m1t")
            nc.sync.dma_start(out=m1t[:], in_=m1)
            slabt = bp.tile([CIMG, NRP, 362], dt.bfloat16, name="slabt")
            for rr in range(0, NRP, 9):
                nrr = min(9, NRP - rr)
                nc.sync.dma_start(out=slabt[:, rr:rr + nrr, :],
                                  in_=slab[:, rr:rr + nrr, :])
            w2 = cpool.tile([CIMG, 9, CIMG], dt.bfloat16, name="w2")
            nc.sync.dma_start(out=w2[:], in_=wd2.rearrange("t p o -> p t o"))
            sb2t = cpool.tile([CIMG, 2], dt.float32, name="sb2t")
            nc.sync.dma_start(out=sb2t[:], in_=sb2)
            h1 = bp.tile([CIMG, NR1, 182], dt.bfloat16, name="h1")
            nc.vector.memset(h1[:, :, 0:1], 0.0)
            nc.vector.memset(h1[:, :, 181:182], 0.0)
            # ds1: stride-2 3x3; out row t reads slab rows 2t..2t+2 (slab row 0
            # = pooled row 2o0-3, so out row t (global o0-1+t) reads
            # 2(o0-1+t)-1..+1 - (2o0-3) = 2t..2t+2); col c reads 2c..2c+2
            RP = 2
            for t0 in range(0, NR1, RP):
                nr = min(RP, NR1 - t0)
                ps = pp.tile([CIMG, nr, 180], dt.float32, tag="ps1", name="ps")
                gi = 0
                for ky in range(3):
                    for kx in range(3):
                        rhs = bass.AP(slabt.tensor,
                                      slabt.offset + (2 * t0 + ky) * 362 + kx,
                                      [slabt.ap[0], [2 * 362, nr], [2, 180]])
                        nc.tensor.matmul(ps[:], w1[:, ky * 3 + kx, :], rhs,
                                         start=(gi == 0), stop=(gi == 8))
                        gi += 1
                ev = wp.tile([CIMG, nr, 180], dt.bfloat16, tag="ev", name="ev")
                nc.scalar.activation(ev[:], ps[:], RELU, bias=sb1t[:, 1:2],
                                     scale=sb1t[:, 0:1])
                mbb = bass.AP(m1t.tensor, m1t.offset + t0,
                              [[m1t.ap[0][0], CIMG], [1, nr], [0, 180]])
                nc.vector.tensor_tensor(out=h1[:, t0:t0 + nr, 1:181],
                                        in0=ev[:], in1=mbb,
                                        op=mybir.AluOpType.mult)
            # ds2: 3x3 pad 1: out row o reads h1 rows o..o+2, col c: c..c+2
            yo = bp.tile([CIMG, C_OUT_ROWS, 180], dt.float32, name="yo")
            for o0 in range(0, C_OUT_ROWS, RP):
                nr = min(RP, C_OUT_ROWS - o0)
                ps = pp.tile([CIMG, nr, 180], dt.float32, tag="ps2", name="ps")
                gi = 0
                for ky in range(3):
                    for kx in range(3):
                        rhs = bass.AP(h1.tensor,
                                      h1.offset + (o0 + ky) * 182 + kx,
                                      [h1.ap[0], [182, nr], [1, 180]])
                        nc.tensor.matmul(ps[:], w2[:, ky * 3 + kx, :], rhs,
                                         start=(gi == 0), stop=(gi == 8))
                        gi += 1
                nc.scalar.activation(yo[:, o0:o0 + nr, :], ps[:], RELU,
                                     bias=sb2t[:, 1:2], scale=sb2t[:, 0:1])
                if (o0 // RP) % 3 == 2 or o0 + nr >= C_OUT_ROWS:
                    lo = (o0 // (3 * RP)) * 3 * RP
                    nc.sync.dma_start(out=yout[:, lo:o0 + nr, :],
                                      in_=yo[:, lo:o0 + nr, :])
    nc.compile()
    return nc


_CACHE = {}


def run_launch_a(inputs):
    if "A" not in _CACHE:
        _CACHE["A"] = build_launch_a()
    nc = _CACHE["A"]
    maps = _prep_a_inputs(inputs)
    res = run_bass_kernel_spmd(nc, maps, list(range(NCORES)))
    depth = np.zeros((NPIX, DD), np.float32)
    feat = np.zeros((NPIX, CIMG), np.float32)
    for c in range(NCORES):
        r = res.results[c]
        for s, (cam, h0) in enumerate([SEG_A[c], SEG_B[c]]):
            S = SEGS[s]
            npix = S["nout"] * FW
            base = (cam * FH + h0) * FW
            a0, pcs = (0, 11) if s == 0 else (11, 6)
            dsg = r["out_depth"][:, a0:a0 + pcs].transpose(1, 0, 2)
            depth[base:base + npix] = dsg.reshape(pcs * 128, DD)[:npix]
            fsg = r["out_feat"][:, a0:a0 + pcs].transpose(1, 0, 2)
            feat[base:base + npix] = fsg.reshape(pcs * 128, CIMG)[:npix]
    return depth, feat


def _build_chunks(flat, kept, depth_rows):
    """Group points by (camera, column-block); per group build the
    [pix, voxel-slot] depth-weight matrix over the group's voxel union.
    Splits column blocks whose union exceeds the PSUM window (512)."""
    fl = flat.reshape(N, DD, FH, FW)
    kp = kept.reshape(N, DD, FH, FW)
    chunks = []                      # (pix_ids, Wdense[npix, nv], vox_ids)

    def add_group(n, w0, w1):
        nw = w1 - w0
        f = fl[n, :, :, w0:w1]                       # [DD, FH, nw]
        k = kp[n, :, :, w0:w1]
        vids = np.unique(f[k])
        if len(vids) > 512 and nw > 1:
            mid = w0 + nw // 2
            add_group(n, w0, mid)
            add_group(n, mid, w1)
            return
        nv = max(len(vids), 1)
        # pixel local idx = (w - w0) * FH + h; point (d, h, w)
        slot = np.searchsorted(vids, f[k]) if len(vids) else np.zeros(0, np.int64)
        dd, hh, ww = np.nonzero(k)
        pix_loc = ww * FH + hh
        pixcol = n * FH * FW + hh * FW + (ww + w0)
        dep = depth_rows[pixcol, dd]
        Wd = np.bincount(pix_loc * nv + slot, weights=dep,
                         minlength=nw * FH * nv).reshape(nw * FH, nv)
        pix_ids = (n * FH * FW + np.arange(FH)[None, :] * FW
                   + (w0 + np.arange(nw))[:, None]).reshape(-1)
        chunks.append((pix_ids, Wd, vids))

    for n in range(N):
        for w0 in range(0, FW, 4):
            add_group(n, w0, w0 + 4)
    return chunks


def _prep_b_inputs(chunks, featflat_bf):
    """Balance chunks across cores by window size; build per-core maps with
    the packed per-slot layout (chunk k size = max over cores, desc-sorted)."""
    order = sorted(range(len(chunks)), key=lambda i: -chunks[i][1].shape[1])
    load = np.zeros(NCORES, np.int64)
    per_core = [[] for _ in range(NCORES)]
    for i in order:
        c = int(np.argmin(load))
        per_core[c].append(i)
        load[c] += chunks[i][1].shape[1]
    NCH = max(len(p) for p in per_core)
    sizes = np.zeros(NCH, np.int64)
    for p in per_core:
        for k, i in enumerate(p):
            sizes[k] = max(sizes[k], chunks[i][1].shape[1])
    sizes = (sizes + 15) // 16 * 16
    offs = np.concatenate([[0], np.cumsum(sizes)]).astype(int)
    S = int(offs[-1])
    maps, scatter = [], []
    for c in range(NCORES):
        wm = np.zeros((128, S), bf16)
        ft = np.zeros((128, NCH, CIMG), bf16)
        sc = []
        for k, i in enumerate(per_core[c]):
            pix_ids, Wd, vids = chunks[i]
            npix, nv = Wd.shape
            wm[0:npix, offs[k]:offs[k] + nv] = Wd
            ft[0:npix, k, :] = featflat_bf[pix_ids]
            sc.append((int(offs[k]), vids))
        maps.append(dict(wmat=wm, feats=ft))
        scatter.append(sc)
    return maps, scatter, tuple(int(s) for s in sizes)


def _prep_c_inputs(inputs, pooled_t):
    """pooled_t: [CIMG, 360, 360] f32 -> per-core slabs + masks + weights."""
    NR1 = C_OUT_ROWS + 2
    NRP = 2 * NR1 + 1
    w1 = np.asarray(inputs["ds1_w"], np.float32)
    w2 = np.asarray(inputs["ds2_w"], np.float32)
    wd1 = np.stack([w1[:, :, ky, kx].T for ky in range(3) for kx in range(3)])
    wd2 = np.stack([w2[:, :, ky, kx].T for ky in range(3) for kx in range(3)])
    sb1 = np.stack([np.asarray(inputs["ds1_s"], np.float32),
                    np.asarray(inputs["ds1_t"], np.float32)], 1)
    sb2 = np.stack([np.asarray(inputs["ds2_s"], np.float32),
                    np.asarray(inputs["ds2_t"], np.float32)], 1)
    shared = dict(wd1=wd1.astype(bf16), wd2=wd2.astype(bf16), sb1=sb1, sb2=sb2)
    maps = []
    pt_bf = pooled_t.astype(bf16)
    for c in range(NCORES):
        o0g = C_OUT_ROWS * c
        p0 = 2 * o0g - 3
        slab = np.zeros((CIMG, NRP, 362), bf16)
        lo, hi = max(0, p0), min(NX, p0 + NRP)
        if hi > lo:
            slab[:, lo - p0:hi - p0, 1:361] = pt_bf[:, lo:hi, :]
        t1g = np.arange(NR1) + (o0g - 1)
        m1 = np.broadcast_to(((t1g >= 0) & (t1g < 180))[None, :],
                             (128, NR1)).astype(bf16)
        maps.append(dict(shared, slab=slab, m1=np.ascontiguousarray(m1)))
    return maps


def kernel(**inputs):
    inputs = {k: np.asarray(v) for k, v in inputs.items()}
    flat, kept = _host_geometry(inputs["cam2lidar_rots"],
                                inputs["cam2lidar_trans"], inputs["intrins"],
                                inputs["post_rots"], inputs["post_trans"])
    depth_rows, feat_rows = run_launch_a(inputs)
    featflat_bf = feat_rows.astype(bf16)

    chunks = _build_chunks(flat, kept, depth_rows)
    bmaps, scatter, sizes = _prep_b_inputs(chunks, featflat_bf)
    key = ("B", sizes)
    if key not in _CACHE:
        _CACHE[key] = build_launch_b(sizes)
    res_b = run_bass_kernel_spmd(_CACHE[key], bmaps, list(range(NCORES)))

    allvox = np.concatenate([vids for c in range(NCORES)
                             for _, vids in scatter[c]])
    allval = np.concatenate(
        [res_b.results[c]["owin"][:, o0:o0 + len(vids)].T.astype(np.float32)
         for c in range(NCORES) for o0, vids in scatter[c]])
    o = np.argsort(allvox, kind="stable")
    allvox, allval = allvox[o], allval[o]
    starts = np.flatnonzero(np.r_[True, allvox[1:] != allvox[:-1]])
    pooled = np.zeros((NX * NX, CIMG), np.float32)
    pooled[allvox[starts]] = np.add.reduceat(allval, starts, axis=0)
    pooled_t = np.ascontiguousarray(
        pooled.reshape(NX, NX, CIMG).transpose(2, 0, 1))

    if "C" not in _CACHE:
        _CACHE["C"] = build_launch_c()
    cmaps = _prep_c_inputs(inputs, pooled_t)
    res_c = run_bass_kernel_spmd(_CACHE["C"], cmaps, list(range(NCORES)))
    out = np.zeros((1, CIMG, 180, 180), np.float32)
    for c in range(NCORES):
        o0g = C_OUT_ROWS * c
        nr = min(C_OUT_ROWS, 180 - o0g)
        if nr > 0:
            out[0, :, o0g:o0g + nr, :] = res_c.results[c]["yout"][:, 0:nr, :]
    return out



# revision 74
# speedup vs baseline: 1.0296x; 1.0246x over previous
"""DepthLSSTransform Trainium kernel: 3 SPMD launches over 8 NeuronCores.

Launch A: per-camera conv pipeline (dtransform + depthnet + softmax) on
          24-row bands (one 16-row + one 8-row segment per core).
Launch B: bev_pool segment-sum via one-hot matmuls over a host-built
          virtual-window schedule (sorted-by-voxel points).
Launch C: BEV downsample convs, spatially sharded.
Host: geometry/voxel indices, scheduling, gathers, folds (orchestration).
"""
import numpy as np
import ml_dtypes

import concourse.bass as bass
import concourse.tile as tile
from concourse import bacc, mybir
from concourse.bass_utils import run_bass_kernel_spmd

dt = mybir.dt
bf16 = ml_dtypes.bfloat16

# ---- problem constants (hardcoded per contract) ----
B, N = 1, 6
CIN, CIMG, DD = 256, 80, 59
FH, FW, IH, IW = 32, 88, 256, 704
XY0, DXY, NX = -54.0, 0.3, 360
Z0, DZ, NZ = -10.0, 20.0, 1
NPTS = N * DD * FH * FW
NPIX = N * FH * FW
NCORES = 8
QV = 4                      # chunks of 128 points per virtual window

# per-core segments: (camera, h0) for seg A (16 rows) and seg B (8 rows)
SEG_A = [(0, 0), (1, 0), (1, 16), (2, 16), (3, 0), (4, 0), (4, 16), (5, 16)]
SEG_B = [(0, 16), (0, 24), (2, 0), (2, 8), (3, 16), (3, 24), (5, 0), (5, 8)]
# band pixel ranges in global row order (row = n*32 + h)
ROWS_OF_CORE = [[(SEG_A[c][0] * FH + SEG_A[c][1] + r) for r in range(16)] +
                [(SEG_B[c][0] * FH + SEG_B[c][1] + r) for r in range(8)]
                for c in range(NCORES)]

# segment geometry: rows16 segment: d rows [8h0-34, 8h0+158) (192), dt2 out
# rows [2h0-8, 2h0+39) (47), dt3 [h0-3, h0+19) (22), dn1 [h0-1, h0+17) (18)
SEGS = [dict(nout=16, nd=192, nq=48, nt2=47, nt3=22, nn1=18),
        dict(nout=8, nd=128, nq=32, nt2=31, nt3=14, nn1=10)]


def _seg_ranges(h0, S):
    return dict(d0=8 * h0 - 34, q0=2 * h0 - 8, t0=h0 - 3, r0=h0 - 1, o0=h0)


# ---------------------------------------------------------------- launch A
def build_launch_a(debug=False, psum_bufs=3, work_bufs=3, stages=9):
    nc = bacc.Bacc("TRN2", target_bir_lowering=False, debug=False,
                   num_devices=NCORES)
    AP = {}

    def inp(name, shape, dtype=dt.bfloat16):
        AP[name] = nc.dram_tensor(name, shape, dtype, kind="ExternalInput").ap()
        return AP[name]

    # per segment inputs (s = 0: 16-row, 1: 8-row); flat free dims so DMAs
    # are single-descriptor-per-partition and tile deps stay precise
    for s, S in enumerate(SEGS):
        inp(f"dph{s}", [128, S["nq"] * 177])
        inp(f"masks{s}", [128, S["nt2"] + S["nt3"] + S["nn1"]])
        inp(f"xseg{s}", [2, 128, S["nt3"] * 92])        # x_img slice (padded)
    # packed f32 constants: [alpha, beta, s_dt2, t_dt2, s_dt3, t_dt3,
    #  s_dn1(2), t_dn1(2), s_dn2(2), t_dn2(2), b_dn3(139)] -> [128, 153]
    inp("consts", [128, 153], dt.float32)
    # conv weights (host-prepped layouts)
    inp("w_dt2", [4, 128, 32])                          # groups (dky,dmx)
    inp("w_dt3", [9, 128, 64])
    inp("w_dn1", [9, 3, 128, 256])                      # tap, icchunk(128,128,64pad) -> 256
    inp("w_dn2", [9, 2, 128, 256])
    inp("w_dn3", [2, 128, 139])

    DBG = {}
    dbg_specs = [] if not debug else [("dbg_t1", [128, SEGS[0]["nq"], 177], dt.bfloat16),
                        ("dbg_dt2o", [32, SEGS[0]["nt2"] + 1, 180], dt.bfloat16),
                        ("dbg_dtc", [64, SEGS[0]["nt3"], 92], dt.bfloat16),
                        ("dbg_n1o", [128, SEGS[0]["nn1"], 92], dt.bfloat16),
                        ("dbg_n2o", [128, SEGS[0]["nout"], 88], dt.bfloat16)]
    for nm, sh, dty in dbg_specs:
        DBG[nm] = nc.dram_tensor(nm, sh, dty, kind="ExternalOutput").ap()
    # chunk-major outputs: pixel (a*128+p) of segment s at [p, a0_s + a, :]
    out_depth = nc.dram_tensor("out_depth", [128, 17, DD], dt.float32,
                               kind="ExternalOutput").ap()
    out_feat = nc.dram_tensor("out_feat", [128, 17, CIMG], dt.bfloat16,
                              kind="ExternalOutput").ap()

    # HBM scratch, phase-major: [c32, a2, b2, q', x90] (q' = dt2-row // 2)
    scr = {}
    for s, S in enumerate(SEGS):
        scr[f"dt2o{s}"] = nc.dram_tensor(
            f"dt2o{s}", [32, 2, 2, (S["nt2"] + 1) // 2, 90], dt.bfloat16).ap()

    RELU = mybir.ActivationFunctionType.Relu
    with tile.TileContext(nc) as tc:
        with tc.tile_pool(name="const", bufs=1) as cpool, \
             tc.tile_pool(name="work", bufs=work_bufs) as wpool, \
             tc.tile_pool(name="big", bufs=1) as bpool, \
             tc.tile_pool(name="psum", bufs=2, space="PSUM") as ppool, \
             tc.tile_pool(name="psum2", bufs=4, space="PSUM") as ppool2:
            # ---- DMA issue order = consumption order (the SP queue and the
            # modeled DMA engines serialize; early-stage inputs must land first)
            cts = cpool.tile([128, 153], dt.float32, name="cts")
            nc.sync.dma_start(out=cts[:], in_=AP["consts"])
            # tiny activation right away so the act-table load happens while
            # the first dph chunk is still in flight
            warm = wpool.tile([128, 1], dt.float32, tag="warm", name="warm")
            nc.scalar.activation(warm[:], cts[:, 0:1], RELU)
            ct = {"dt1_alpha": cts[:, 0:1], "dt1_beta": cts[:, 1:2],
                  "s_dt2": cts[:, 2:3], "t_dt2": cts[:, 3:4],
                  "s_dt3": cts[:, 4:5], "t_dt3": cts[:, 5:6],
                  "s_dn1": cts[:, 6:8], "t_dn1": cts[:, 8:10],
                  "s_dn2": cts[:, 10:12], "t_dn2": cts[:, 12:14],
                  "b_dn3": cts[:, 14:153]}
            wt = {}

            def load_w(nm, pat):
                sh = list(AP[nm].shape)
                wt[nm] = cpool.tile([sh[-2], int(np.prod(sh[:-2])), sh[-1]],
                                    dt.bfloat16, tag=nm, name=f'wt_{nm}')
                nc.sync.dma_start(out=wt[nm][:], in_=AP[nm].rearrange(pat))

            # first dph chunk small so dt2 starts ASAP; host has already
            # applied dt1 (relu(alpha*d+beta), pads zeroed) into dph.
            # The big dn-weights are issued later (stage_wload) so they don't
            # sit ahead of the dt2->dt3 scratch roundtrip in the serial DMA
            # stream.
            QCHUNKS = {0: [8, 12, 14, 14], 1: [8, 12, 12]}
            dphs, malls = {}, {}

            def load_dph(s):
                S = SEGS[s]
                nq = S["nq"]
                dphs[s] = bpool.tile([128, nq * 177], dt.bfloat16,
                                     tag=f"dph{s}", name=f"dph{s}")
                qq = 0
                for nqq in QCHUNKS[s]:
                    nc.sync.dma_start(
                        out=dphs[s][:, qq * 177:(qq + nqq) * 177],
                        in_=AP[f"dph{s}"][:, qq * 177:(qq + nqq) * 177])
                    qq += nqq
                malls[s] = wpool.tile([128, S["nt2"] + S["nt3"] + S["nn1"]],
                                      dt.bfloat16, tag=f"msk{s}", name="mall")
                nc.sync.dma_start(out=malls[s][:], in_=AP[f"masks{s}"])

            load_w("w_dt2", "g p o -> p g o")
            load_dph(0)
            load_dph(1)
            load_w("w_dt3", "g p o -> p g o")

            def stage_wload():
                load_w("w_dn1", "t i p o -> p (t i) o")
                load_w("w_dn2", "t i p o -> p (t i) o")
                load_w("w_dn3", "g p o -> p g o")

            feat_sb = {}
            depth_sb = {}
            st = {s: {} for s in range(len(SEGS))}

            def stage_dt2(s):
                S = SEGS[s]
                nt2, t1, mall = S["nt2"], dphs[s], malls[s]
                Q2 = (nt2 + 1) // 2
                # phase-major layout [c32, a2, b2, q', x90]: row q=(2q'+a),
                # col c at (b=c%2, x=c//2+1); makes scr write + ph3 reads
                # fully contiguous per partition
                o2 = bpool.tile([32, 2, 2, Q2, 90], dt.bfloat16, tag=f"o2{s}",
                                name=f"o2{s}")
                st[s]["o2"] = o2
                o2f = o2.rearrange("p a b q x -> p (a b q) x")
                nc.vector.memset(o2f[:, :, 0:1], 0.0)          # x pad left
                nc.vector.memset(o2f[:, :, 89:90], 0.0)        # x pad right
                nc.vector.memset(o2[:, 1, :, Q2 - 1, :], 0.0)  # pad row q=nt2
                m2 = bass.AP(mall.tensor, mall.offset, [mall.ap[0], [1, nt2]])
                RPP2 = 2
                for q0 in range(0, nt2, RPP2):
                    nr = min(RPP2, nt2 - q0)
                    ps = ppool2.tile([32, nr, 176], dt.float32, tag="ps2",
                                     name="ps2")
                    gi = 0
                    for dky in range(2):
                        for dmx in range(2):
                            g = dky * 2 + dmx
                            rhs = bass.AP(
                                t1.tensor, t1.offset + (q0 + dky) * 177 + dmx,
                                [t1.ap[0], [177, nr], [1, 176]])
                            nc.tensor.matmul(ps[:], wt["w_dt2"][:, g, :], rhs,
                                             start=(gi == 0), stop=(gi == 3))
                            gi += 1
                    ev = wpool.tile([32, nr, 176], dt.bfloat16, tag=f"ev2{s}")
                    nc.scalar.activation(ev[:], ps[:], RELU,
                                         bias=ct["t_dt2"][0:32, 0:1],
                                         scale=ct["s_dt2"][0:32, 0:1])
                    mbb = bass.AP(m2.tensor, m2.offset + q0,
                                  [[m2.ap[0][0], 32], [1, nr], [0, 176]])
                    # rows (q0, q0+1) -> a=(0,1) at q'=q0//2; c -> (x, b)
                    o2dst = bass.AP(o2.tensor,
                                    o2.offset + (q0 // 2) * 90 + 1,
                                    [[o2.ap[0][0], 32], [2 * Q2 * 90, nr],
                                     [1, 88], [Q2 * 90, 2]])
                    nc.vector.tensor_tensor(out=o2dst, in0=ev[:], in1=mbb,
                                            op=mybir.AluOpType.mult)

            def scr_write(s):
                # on the idle GPSIMD (SWDGE) queue: its sem wait must not
                # head-of-line-block the streaming SP DMA queue
                nc.gpsimd.dma_start(out=scr[f"dt2o{s}"], in_=st[s]["o2"][:])

            def stage_dt3(s):
                S = SEGS[s]
                nt2, nt3, mall = S["nt2"], S["nt3"], malls[s]
                Q2 = (nt2 + 1) // 2
                nry3 = nt3 + 2
                ph3 = bpool.tile([128, nry3 * 90], dt.bfloat16, tag=f"ph3{s}",
                                 name=f"ph3{s}")
                sd2 = scr[f"dt2o{s}"]
                # one DMA: partition (g, c) <- scr[(c, g)] nested dims
                pap3 = bass.AP(sd2.tensor, sd2.offset,
                               [[Q2 * 90, 4], [4 * Q2 * 90, 32],
                                [1, nry3 * 90]])
                nc.gpsimd.dma_start(out=ph3[:], in_=pap3)
                # concat input tile: [64 dt3 | pad] plus x_img tiles
                dtc = bpool.tile([64, nt3, 92], dt.bfloat16, tag=f"dtc{s}",
                                 name=f"dtc{s}")
                st[s]["dtc"] = dtc
                nc.vector.memset(dtc[:, :, 0:2], 0.0)
                nc.vector.memset(dtc[:, :, 90:92], 0.0)
                m3 = bass.AP(mall.tensor, mall.offset + nt2,
                             [mall.ap[0], [1, nt3]])
                RPP3 = 4
                for t0 in range(0, nt3, RPP3):
                    nr = min(RPP3, nt3 - t0)
                    ps = ppool.tile([64, nr, 88], dt.float32, tag=f"ps{s}")
                    gi = 0
                    for dky in range(3):
                        for dmx in range(3):
                            g = dky * 3 + dmx
                            rhs = bass.AP(ph3.tensor,
                                          ph3.offset + (t0 + dky) * 90 + dmx,
                                          [ph3.ap[0], [90, nr], [1, 88]])
                            nc.tensor.matmul(ps[:], wt["w_dt3"][:, g, :], rhs,
                                             start=(gi == 0), stop=(gi == 8))
                            gi += 1
                    ev = wpool.tile([64, nr, 88], dt.bfloat16, tag=f"ev3{s}")
                    nc.scalar.activation(ev[:], ps[:], RELU,
                                         bias=ct["t_dt3"][0:64, 0:1],
                                         scale=ct["s_dt3"][0:64, 0:1])
                    mbb = bass.AP(m3.tensor, m3.offset + t0,
                                  [m3.ap[0], [1, nr], [0, 88]])
                    nc.vector.tensor_tensor(out=dtc[:, t0:t0 + nr, 2:90],
                                            in0=ev[:], in1=mbb[0:64],
                                            op=mybir.AluOpType.mult)

            def stage_xload(s):
                S = SEGS[s]
                xs = []
                for g in range(2):
                    xt = bpool.tile([128, S["nt3"] * 92], dt.bfloat16,
                                    tag=f"x{g}_{s}", name=f"xseg_t{g}")
                    nc.sync.dma_start(out=xt[:], in_=AP[f"xseg{s}"][g])
                    xs.append(xt)
                st[s]["xs"] = xs

            def stage_dn1(s):
                S = SEGS[s]
                nt2, nt3, nn1 = S["nt2"], S["nt3"], S["nn1"]
                mall, dtc, xs = malls[s], st[s]["dtc"], st[s]["xs"]
                mn1 = bass.AP(mall.tensor, mall.offset + nt2 + nt3,
                              [mall.ap[0], [1, nn1]])
                n1o = []
                for g in range(2):
                    t = bpool.tile([128, nn1, 92], dt.bfloat16,
                                   tag=f"n1o{g}_{s}", name=f"n1o{g}_{s}")
                    nc.vector.memset(t[:, :, 0:2], 0.0)
                    nc.vector.memset(t[:, :, 90:92], 0.0)
                    n1o.append(t)
                st[s]["n1o"] = n1o
                RPP = 5
                for ocg in range(2):
                    for r0 in range(0, nn1, RPP):
                        nr = min(RPP, nn1 - r0)
                        ps = ppool.tile([128, nr, 88], dt.float32, tag=f"ps{s}")
                        gi = 0
                        for ky in range(3):
                            for kx in range(3):
                                tap = ky * 3 + kx
                                for icc, srcT in enumerate((xs[0], xs[1], dtc)):
                                    kk = 128 if icc < 2 else 64
                                    rhs = bass.AP(
                                        srcT.tensor,
                                        srcT.offset + (r0 + ky + 1) * 92 + kx + 1,
                                        [srcT.ap[0], [92, nr], [1, 88]])
                                    lhs = wt["w_dn1"][0:kk, tap * 3 + icc,
                                                      ocg * 128:(ocg + 1) * 128]
                                    nc.tensor.matmul(ps[:], lhs, rhs,
                                                     start=(gi == 0),
                                                     stop=(gi == 26))
                                    gi += 1
                        ev = wpool.tile([128, nr, 88], dt.bfloat16, tag=f"evn1{s}")
                        nc.scalar.activation(ev[:], ps[:], RELU,
                                             bias=ct["t_dn1"][:, ocg:ocg + 1],
                                             scale=ct["s_dn1"][:, ocg:ocg + 1])
                        mbb = bass.AP(mn1.tensor, mn1.offset + r0,
                                      [mn1.ap[0], [1, nr], [0, 88]])
                        nc.vector.tensor_tensor(
                            out=n1o[ocg][:, r0:r0 + nr, 2:90],
                            in0=ev[:], in1=mbb, op=mybir.AluOpType.mult)

            def stage_dn2(s):
                S = SEGS[s]
                nout, n1o = S["nout"], st[s]["n1o"]
                RPP = 5
                n2o = []
                for g in range(2):
                    n2o.append(bpool.tile([128, nout, 88], dt.bfloat16,
                                          tag=f"n2o{g}_{s}", name=f"n2o{g}_{s}"))
                st[s]["n2o"] = n2o
                dn3 = stage_dn3(s)
                next(dn3)                        # prime: allocates out tiles
                for r0 in range(0, nout, RPP):
                    nr = min(RPP, nout - r0)
                    for ocg in range(2):
                        ps = ppool.tile([128, nr, 88], dt.float32, tag=f"ps{s}")
                        gi = 0
                        for ky in range(3):
                            for kx in range(3):
                                tap = ky * 3 + kx
                                for icc in range(2):
                                    rhs = bass.AP(
                                        n1o[icc].tensor,
                                        n1o[icc].offset + (r0 + ky) * 92 + kx + 1,
                                        [n1o[icc].ap[0], [92, nr], [1, 88]])
                                    lhs = wt["w_dn2"][:, tap * 2 + icc,
                                                      ocg * 128:(ocg + 1) * 128]
                                    nc.tensor.matmul(ps[:], lhs, rhs,
                                                     start=(gi == 0),
                                                     stop=(gi == 17))
                                    gi += 1
                        ev = wpool.tile([128, nr, 88], dt.bfloat16, tag=f"evn2{s}")
                        nc.scalar.activation(ev[:], ps[:], RELU,
                                             bias=ct["t_dn2"][:, ocg:ocg + 1],
                                             scale=ct["s_dn2"][:, ocg:ocg + 1])
                        nc.vector.tensor_copy(n2o[ocg][:, r0:r0 + nr, :], ev[:])
                    try:
                        dn3.send(r0 + nr)        # emit dn3 chunks now ready
                    except StopIteration:
                        pass

            def stage_dn3(s):
                """Generator: receives the count of completed dn2 rows and
                emits dn3+softmax for pixel chunks whose rows are ready."""
                S = SEGS[s]
                nout, n2o = S["nout"], st[s]["n2o"]
                npix = nout * FW
                feat_sb[s] = bpool.tile([128, ((npix + 127) // 128) * CIMG],
                                        dt.bfloat16, tag=f"feat{s}", name=f"feat_sb{s}")
                depth_sb[s] = bpool.tile([128, ((npix + 127) // 128) * DD],
                                         dt.float32, tag=f"depth{s}", name=f"depth_sb{s}")
                n2f = [t.rearrange("p a b -> p (a b)") for t in n2o]
                a0 = 0 if s == 0 else 11
                pcs = (npix + 127) // 128
                rows_done = yield
                for pc in range(pcs):
                    if pc == pcs - 1:
                        # flush all-but-last chunk now so only the final
                        # chunk's output DMA sits in the tail
                        dsl = bass.AP(out_depth.tensor,
                                      out_depth.offset + a0 * DD,
                                      [[17 * DD, 128], [1, (pcs - 1) * DD]])
                        nc.sync.dma_start(
                            out=dsl, in_=depth_sb[s][:, 0:(pcs - 1) * DD])
                        fsl = bass.AP(out_feat.tensor,
                                      out_feat.offset + a0 * CIMG,
                                      [[17 * CIMG, 128], [1, (pcs - 1) * CIMG]])
                        nc.sync.dma_start(
                            out=fsl, in_=feat_sb[s][:, 0:(pcs - 1) * CIMG])
                    m = min(128, npix - pc * 128)
                    # rows needed by pixels [pc*128, pc*128+m)
                    need = (pc * 128 + m - 1) // FW + 1
                    while rows_done < need:
                        rows_done = yield
                    ps = ppool.tile([m, 139], dt.float32, tag=f"ps{s}")
                    for icc in range(2):
                        nc.tensor.matmul(ps[:], n2f[icc][:, pc * 128:pc * 128 + m],
                                         wt["w_dn3"][:, icc, :],
                                         start=(icc == 0), stop=(icc == 1))
                    # add bias via vector then softmax over first 59
                    lg = wpool.tile([m, 139], dt.float32, tag=f"lg{s}")
                    nc.vector.tensor_tensor(out=lg[:], in0=ps[:],
                                            in1=ct["b_dn3"][0:m],
                                            op=mybir.AluOpType.add)
                    mx = wpool.tile([m, 1], dt.float32, tag=f"mx{s}")
                    nc.vector.reduce_max(mx[:], lg[:, 0:DD],
                                         axis=mybir.AxisListType.X, negate=True)
                    ex = wpool.tile([m, DD], dt.float32, tag=f"ex{s}")
                    nc.scalar.activation(ex[:], lg[:, 0:DD],
                                         mybir.ActivationFunctionType.Exp,
                                         bias=mx[:, 0:1], scale=1.0)
                    sm = wpool.tile([m, 1], dt.float32, tag=f"sm{s}")
                    nc.vector.reduce_sum(sm[:], ex[:], axis=mybir.AxisListType.X)
                    rc = wpool.tile([m, 1], dt.float32, tag=f"rc{s}")
                    nc.vector.reciprocal(rc[:], sm[:])
                    nc.vector.tensor_scalar(
                        out=depth_sb[s][0:m, pc * DD:(pc + 1) * DD], in0=ex[:],
                        scalar1=rc[:, 0:1], scalar2=None,
                        op0=mybir.AluOpType.mult)
                    nc.vector.tensor_copy(
                        feat_sb[s][0:m, pc * CIMG:(pc + 1) * CIMG],
                        lg[:, DD:DD + CIMG])

                # final chunk's outputs
                dsl = bass.AP(out_depth.tensor,
                              out_depth.offset + (a0 + pcs - 1) * DD,
                              [[17 * DD, 128], [1, DD]])
                nc.sync.dma_start(out=dsl,
                                  in_=depth_sb[s][:, (pcs - 1) * DD:pcs * DD])
                fsl = bass.AP(out_feat.tensor,
                              out_feat.offset + (a0 + pcs - 1) * CIMG,
                              [[17 * CIMG, 128], [1, CIMG]])
                nc.sync.dma_start(out=fsl,
                                  in_=feat_sb[s][:, (pcs - 1) * CIMG:pcs * CIMG])

            # schedule: dt1 is folded into the host's dph prep; dt2(1)/dt3(0)
            # hide the scr roundtrips; dn3 is fused into dn2 so softmax
            # pipelines under matmuls
            stage_dt2(0)
            scr_write(0)
            stage_dt2(1)
            stage_dt3(0)
            stage_xload(0)
            scr_write(1)
            stage_dt3(1)
            stage_xload(1)
            stage_wload()
            stage_dn1(0)
            stage_dn1(1)
            stage_dn2(0)
            stage_dn2(1)
    nc.compile()
    return nc


# ------------------------------------------------------------ host helpers
def _host_geometry(rots, trans, intr, post_rots, post_trans):
    import jax
    import jax.numpy as jnp
    with jax.default_device(jax.devices("cpu")[0]):
        f32 = jnp.float32
        ds = jnp.arange(1.0, 60.0, 1.0, dtype=f32)
        xs = jnp.linspace(0.0, IW - 1.0, FW, dtype=f32)
        ys = jnp.linspace(0.0, IH - 1.0, FH, dtype=f32)
        dm = jnp.broadcast_to(ds[:, None, None], (DD, FH, FW))
        xm = jnp.broadcast_to(xs[None, None, :], (DD, FH, FW))
        ym = jnp.broadcast_to(ys[None, :, None], (DD, FH, FW))
        fr = jnp.stack([xm, ym, dm], -1)
        pts = fr[None, None] - jnp.asarray(post_trans)[:, :, None, None, None, :]
        pts = jnp.einsum("bnij,bndhwj->bndhwi",
                         jnp.linalg.inv(jnp.asarray(post_rots)), pts)
        pts = jnp.concatenate([pts[..., :2] * pts[..., 2:3], pts[..., 2:3]], -1)
        comb = jnp.einsum("bnij,bnjk->bnik", jnp.asarray(rots),
                          jnp.linalg.inv(jnp.asarray(intr)))
        pts = jnp.einsum("bnij,bndhwj->bndhwi", comb, pts) \
            + jnp.asarray(trans)[:, :, None, None, None, :]
        lo = jnp.array([XY0, XY0, Z0], dtype=f32)
        dxv = jnp.array([DXY, DXY, DZ], dtype=f32)
        g = ((pts - lo) / dxv).astype(jnp.int32).reshape(-1, 3)
        kept = ((g[:, 0] >= 0) & (g[:, 0] < NX) & (g[:, 1] >= 0) & (g[:, 1] < NX)
                & (g[:, 2] >= 0) & (g[:, 2] < NZ))
        flat = (g[:, 2] * NX + g[:, 0]) * NX + g[:, 1]
        return np.asarray(flat, np.int64), np.asarray(kept)


def _prep_a_inputs(inputs):
    """Build per-core input maps for launch A."""
    d = np.asarray(inputs["d"], np.float32).reshape(N, IH, IW)
    x_img = np.asarray(inputs["x_img"], np.float32)

    # dt1 folded affine: relu(alpha*d + beta), alpha = s*w, beta = s*b + t
    a1 = (inputs["dt1_s"] * inputs["dt1_w"][:, 0, 0, 0]).astype(np.float32)
    b1 = (inputs["dt1_s"] * inputs["dt1_b"] + inputs["dt1_t"]).astype(np.float32)
    cab = np.arange(128)
    dt1_alpha = a1[cab // 16][:, None]
    dt1_beta = b1[cab // 16][:, None]

    def wprep_dt2():
        w = np.asarray(inputs["dt2_w"], np.float32)      # [32,8,5,5]
        out = np.zeros((4, 128, 32), np.float32)
        for ky in range(5):
            for kx in range(5):
                a, dky = ky % 4, ky // 4
                bph, dmx = (kx + 2) % 4, (kx + 2) // 4
                g = dky * 2 + dmx
                rows = (np.arange(8)) * 16 + a * 4 + bph
                out[g, rows, :] = w[:, :, ky, kx].T
        return out.astype(bf16)

    def wprep_dt3():
        w = np.asarray(inputs["dt3_w"], np.float32)      # [64,32,5,5]
        out = np.zeros((9, 128, 64), np.float32)
        for ky in range(5):
            for kx in range(5):
                a, dky = ky % 2, ky // 2
                bph, dmx = kx % 2, (kx + 2) // 2 - 1
                g = dky * 3 + dmx
                rows = (a * 2 + bph) * 32 + np.arange(32)
                out[g, rows, :] = w[:, :, ky, kx].T
        return out.astype(bf16)

    def wprep_3x3(w, icc_sizes):
        O, I = w.shape[0], w.shape[1]
        nic = len(icc_sizes)
        out = np.zeros((9, nic, 128, O), np.float32)
        for ky in range(3):
            for kx in range(3):
                tap = ky * 3 + kx
                ic0 = 0
                for icc, sz in enumerate(icc_sizes):
                    out[tap, icc, 0:sz, :] = w[:, ic0:ic0 + sz, ky, kx].T
                    ic0 += sz
        return out.astype(bf16)

    # NOTE: dn1 input concat order is [dt3(64) | x_img(256)] in the reference;
    # our matmul chunks are (x0:128, x1:128, dt3:64) -> weight cols must match:
    w_dn1_full = np.asarray(inputs["dn1_w"], np.float32)
    w_dn1 = np.zeros((9, 3, 128, 256), np.float32)
    for ky in range(3):
        for kx in range(3):
            tap = ky * 3 + kx
            w_dn1[tap, 0, :, :] = w_dn1_full[:, 64:192, ky, kx].T
            w_dn1[tap, 1, :, :] = w_dn1_full[:, 192:320, ky, kx].T
            w_dn1[tap, 2, 0:64, :] = w_dn1_full[:, 0:64, ky, kx].T
    w_dn1 = w_dn1.astype(bf16)
    w_dn2 = wprep_3x3(np.asarray(inputs["dn2_w"], np.float32), [128, 128])
    w_dn3 = np.asarray(inputs["dn3_w"], np.float32)[:, :, 0, 0]  # [139, 256]
    w_dn3p = np.zeros((2, 128, 139), np.float32)
    w_dn3p[0] = w_dn3[:, 0:128].T
    w_dn3p[1] = w_dn3[:, 128:256].T

    def fold_bias(b, s, t):
        # conv bias b then bn scale/shift: relu(s*(x+b) + t) = relu(s*x + (s*b+t))
        return np.asarray(s, np.float32), np.asarray(s * b + t, np.float32)

    s2, t2 = fold_bias(inputs["dt2_b"], inputs["dt2_s"], inputs["dt2_t"])
    s3, t3 = fold_bias(inputs["dt3_b"], inputs["dt3_s"], inputs["dt3_t"])
    sn1, tn1 = fold_bias(inputs["dn1_b"], inputs["dn1_s"], inputs["dn1_t"])
    sn2, tn2 = fold_bias(inputs["dn2_b"], inputs["dn2_s"], inputs["dn2_t"])
    b_dn3 = np.broadcast_to(np.asarray(inputs["dn3_b"], np.float32)[None, :],
                            (128, 139)).copy()

    consts = np.zeros((128, 153), np.float32)
    consts[:, 0] = dt1_alpha[:, 0]
    consts[:, 1] = dt1_beta[:, 0]
    consts[:, 2] = np.tile(s2, 4)
    consts[:, 3] = np.tile(t2, 4)
    consts[:, 4] = np.tile(s3, 2)
    consts[:, 5] = np.tile(t3, 2)
    consts[:, 6:8] = sn1.reshape(2, 128).T
    consts[:, 8:10] = tn1.reshape(2, 128).T
    consts[:, 10:12] = sn2.reshape(2, 128).T
    consts[:, 12:14] = tn2.reshape(2, 128).T
    consts[:, 14:153] = b_dn3
    shared = dict(
        consts=consts,
        w_dt2=wprep_dt2(), w_dt3=wprep_dt3(), w_dn1=w_dn1, w_dn2=w_dn2,
        w_dn3=w_dn3p.astype(bf16),
    )

    maps = []
    for c in range(NCORES):
        m = dict(shared)
        for s, (cam, h0) in enumerate([SEG_A[c], SEG_B[c]]):
            S = SEGS[s]
            d0 = 8 * h0 - 34
            dseg = np.zeros((S["nd"], 712), np.float32)
            vseg = np.zeros((S["nd"], 712), bool)
            lo, hi = max(0, d0), min(IH, d0 + S["nd"])
            if hi > lo:
                dseg[lo - d0:hi - d0, 4:708] = d[cam, lo:hi]
                vseg[lo - d0:hi - d0, 4:708] = True
            nq = S["nq"]
            ph = dseg.reshape(nq, 4, 178, 4)[:, :, :177, :]     # ry a rx b
            ph = ph.transpose(1, 3, 0, 2)                        # a b ry rx
            vph = vseg.reshape(nq, 4, 178, 4)[:, :, :177, :].transpose(1, 3, 0, 2)
            # dt1 applied on host: relu(alpha*d + beta), zero at pads
            dphc = np.where(vph[None],
                            np.maximum(a1[:, None, None, None, None] * ph[None]
                                       + b1[:, None, None, None, None], 0.0),
                            0.0)                                 # [8,4,4,nq,177]
            m[f"dph{s}"] = dphc.reshape(128, nq * 177).astype(bf16)
            q0, t0, r0 = 2 * h0 - 8, h0 - 3, h0 - 1
            qr = np.arange(S["nt2"]) + q0
            m2m = np.broadcast_to(((qr >= 0) & (qr < 64))[None, :],
                                  (128, S["nt2"]))
            tr = np.arange(S["nt3"]) + t0
            m3m = np.broadcast_to(((tr >= 0) & (tr < FH))[None, :],
                                  (128, S["nt3"]))
            rr = np.arange(S["nn1"]) + r0
            mn1m = np.broadcast_to(((rr >= 0) & (rr < FH))[None, :],
                                   (128, S["nn1"]))
            m[f"masks{s}"] = np.concatenate(
                [m2m, m3m, mn1m], axis=1).astype(bf16)
            xseg = np.zeros((2, 128, S["nt3"], 92), np.float32)
            lo2, hi2 = max(0, t0), min(FH, t0 + S["nt3"])
            if hi2 > lo2:
                xseg[:, :, lo2 - t0:hi2 - t0, 2:90] = \
                    x_img[cam, :, lo2:hi2, :].reshape(2, 128, hi2 - lo2, FW)
            m[f"xseg{s}"] = xseg.reshape(2, 128, S["nt3"] * 92).astype(bf16)
        maps.append(m)
    return maps


# ---------------------------------------------------------------- launch B
def build_launch_b(sizes):
    """Per chunk k: [128pix x 80ch] stationary feat tile x host-built
    [128pix x sizes[k] voxel-slot] depth-weight matrix -> [80, nv] window
    sums. W and out use packed (variable-size) layouts; W loads in a few
    batched DMAs, out in one."""
    nc = bacc.Bacc("TRN2", target_bir_lowering=False, debug=False,
                   num_devices=NCORES)
    NCH = len(sizes)
    offs = np.concatenate([[0], np.cumsum(sizes)]).astype(int)
    S = int(offs[-1])
    wmat = nc.dram_tensor("wmat", [128, S], dt.bfloat16,
                          kind="ExternalInput").ap()
    feats = nc.dram_tensor("feats", [128, NCH, CIMG], dt.bfloat16,
                           kind="ExternalInput").ap()
    owin = nc.dram_tensor("owin", [CIMG, S], dt.bfloat16,
                          kind="ExternalOutput").ap()
    NB = 4                                   # W DMA batches
    bnd = [int(round(NCH * i / NB)) for i in range(NB + 1)]
    with tile.TileContext(nc) as tc:
        with tc.tile_pool(name="const", bufs=1) as cpool, \
             tc.tile_pool(name="ps", bufs=4, space="PSUM") as pp:
            ft = cpool.tile([128, NCH, CIMG], dt.bfloat16, name="ft")
            nc.sync.dma_start(out=ft[:], in_=feats)
            wt = cpool.tile([128, S], dt.bfloat16, name="wt")
            for b in range(NB):
                lo, hi = offs[bnd[b]], offs[bnd[b + 1]]
                if hi > lo:
                    nc.sync.dma_start(out=wt[:, lo:hi], in_=wmat[:, lo:hi])
            ot = cpool.tile([CIMG, S], dt.bfloat16, name="ot")
            for k in range(NCH):
                nv, o0 = int(sizes[k]), int(offs[k])
                ps = pp.tile([CIMG, 512], dt.float32, tag="ps", name="ps")
                nc.tensor.matmul(ps[:, 0:nv], ft[:, k, :], wt[:, o0:o0 + nv],
                                 start=True, stop=True)
                if k % 2 == 0:
                    nc.scalar.activation(ot[:, o0:o0 + nv], ps[:, 0:nv],
                                         mybir.ActivationFunctionType.Copy)
                else:
                    nc.vector.tensor_copy(ot[:, o0:o0 + nv], ps[:, 0:nv])
            nc.sync.dma_start(out=owin, in_=ot[:])
    nc.compile()
    return nc


# ---------------------------------------------------------------- launch C
C_OUT_ROWS = 23              # ds2-out rows per core (8*23 = 184 >= 180)


def build_launch_c():
    nc = bacc.Bacc("TRN2", target_bir_lowering=False, debug=False,
                   num_devices=NCORES)
    NR1 = C_OUT_ROWS + 2                         # ds1-out rows incl halo (25)
    NRP = 2 * NR1 + 1                            # pooled rows needed (51)
    slab = nc.dram_tensor("slab", [CIMG, NRP, 362], dt.bfloat16,
                          kind="ExternalInput").ap()
    m1 = nc.dram_tensor("m1", [128, NR1], dt.bfloat16, kind="ExternalInput").ap()
    wd1 = nc.dram_tensor("wd1", [9, CIMG, CIMG], dt.bfloat16,
                         kind="ExternalInput").ap()
    wd2 = nc.dram_tensor("wd2", [9, CIMG, CIMG], dt.bfloat16,
                         kind="ExternalInput").ap()
    sb1 = nc.dram_tensor("sb1", [CIMG, 2], dt.float32, kind="ExternalInput").ap()
    sb2 = nc.dram_tensor("sb2", [CIMG, 2], dt.float32, kind="ExternalInput").ap()
    yout = nc.dram_tensor("yout", [CIMG, C_OUT_ROWS, 180], dt.float32,
                          kind="ExternalOutput").ap()
    RELU = mybir.ActivationFunctionType.Relu
    with tile.TileContext(nc) as tc:
        with tc.tile_pool(name="const", bufs=1) as cpool,              tc.tile_pool(name="work", bufs=2) as wp,              tc.tile_pool(name="big", bufs=1) as bp,              tc.tile_pool(name="ps", bufs=3, space="PSUM") as pp:
            # weights/consts first so ds1 can start on the first slab chunk
            w1 = cpool.tile([CIMG, 9, CIMG], dt.bfloat16, name="w1")
            nc.sync.dma_start(out=w1[:], in_=wd1.rearrange("t p o -> p t o"))
            sb1t = cpool.tile([CIMG, 2], dt.float32, name="sb1t")
            nc.sync.dma_start(out=sb1t[:], in_=sb1)
            m1t = wp.tile([128, NR1], dt.bfloat16, name="m1t")
            nc.sync.dma_start(out=m1t[:], in_=m1)
            slabt = bp.tile([CIMG, NRP, 362], dt.bfloat16, name="slabt")
            for rr in range(0, NRP, 9):
                nrr = min(9, NRP - rr)
                nc.sync.dma_start(out=slabt[:, rr:rr + nrr, :],
                                  in_=slab[:, rr:rr + nrr, :])
            w2 = cpool.tile([CIMG, 9, CIMG], dt.bfloat16, name="w2")
            nc.sync.dma_start(out=w2[:], in_=wd2.rearrange("t p o -> p t o"))
            sb2t = cpool.tile([CIMG, 2], dt.float32, name="sb2t")
            nc.sync.dma_start(out=sb2t[:], in_=sb2)
            h1 = bp.tile([CIMG, NR1, 182], dt.bfloat16, name="h1")
            nc.vector.memset(h1[:, :, 0:1], 0.0)
            nc.vector.memset(h1[:, :, 181:182], 0.0)
            # ds1: stride-2 3x3; out row t reads slab rows 2t..2t+2 (slab row 0
            # = pooled row 2o0-3, so out row t (global o0-1+t) reads
            # 2(o0-1+t)-1..+1 - (2o0-3) = 2t..2t+2); col c reads 2c..2c+2
            RP = 2
            for t0 in range(0, NR1, RP):
                nr = min(RP, NR1 - t0)
                ps = pp.tile([CIMG, nr, 180], dt.float32, tag="ps1", name="ps")
                gi = 0
                for ky in range(3):
                    for kx in range(3):
                        rhs = bass.AP(slabt.tensor,
                                      slabt.offset + (2 * t0 + ky) * 362 + kx,
                                      [slabt.ap[0], [2 * 362, nr], [2, 180]])
                        nc.tensor.matmul(ps[:], w1[:, ky * 3 + kx, :], rhs,
                                         start=(gi == 0), stop=(gi == 8))
                        gi += 1
                ev = wp.tile([CIMG, nr, 180], dt.bfloat16, tag="ev", name="ev")
                nc.scalar.activation(ev[:], ps[:], RELU, bias=sb1t[:, 1:2],
                                     scale=sb1t[:, 0:1])
                mbb = bass.AP(m1t.tensor, m1t.offset + t0,
                              [[m1t.ap[0][0], CIMG], [1, nr], [0, 180]])
                nc.vector.tensor_tensor(out=h1[:, t0:t0 + nr, 1:181],
                                        in0=ev[:], in1=mbb,
                                        op=mybir.AluOpType.mult)
            # ds2: 3x3 pad 1: out row o reads h1 rows o..o+2, col c: c..c+2
            yo = bp.tile([CIMG, C_OUT_ROWS, 180], dt.float32, name="yo")
            for o0 in range(0, C_OUT_ROWS, RP):
                nr = min(RP, C_OUT_ROWS - o0)
                ps = pp.tile([CIMG, nr, 180], dt.float32, tag="ps2", name="ps")
                gi = 0
                for ky in range(3):
                    for kx in range(3):
                        rhs = bass.AP(h1.tensor,
                                      h1.offset + (o0 + ky) * 182 + kx,
                                      [h1.ap[0], [182, nr], [1, 180]])
                        nc.tensor.matmul(ps[:], w2[:, ky * 3 + kx, :], rhs,
                                         start=(gi == 0), stop=(gi == 8))
                        gi += 1
                nc.scalar.activation(yo[:, o0:o0 + nr, :], ps[:], RELU,
                                     bias=sb2t[:, 1:2], scale=sb2t[:, 0:1])
                if (o0 // RP) % 3 == 2 or o0 + nr >= C_OUT_ROWS:
                    lo = (o0 // (3 * RP)) * 3 * RP
                    nc.sync.dma_start(out=yout[:, lo:o0 + nr, :],
                                      in_=yo[:, lo:o0 + nr, :])
    nc.compile()
    return nc


_CACHE = {}


def run_launch_a(inputs):
    if "A" not in _CACHE:
        _CACHE["A"] = build_launch_a()
    nc = _CACHE["A"]
    maps = _prep_a_inputs(inputs)
    res = run_bass_kernel_spmd(nc, maps, list(range(NCORES)))
    depth = np.zeros((NPIX, DD), np.float32)
    feat = np.zeros((NPIX, CIMG), np.float32)
    for c in range(NCORES):
        r = res.results[c]
        for s, (cam, h0) in enumerate([SEG_A[c], SEG_B[c]]):
            S = SEGS[s]
            npix = S["nout"] * FW
            base = (cam * FH + h0) * FW
            a0, pcs = (0, 11) if s == 0 else (11, 6)
            dsg = r["out_depth"][:, a0:a0 + pcs].transpose(1, 0, 2)
            depth[base:base + npix] = dsg.reshape(pcs * 128, DD)[:npix]
            fsg = r["out_feat"][:, a0:a0 + pcs].transpose(1, 0, 2)
            feat[base:base + npix] = fsg.reshape(pcs * 128, CIMG)[:npix]
    return depth, feat


def _build_chunks(flat, kept, depth_rows):
    """Group points by (camera, column-block); per group build the
    [pix, voxel-slot] depth-weight matrix over the group's voxel union.
    Splits column blocks whose union exceeds the PSUM window (512)."""
    fl = flat.reshape(N, DD, FH, FW)
    kp = kept.reshape(N, DD, FH, FW)
    chunks = []                      # (pix_ids, Wdense[npix, nv], vox_ids)

    def add_group(n, w0, w1):
        nw = w1 - w0
        f = fl[n, :, :, w0:w1]                       # [DD, FH, nw]
        k = kp[n, :, :, w0:w1]
        vids = np.unique(f[k])
        if len(vids) > 512 and nw > 1:
            mid = w0 + nw // 2
            add_group(n, w0, mid)
            add_group(n, mid, w1)
            return
        nv = max(len(vids), 1)
        # pixel local idx = (w - w0) * FH + h; point (d, h, w)
        slot = np.searchsorted(vids, f[k]) if len(vids) else np.zeros(0, np.int64)
        dd, hh, ww = np.nonzero(k)
        pix_loc = ww * FH + hh
        pixcol = n * FH * FW + hh * FW + (ww + w0)
        dep = depth_rows[pixcol, dd]
        Wd = np.bincount(pix_loc * nv + slot, weights=dep,
                         minlength=nw * FH * nv).reshape(nw * FH, nv)
        pix_ids = (n * FH * FW + np.arange(FH)[None, :] * FW
                   + (w0 + np.arange(nw))[:, None]).reshape(-1)
        chunks.append((pix_ids, Wd, vids))

    for n in range(N):
        for w0 in range(0, FW, 4):
            add_group(n, w0, w0 + 4)
    return chunks


def _prep_b_inputs(chunks, featflat_bf):
    """Balance chunks across cores by window size; build per-core maps with
    the packed per-slot layout (chunk k size = max over cores, desc-sorted)."""
    order = sorted(range(len(chunks)), key=lambda i: -chunks[i][1].shape[1])
    load = np.zeros(NCORES, np.int64)
    per_core = [[] for _ in range(NCORES)]
    for i in order:
        c = int(np.argmin(load))
        per_core[c].append(i)
        load[c] += chunks[i][1].shape[1]
    NCH = max(len(p) for p in per_core)
    sizes = np.zeros(NCH, np.int64)
    for p in per_core:
        for k, i in enumerate(p):
            sizes[k] = max(sizes[k], chunks[i][1].shape[1])
    sizes = (sizes + 15) // 16 * 16
    offs = np.concatenate([[0], np.cumsum(sizes)]).astype(int)
    S = int(offs[-1])
    maps, scatter = [], []
    for c in range(NCORES):
        wm = np.zeros((128, S), bf16)
        ft = np.zeros((128, NCH, CIMG), bf16)
        sc = []
        for k, i in enumerate(per_core[c]):
            pix_ids, Wd, vids = chunks[i]
            npix, nv = Wd.shape
            wm[0:npix, offs[k]:offs[k] + nv] = Wd
            ft[0:npix, k, :] = featflat_bf[pix_ids]
            sc.append((int(offs[k]), vids))
        maps.append(dict(wmat=wm, feats=ft))
        scatter.append(sc)
    return maps, scatter, tuple(int(s) for s in sizes)


def _prep_c_inputs(inputs, pooled_t):
    """pooled_t: [CIMG, 360, 360] f32 -> per-core slabs + masks + weights."""
    NR1 = C_OUT_ROWS + 2
    NRP = 2 * NR1 + 1
    w1 = np.asarray(inputs["ds1_w"], np.float32)
    w2 = np.asarray(inputs["ds2_w"], np.float32)
    wd1 = np.stack([w1[:, :, ky, kx].T for ky in range(3) for kx in range(3)])
    wd2 = np.stack([w2[:, :, ky, kx].T for ky in range(3) for kx in range(3)])
    sb1 = np.stack([np.asarray(inputs["ds1_s"], np.float32),
                    np.asarray(inputs["ds1_t"], np.float32)], 1)
    sb2 = np.stack([np.asarray(inputs["ds2_s"], np.float32),
                    np.asarray(inputs["ds2_t"], np.float32)], 1)
    shared = dict(wd1=wd1.astype(bf16), wd2=wd2.astype(bf16), sb1=sb1, sb2=sb2)
    maps = []
    pt_bf = pooled_t.astype(bf16)
    for c in range(NCORES):
        o0g = C_OUT_ROWS * c
        p0 = 2 * o0g - 3
        slab = np.zeros((CIMG, NRP, 362), bf16)
        lo, hi = max(0, p0), min(NX, p0 + NRP)
        if hi > lo:
            slab[:, lo - p0:hi - p0, 1:361] = pt_bf[:, lo:hi, :]
        t1g = np.arange(NR1) + (o0g - 1)
        m1 = np.broadcast_to(((t1g >= 0) & (t1g < 180))[None, :],
                             (128, NR1)).astype(bf16)
        maps.append(dict(shared, slab=slab, m1=np.ascontiguousarray(m1)))
    return maps


def kernel(**inputs):
    inputs = {k: np.asarray(v) for k, v in inputs.items()}
    flat, kept = _host_geometry(inputs["cam2lidar_rots"],
                                inputs["cam2lidar_trans"], inputs["intrins"],
                                inputs["post_rots"], inputs["post_trans"])
    depth_rows, feat_rows = run_launch_a(inputs)
    featflat_bf = feat_rows.astype(bf16)

    chunks = _build_chunks(flat, kept, depth_rows)
    bmaps, scatter, sizes = _prep_b_inputs(chunks, featflat_bf)
    key = ("B", sizes)
    if key not in _CACHE:
        _CACHE[key] = build_launch_b(sizes)
    res_b = run_bass_kernel_spmd(_CACHE[key], bmaps, list(range(NCORES)))

    allvox = np.concatenate([vids for c in range(NCORES)
                             for _, vids in scatter[c]])
    allval = np.concatenate(
        [res_b.results[c]["owin"][:, o0:o0 + len(vids)].T.astype(np.float32)
         for c in range(NCORES) for o0, vids in scatter[c]])
    o = np.argsort(allvox, kind="stable")
    allvox, allval = allvox[o], allval[o]
    starts = np.flatnonzero(np.r_[True, allvox[1:] != allvox[:-1]])
    pooled = np.zeros((NX * NX, CIMG), np.float32)
    pooled[allvox[starts]] = np.add.reduceat(allval, starts, axis=0)
    pooled_t = np.ascontiguousarray(
        pooled.reshape(NX, NX, CIMG).transpose(2, 0, 1))

    if "C" not in _CACHE:
        _CACHE["C"] = build_launch_c()
    cmaps = _prep_c_inputs(inputs, pooled_t)
    res_c = run_bass_kernel_spmd(_CACHE["C"], cmaps, list(range(NCORES)))
    out = np.zeros((1, CIMG, 180, 180), np.float32)
    for c in range(NCORES):
        o0g = C_OUT_ROWS * c
        nr = min(C_OUT_ROWS, 180 - o0g)
        if nr > 0:
            out[0, :, o0g:o0g + nr, :] = res_c.results[c]["yout"][:, 0:nr, :]
    return out



# revision 97
# speedup vs baseline: 1.0438x; 1.0137x over previous
"""DepthLSSTransform Trainium kernel: 3 SPMD launches over 8 NeuronCores.

Launch A: per-camera conv pipeline (dtransform + depthnet + softmax) on
          24-row bands (one 16-row + one 8-row segment per core).
Launch B: bev_pool segment-sum via one-hot matmuls over a host-built
          virtual-window schedule (sorted-by-voxel points).
Launch C: BEV downsample convs, spatially sharded.
Host: geometry/voxel indices, scheduling, gathers, folds (orchestration).
"""
import numpy as np
import ml_dtypes

import concourse.bass as bass
import concourse.tile as tile
from concourse import bacc, mybir
from concourse.bass_utils import run_bass_kernel_spmd

dt = mybir.dt
bf16 = ml_dtypes.bfloat16

# ---- problem constants (hardcoded per contract) ----
B, N = 1, 6
CIN, CIMG, DD = 256, 80, 59
FH, FW, IH, IW = 32, 88, 256, 704
XY0, DXY, NX = -54.0, 0.3, 360
Z0, DZ, NZ = -10.0, 20.0, 1
NPTS = N * DD * FH * FW
NPIX = N * FH * FW
NCORES = 8
QV = 4                      # chunks of 128 points per virtual window

# per-core segments: (camera, h0) for seg A (16 rows) and seg B (8 rows)
SEG_A = [(0, 0), (1, 0), (1, 16), (2, 16), (3, 0), (4, 0), (4, 16), (5, 16)]
SEG_B = [(0, 16), (0, 24), (2, 0), (2, 8), (3, 16), (3, 24), (5, 0), (5, 8)]
# band pixel ranges in global row order (row = n*32 + h)
ROWS_OF_CORE = [[(SEG_A[c][0] * FH + SEG_A[c][1] + r) for r in range(16)] +
                [(SEG_B[c][0] * FH + SEG_B[c][1] + r) for r in range(8)]
                for c in range(NCORES)]

# segment geometry: rows16 segment: d rows [8h0-34, 8h0+158) (192), dt2 out
# rows [2h0-8, 2h0+39) (47), dt3 [h0-3, h0+19) (22), dn1 [h0-1, h0+17) (18)
SEGS = [dict(nout=16, nd=192, nq=48, nt2=47, nt3=22, nn1=18),
        dict(nout=8, nd=128, nq=32, nt2=31, nt3=14, nn1=10)]


def _seg_ranges(h0, S):
    return dict(d0=8 * h0 - 34, q0=2 * h0 - 8, t0=h0 - 3, r0=h0 - 1, o0=h0)


# ---------------------------------------------------------------- launch A
def build_launch_a(debug=False, psum_bufs=3, work_bufs=3, stages=9):
    nc = bacc.Bacc("TRN2", target_bir_lowering=False, debug=False,
                   num_devices=NCORES)
    AP = {}

    def inp(name, shape, dtype=dt.bfloat16):
        AP[name] = nc.dram_tensor(name, shape, dtype, kind="ExternalInput").ap()
        return AP[name]

    # per segment inputs (s = 0: 16-row, 1: 8-row); flat free dims so DMAs
    # are single-descriptor-per-partition and tile deps stay precise
    for s, S in enumerate(SEGS):
        inp(f"dph{s}", [128, S["nq"] * 177])
        inp(f"masks{s}", [128, S["nt2"] + S["nt3"] + S["nn1"]])
        inp(f"xseg{s}", [2, 128, S["nt3"] * 92])        # x_img slice (padded)
    # packed f32 constants: [alpha, beta, s_dt2, t_dt2, s_dt3, t_dt3,
    #  s_dn1(2), t_dn1(2), s_dn2(2), t_dn2(2), b_dn3(139)] -> [128, 153]
    inp("consts", [128, 153], dt.float32)
    # conv weights (host-prepped layouts)
    inp("w_dt2", [4, 128, 32])                          # groups (dky,dmx)
    inp("w_dt3", [9, 128, 64])
    inp("w_dn1", [9, 3, 128, 256])                      # tap, icchunk(128,128,64pad) -> 256
    inp("w_dn2", [9, 2, 128, 256])
    inp("w_dn3", [2, 128, 139])

    DBG = {}
    dbg_specs = [] if not debug else [("dbg_t1", [128, SEGS[0]["nq"], 177], dt.bfloat16),
                        ("dbg_dt2o", [32, SEGS[0]["nt2"] + 1, 180], dt.bfloat16),
                        ("dbg_dtc", [64, SEGS[0]["nt3"], 92], dt.bfloat16),
                        ("dbg_n1o", [128, SEGS[0]["nn1"], 92], dt.bfloat16),
                        ("dbg_n2o", [128, SEGS[0]["nout"], 88], dt.bfloat16)]
    for nm, sh, dty in dbg_specs:
        DBG[nm] = nc.dram_tensor(nm, sh, dty, kind="ExternalOutput").ap()
    # chunk-major outputs: pixel (a*128+p) of segment s at [p, a0_s + a, :]
    out_depth = nc.dram_tensor("out_depth", [128, 17, DD], dt.float32,
                               kind="ExternalOutput").ap()
    out_feat = nc.dram_tensor("out_feat", [128, 17, CIMG], dt.bfloat16,
                              kind="ExternalOutput").ap()

    # HBM scratch, phase-major: [c32, a2, b2, q', x90] (q' = dt2-row // 2)
    scr = {}
    for s, S in enumerate(SEGS):
        scr[f"dt2o{s}"] = nc.dram_tensor(
            f"dt2o{s}", [32, 2, 2, (S["nt2"] + 1) // 2, 90], dt.bfloat16).ap()

    RELU = mybir.ActivationFunctionType.Relu
    with tile.TileContext(nc) as tc:
        with tc.tile_pool(name="const", bufs=1) as cpool, \
             tc.tile_pool(name="work", bufs=work_bufs) as wpool, \
             tc.tile_pool(name="big", bufs=1) as bpool, \
             tc.tile_pool(name="psum", bufs=2, space="PSUM") as ppool, \
             tc.tile_pool(name="psum2", bufs=4, space="PSUM") as ppool2:
            # ---- DMA issue order = consumption order (the SP queue and the
            # modeled DMA engines serialize; early-stage inputs must land first)
            cts = cpool.tile([128, 153], dt.float32, name="cts")
            nc.sync.dma_start(out=cts[:], in_=AP["consts"])
            # tiny activation right away so the act-table load happens while
            # the first dph chunk is still in flight
            warm = wpool.tile([128, 1], dt.float32, tag="warm", name="warm")
            nc.scalar.activation(warm[:], cts[:, 0:1], RELU)
            ct = {"dt1_alpha": cts[:, 0:1], "dt1_beta": cts[:, 1:2],
                  "s_dt2": cts[:, 2:3], "t_dt2": cts[:, 3:4],
                  "s_dt3": cts[:, 4:5], "t_dt3": cts[:, 5:6],
                  "s_dn1": cts[:, 6:8], "t_dn1": cts[:, 8:10],
                  "s_dn2": cts[:, 10:12], "t_dn2": cts[:, 12:14],
                  "b_dn3": cts[:, 14:153]}
            wt = {}

            def load_w(nm, pat):
                sh = list(AP[nm].shape)
                wt[nm] = cpool.tile([sh[-2], int(np.prod(sh[:-2])), sh[-1]],
                                    dt.bfloat16, tag=nm, name=f'wt_{nm}')
                nc.sync.dma_start(out=wt[nm][:], in_=AP[nm].rearrange(pat))

            # first dph chunk small so dt2 starts ASAP; host has already
            # applied dt1 (relu(alpha*d+beta), pads zeroed) into dph.
            # The big dn-weights are issued later (stage_wload) so they don't
            # sit ahead of the dt2->dt3 scratch roundtrip in the serial DMA
            # stream.
            QCHUNKS = {0: [8, 12, 14, 14], 1: [8, 12, 12]}
            dphs, malls = {}, {}

            def load_dph(s):
                S = SEGS[s]
                nq = S["nq"]
                dphs[s] = bpool.tile([128, nq * 177], dt.bfloat16,
                                     tag=f"dph{s}", name=f"dph{s}")
                qq = 0
                for nqq in QCHUNKS[s]:
                    nc.sync.dma_start(
                        out=dphs[s][:, qq * 177:(qq + nqq) * 177],
                        in_=AP[f"dph{s}"][:, qq * 177:(qq + nqq) * 177])
                    qq += nqq
                malls[s] = wpool.tile([128, S["nt2"] + S["nt3"] + S["nn1"]],
                                      dt.bfloat16, tag=f"msk{s}", name="mall")
                nc.sync.dma_start(out=malls[s][:], in_=AP[f"masks{s}"])

            load_w("w_dt2", "g p o -> p g o")
            load_dph(0)
            load_dph(1)
            load_w("w_dt3", "g p o -> p g o")

            def stage_wload():
                load_w("w_dn1", "t i p o -> p (t i) o")
                load_w("w_dn2", "t i p o -> p (t i) o")
                load_w("w_dn3", "g p o -> p g o")

            feat_sb = {}
            depth_sb = {}
            st = {s: {} for s in range(len(SEGS))}

            def stage_dt2(s):
                S = SEGS[s]
                nt2, t1, mall = S["nt2"], dphs[s], malls[s]
                Q2 = (nt2 + 1) // 2
                # phase-major layout [c32, a2, b2, q', x90]: row q=(2q'+a),
                # col c at (b=c%2, x=c//2+1); makes scr write + ph3 reads
                # fully contiguous per partition
                o2 = bpool.tile([32, 2, 2, Q2, 90], dt.bfloat16, tag=f"o2{s}",
                                name=f"o2{s}")
                st[s]["o2"] = o2
                o2f = o2.rearrange("p a b q x -> p (a b q) x")
                nc.vector.memset(o2f[:, :, 0:1], 0.0)          # x pad left
                nc.vector.memset(o2f[:, :, 89:90], 0.0)        # x pad right
                nc.vector.memset(o2[:, 1, :, Q2 - 1, :], 0.0)  # pad row q=nt2
                m2 = bass.AP(mall.tensor, mall.offset, [mall.ap[0], [1, nt2]])
                RPP2 = 2
                for q0 in range(0, nt2, RPP2):
                    nr = min(RPP2, nt2 - q0)
                    ps = ppool2.tile([32, nr, 176], dt.float32, tag="ps2",
                                     name="ps2")
                    gi = 0
                    for dky in range(2):
                        for dmx in range(2):
                            g = dky * 2 + dmx
                            rhs = bass.AP(
                                t1.tensor, t1.offset + (q0 + dky) * 177 + dmx,
                                [t1.ap[0], [177, nr], [1, 176]])
                            nc.tensor.matmul(ps[:], wt["w_dt2"][:, g, :], rhs,
                                             start=(gi == 0), stop=(gi == 3))
                            gi += 1
                    ev = wpool.tile([32, nr, 176], dt.bfloat16, tag=f"ev2{s}")
                    nc.scalar.activation(ev[:], ps[:], RELU,
                                         bias=ct["t_dt2"][0:32, 0:1],
                                         scale=ct["s_dt2"][0:32, 0:1])
                    mbb = bass.AP(m2.tensor, m2.offset + q0,
                                  [[m2.ap[0][0], 32], [1, nr], [0, 176]])
                    # rows (q0, q0+1) -> a=(0,1) at q'=q0//2; c -> (x, b)
                    o2dst = bass.AP(o2.tensor,
                                    o2.offset + (q0 // 2) * 90 + 1,
                                    [[o2.ap[0][0], 32], [2 * Q2 * 90, nr],
                                     [1, 88], [Q2 * 90, 2]])
                    nc.vector.tensor_tensor(out=o2dst, in0=ev[:], in1=mbb,
                                            op=mybir.AluOpType.mult)

            def scr_write(s):
                # on the idle GPSIMD (SWDGE) queue: its sem wait must not
                # head-of-line-block the streaming SP DMA queue
                nc.gpsimd.dma_start(out=scr[f"dt2o{s}"], in_=st[s]["o2"][:])

            def stage_dt3(s):
                S = SEGS[s]
                nt2, nt3, mall = S["nt2"], S["nt3"], malls[s]
                Q2 = (nt2 + 1) // 2
                nry3 = nt3 + 2
                ph3 = bpool.tile([128, nry3 * 90], dt.bfloat16, tag=f"ph3{s}",
                                 name=f"ph3{s}")
                sd2 = scr[f"dt2o{s}"]
                # one DMA: partition (g, c) <- scr[(c, g)] nested dims
                pap3 = bass.AP(sd2.tensor, sd2.offset,
                               [[Q2 * 90, 4], [4 * Q2 * 90, 32],
                                [1, nry3 * 90]])
                nc.gpsimd.dma_start(out=ph3[:], in_=pap3)
                # concat input tile: [64 dt3 | pad] plus x_img tiles
                dtc = bpool.tile([64, nt3, 92], dt.bfloat16, tag=f"dtc{s}",
                                 name=f"dtc{s}")
                st[s]["dtc"] = dtc
                nc.vector.memset(dtc[:, :, 0:2], 0.0)
                nc.vector.memset(dtc[:, :, 90:92], 0.0)
                m3 = bass.AP(mall.tensor, mall.offset + nt2,
                             [mall.ap[0], [1, nt3]])
                RPP3 = 4
                for t0 in range(0, nt3, RPP3):
                    nr = min(RPP3, nt3 - t0)
                    ps = ppool.tile([64, nr, 88], dt.float32, tag=f"ps{s}")
                    gi = 0
                    for dky in range(3):
                        for dmx in range(3):
                            g = dky * 3 + dmx
                            rhs = bass.AP(ph3.tensor,
                                          ph3.offset + (t0 + dky) * 90 + dmx,
                                          [ph3.ap[0], [90, nr], [1, 88]])
                            nc.tensor.matmul(ps[:], wt["w_dt3"][:, g, :], rhs,
                                             start=(gi == 0), stop=(gi == 8))
                            gi += 1
                    ev = wpool.tile([64, nr, 88], dt.bfloat16, tag=f"ev3{s}")
                    nc.scalar.activation(ev[:], ps[:], RELU,
                                         bias=ct["t_dt3"][0:64, 0:1],
                                         scale=ct["s_dt3"][0:64, 0:1])
                    mbb = bass.AP(m3.tensor, m3.offset + t0,
                                  [m3.ap[0], [1, nr], [0, 88]])
                    nc.vector.tensor_tensor(out=dtc[:, t0:t0 + nr, 2:90],
                                            in0=ev[:], in1=mbb[0:64],
                                            op=mybir.AluOpType.mult)

            def stage_xload(s):
                S = SEGS[s]
                xs = []
                for g in range(2):
                    xt = bpool.tile([128, S["nt3"] * 92], dt.bfloat16,
                                    tag=f"x{g}_{s}", name=f"xseg_t{g}")
                    nc.sync.dma_start(out=xt[:], in_=AP[f"xseg{s}"][g])
                    xs.append(xt)
                st[s]["xs"] = xs

            def stage_dn1(s):
                S = SEGS[s]
                nt2, nt3, nn1 = S["nt2"], S["nt3"], S["nn1"]
                mall, dtc, xs = malls[s], st[s]["dtc"], st[s]["xs"]
                mn1 = bass.AP(mall.tensor, mall.offset + nt2 + nt3,
                              [mall.ap[0], [1, nn1]])
                n1o = []
                for g in range(2):
                    t = bpool.tile([128, nn1, 92], dt.bfloat16,
                                   tag=f"n1o{g}_{s}", name=f"n1o{g}_{s}")
                    nc.vector.memset(t[:, :, 0:2], 0.0)
                    nc.vector.memset(t[:, :, 90:92], 0.0)
                    n1o.append(t)
                st[s]["n1o"] = n1o
                RPP = 5
                for ocg in range(2):
                    for r0 in range(0, nn1, RPP):
                        nr = min(RPP, nn1 - r0)
                        ps = ppool.tile([128, nr, 88], dt.float32, tag=f"ps{s}")
                        gi = 0
                        for ky in range(3):
                            for kx in range(3):
                                tap = ky * 3 + kx
                                for icc, srcT in enumerate((xs[0], xs[1], dtc)):
                                    kk = 128 if icc < 2 else 64
                                    rhs = bass.AP(
                                        srcT.tensor,
                                        srcT.offset + (r0 + ky + 1) * 92 + kx + 1,
                                        [srcT.ap[0], [92, nr], [1, 88]])
                                    lhs = wt["w_dn1"][0:kk, tap * 3 + icc,
                                                      ocg * 128:(ocg + 1) * 128]
                                    nc.tensor.matmul(ps[:], lhs, rhs,
                                                     start=(gi == 0),
                                                     stop=(gi == 26))
                                    gi += 1
                        ev = wpool.tile([128, nr, 88], dt.bfloat16, tag=f"evn1{s}")
                        nc.scalar.activation(ev[:], ps[:], RELU,
                                             bias=ct["t_dn1"][:, ocg:ocg + 1],
                                             scale=ct["s_dn1"][:, ocg:ocg + 1])
                        mbb = bass.AP(mn1.tensor, mn1.offset + r0,
                                      [mn1.ap[0], [1, nr], [0, 88]])
                        nc.vector.tensor_tensor(
                            out=n1o[ocg][:, r0:r0 + nr, 2:90],
                            in0=ev[:], in1=mbb, op=mybir.AluOpType.mult)

            def stage_dn2(s):
                S = SEGS[s]
                nout, n1o = S["nout"], st[s]["n1o"]
                RPP = 5
                n2o = []
                for g in range(2):
                    n2o.append(bpool.tile([128, nout, 88], dt.bfloat16,
                                          tag=f"n2o{g}_{s}", name=f"n2o{g}_{s}"))
                st[s]["n2o"] = n2o
                dn3 = stage_dn3(s)
                next(dn3)                        # prime: allocates out tiles
                for r0 in range(0, nout, RPP):
                    nr = min(RPP, nout - r0)
                    for ocg in range(2):
                        ps = ppool.tile([128, nr, 88], dt.float32, tag=f"ps{s}")
                        gi = 0
                        for ky in range(3):
                            for kx in range(3):
                                tap = ky * 3 + kx
                                for icc in range(2):
                                    rhs = bass.AP(
                                        n1o[icc].tensor,
                                        n1o[icc].offset + (r0 + ky) * 92 + kx + 1,
                                        [n1o[icc].ap[0], [92, nr], [1, 88]])
                                    lhs = wt["w_dn2"][:, tap * 2 + icc,
                                                      ocg * 128:(ocg + 1) * 128]
                                    nc.tensor.matmul(ps[:], lhs, rhs,
                                                     start=(gi == 0),
                                                     stop=(gi == 17))
                                    gi += 1
                        ev = wpool.tile([128, nr, 88], dt.bfloat16, tag=f"evn2{s}")
                        nc.scalar.activation(ev[:], ps[:], RELU,
                                             bias=ct["t_dn2"][:, ocg:ocg + 1],
                                             scale=ct["s_dn2"][:, ocg:ocg + 1])
                        nc.vector.tensor_copy(n2o[ocg][:, r0:r0 + nr, :], ev[:])
                    try:
                        dn3.send(r0 + nr)        # emit dn3 chunks now ready
                    except StopIteration:
                        pass

            def stage_dn3(s):
                """Generator: receives the count of completed dn2 rows and
                emits dn3+softmax for pixel chunks whose rows are ready."""
                S = SEGS[s]
                nout, n2o = S["nout"], st[s]["n2o"]
                npix = nout * FW
                feat_sb[s] = bpool.tile([128, ((npix + 127) // 128) * CIMG],
                                        dt.bfloat16, tag=f"feat{s}", name=f"feat_sb{s}")
                depth_sb[s] = bpool.tile([128, ((npix + 127) // 128) * DD],
                                         dt.float32, tag=f"depth{s}", name=f"depth_sb{s}")
                n2f = [t.rearrange("p a b -> p (a b)") for t in n2o]
                a0 = 0 if s == 0 else 11
                pcs = (npix + 127) // 128
                rows_done = yield
                for pc in range(pcs):
                    if pc == pcs - 1:
                        # flush all-but-last chunk now so only the final
                        # chunk's output DMA sits in the tail
                        dsl = bass.AP(out_depth.tensor,
                                      out_depth.offset + a0 * DD,
                                      [[17 * DD, 128], [1, (pcs - 1) * DD]])
                        nc.sync.dma_start(
                            out=dsl, in_=depth_sb[s][:, 0:(pcs - 1) * DD])
                        fsl = bass.AP(out_feat.tensor,
                                      out_feat.offset + a0 * CIMG,
                                      [[17 * CIMG, 128], [1, (pcs - 1) * CIMG]])
                        nc.sync.dma_start(
                            out=fsl, in_=feat_sb[s][:, 0:(pcs - 1) * CIMG])
                    m = min(128, npix - pc * 128)
                    # rows needed by pixels [pc*128, pc*128+m)
                    need = (pc * 128 + m - 1) // FW + 1
                    while rows_done < need:
                        rows_done = yield
                    ps = ppool.tile([m, 139], dt.float32, tag=f"ps{s}")
                    for icc in range(2):
                        nc.tensor.matmul(ps[:], n2f[icc][:, pc * 128:pc * 128 + m],
                                         wt["w_dn3"][:, icc, :],
                                         start=(icc == 0), stop=(icc == 1))
                    # add bias via vector then softmax over first 59
                    lg = wpool.tile([m, 139], dt.float32, tag=f"lg{s}")
                    nc.vector.tensor_tensor(out=lg[:], in0=ps[:],
                                            in1=ct["b_dn3"][0:m],
                                            op=mybir.AluOpType.add)
                    mx = wpool.tile([m, 1], dt.float32, tag=f"mx{s}")
                    nc.vector.reduce_max(mx[:], lg[:, 0:DD],
                                         axis=mybir.AxisListType.X, negate=True)
                    ex = wpool.tile([m, DD], dt.float32, tag=f"ex{s}")
                    nc.scalar.activation(ex[:], lg[:, 0:DD],
                                         mybir.ActivationFunctionType.Exp,
                                         bias=mx[:, 0:1], scale=1.0)
                    sm = wpool.tile([m, 1], dt.float32, tag=f"sm{s}")
                    nc.vector.reduce_sum(sm[:], ex[:], axis=mybir.AxisListType.X)
                    rc = wpool.tile([m, 1], dt.float32, tag=f"rc{s}")
                    nc.vector.reciprocal(rc[:], sm[:])
                    nc.vector.tensor_scalar(
                        out=depth_sb[s][0:m, pc * DD:(pc + 1) * DD], in0=ex[:],
                        scalar1=rc[:, 0:1], scalar2=None,
                        op0=mybir.AluOpType.mult)
                    nc.vector.tensor_copy(
                        feat_sb[s][0:m, pc * CIMG:(pc + 1) * CIMG],
                        lg[:, DD:DD + CIMG])

                # final chunk's outputs
                dsl = bass.AP(out_depth.tensor,
                              out_depth.offset + (a0 + pcs - 1) * DD,
                              [[17 * DD, 128], [1, DD]])
                nc.sync.dma_start(out=dsl,
                                  in_=depth_sb[s][:, (pcs - 1) * DD:pcs * DD])
                fsl = bass.AP(out_feat.tensor,
                              out_feat.offset + (a0 + pcs - 1) * CIMG,
                              [[17 * CIMG, 128], [1, CIMG]])
                nc.sync.dma_start(out=fsl,
                                  in_=feat_sb[s][:, (pcs - 1) * CIMG:pcs * CIMG])

            # schedule: dt1 is folded into the host's dph prep; dt2(1)/dt3(0)
            # hide the scr roundtrips; dn3 is fused into dn2 so softmax
            # pipelines under matmuls
            stage_dt2(0)
            scr_write(0)
            stage_dt2(1)
            stage_dt3(0)
            stage_xload(0)
            scr_write(1)
            stage_dt3(1)
            stage_xload(1)
            stage_wload()
            stage_dn1(0)
            stage_dn1(1)
            stage_dn2(0)
            stage_dn2(1)
    nc.compile()
    return nc


# ------------------------------------------------------------ host helpers
def _host_geometry(rots, trans, intr, post_rots, post_trans):
    import jax
    import jax.numpy as jnp
    with jax.default_device(jax.devices("cpu")[0]):
        f32 = jnp.float32
        ds = jnp.arange(1.0, 60.0, 1.0, dtype=f32)
        xs = jnp.linspace(0.0, IW - 1.0, FW, dtype=f32)
        ys = jnp.linspace(0.0, IH - 1.0, FH, dtype=f32)
        dm = jnp.broadcast_to(ds[:, None, None], (DD, FH, FW))
        xm = jnp.broadcast_to(xs[None, None, :], (DD, FH, FW))
        ym = jnp.broadcast_to(ys[None, :, None], (DD, FH, FW))
        fr = jnp.stack([xm, ym, dm], -1)
        pts = fr[None, None] - jnp.asarray(post_trans)[:, :, None, None, None, :]
        pts = jnp.einsum("bnij,bndhwj->bndhwi",
                         jnp.linalg.inv(jnp.asarray(post_rots)), pts)
        pts = jnp.concatenate([pts[..., :2] * pts[..., 2:3], pts[..., 2:3]], -1)
        comb = jnp.einsum("bnij,bnjk->bnik", jnp.asarray(rots),
                          jnp.linalg.inv(jnp.asarray(intr)))
        pts = jnp.einsum("bnij,bndhwj->bndhwi", comb, pts) \
            + jnp.asarray(trans)[:, :, None, None, None, :]
        lo = jnp.array([XY0, XY0, Z0], dtype=f32)
        dxv = jnp.array([DXY, DXY, DZ], dtype=f32)
        g = ((pts - lo) / dxv).astype(jnp.int32).reshape(-1, 3)
        kept = ((g[:, 0] >= 0) & (g[:, 0] < NX) & (g[:, 1] >= 0) & (g[:, 1] < NX)
                & (g[:, 2] >= 0) & (g[:, 2] < NZ))
        flat = (g[:, 2] * NX + g[:, 0]) * NX + g[:, 1]
        return np.asarray(flat, np.int64), np.asarray(kept)


def _prep_a_inputs(inputs):
    """Build per-core input maps for launch A."""
    d = np.asarray(inputs["d"], np.float32).reshape(N, IH, IW)
    x_img = np.asarray(inputs["x_img"], np.float32)

    # dt1 folded affine: relu(alpha*d + beta), alpha = s*w, beta = s*b + t
    a1 = (inputs["dt1_s"] * inputs["dt1_w"][:, 0, 0, 0]).astype(np.float32)
    b1 = (inputs["dt1_s"] * inputs["dt1_b"] + inputs["dt1_t"]).astype(np.float32)
    cab = np.arange(128)
    dt1_alpha = a1[cab // 16][:, None]
    dt1_beta = b1[cab // 16][:, None]

    def wprep_dt2():
        w = np.asarray(inputs["dt2_w"], np.float32)      # [32,8,5,5]
        out = np.zeros((4, 128, 32), np.float32)
        for ky in range(5):
            for kx in range(5):
                a, dky = ky % 4, ky // 4
                bph, dmx = (kx + 2) % 4, (kx + 2) // 4
                g = dky * 2 + dmx
                rows = (np.arange(8)) * 16 + a * 4 + bph
                out[g, rows, :] = w[:, :, ky, kx].T
        return out.astype(bf16)

    def wprep_dt3():
        w = np.asarray(inputs["dt3_w"], np.float32)      # [64,32,5,5]
        out = np.zeros((9, 128, 64), np.float32)
        for ky in range(5):
            for kx in range(5):
                a, dky = ky % 2, ky // 2
                bph, dmx = kx % 2, (kx + 2) // 2 - 1
                g = dky * 3 + dmx
                rows = (a * 2 + bph) * 32 + np.arange(32)
                out[g, rows, :] = w[:, :, ky, kx].T
        return out.astype(bf16)

    def wprep_3x3(w, icc_sizes):
        O, I = w.shape[0], w.shape[1]
        nic = len(icc_sizes)
        out = np.zeros((9, nic, 128, O), np.float32)
        for ky in range(3):
            for kx in range(3):
                tap = ky * 3 + kx
                ic0 = 0
                for icc, sz in enumerate(icc_sizes):
                    out[tap, icc, 0:sz, :] = w[:, ic0:ic0 + sz, ky, kx].T
                    ic0 += sz
        return out.astype(bf16)

    # NOTE: dn1 input concat order is [dt3(64) | x_img(256)] in the reference;
    # our matmul chunks are (x0:128, x1:128, dt3:64) -> weight cols must match:
    w_dn1_full = np.asarray(inputs["dn1_w"], np.float32)
    w_dn1 = np.zeros((9, 3, 128, 256), np.float32)
    for ky in range(3):
        for kx in range(3):
            tap = ky * 3 + kx
            w_dn1[tap, 0, :, :] = w_dn1_full[:, 64:192, ky, kx].T
            w_dn1[tap, 1, :, :] = w_dn1_full[:, 192:320, ky, kx].T
            w_dn1[tap, 2, 0:64, :] = w_dn1_full[:, 0:64, ky, kx].T
    w_dn1 = w_dn1.astype(bf16)
    w_dn2 = wprep_3x3(np.asarray(inputs["dn2_w"], np.float32), [128, 128])
    w_dn3 = np.asarray(inputs["dn3_w"], np.float32)[:, :, 0, 0]  # [139, 256]
    w_dn3p = np.zeros((2, 128, 139), np.float32)
    w_dn3p[0] = w_dn3[:, 0:128].T
    w_dn3p[1] = w_dn3[:, 128:256].T

    def fold_bias(b, s, t):
        # conv bias b then bn scale/shift: relu(s*(x+b) + t) = relu(s*x + (s*b+t))
        return np.asarray(s, np.float32), np.asarray(s * b + t, np.float32)

    s2, t2 = fold_bias(inputs["dt2_b"], inputs["dt2_s"], inputs["dt2_t"])
    s3, t3 = fold_bias(inputs["dt3_b"], inputs["dt3_s"], inputs["dt3_t"])
    sn1, tn1 = fold_bias(inputs["dn1_b"], inputs["dn1_s"], inputs["dn1_t"])
    sn2, tn2 = fold_bias(inputs["dn2_b"], inputs["dn2_s"], inputs["dn2_t"])
    b_dn3 = np.broadcast_to(np.asarray(inputs["dn3_b"], np.float32)[None, :],
                            (128, 139)).copy()

    consts = np.zeros((128, 153), np.float32)
    consts[:, 0] = dt1_alpha[:, 0]
    consts[:, 1] = dt1_beta[:, 0]
    consts[:, 2] = np.tile(s2, 4)
    consts[:, 3] = np.tile(t2, 4)
    consts[:, 4] = np.tile(s3, 2)
    consts[:, 5] = np.tile(t3, 2)
    consts[:, 6:8] = sn1.reshape(2, 128).T
    consts[:, 8:10] = tn1.reshape(2, 128).T
    consts[:, 10:12] = sn2.reshape(2, 128).T
    consts[:, 12:14] = tn2.reshape(2, 128).T
    consts[:, 14:153] = b_dn3
    shared = dict(
        consts=consts,
        w_dt2=wprep_dt2(), w_dt3=wprep_dt3(), w_dn1=w_dn1, w_dn2=w_dn2,
        w_dn3=w_dn3p.astype(bf16),
    )

    maps = []
    for c in range(NCORES):
        m = dict(shared)
        for s, (cam, h0) in enumerate([SEG_A[c], SEG_B[c]]):
            S = SEGS[s]
            d0 = 8 * h0 - 34
            dseg = np.zeros((S["nd"], 712), np.float32)
            vseg = np.zeros((S["nd"], 712), bool)
            lo, hi = max(0, d0), min(IH, d0 + S["nd"])
            if hi > lo:
                dseg[lo - d0:hi - d0, 4:708] = d[cam, lo:hi]
                vseg[lo - d0:hi - d0, 4:708] = True
            nq = S["nq"]
            ph = dseg.reshape(nq, 4, 178, 4)[:, :, :177, :]     # ry a rx b
            ph = ph.transpose(1, 3, 0, 2)                        # a b ry rx
            vph = vseg.reshape(nq, 4, 178, 4)[:, :, :177, :].transpose(1, 3, 0, 2)
            # dt1 applied on host: relu(alpha*d + beta), zero at pads
            dphc = np.where(vph[None],
                            np.maximum(a1[:, None, None, None, None] * ph[None]
                                       + b1[:, None, None, None, None], 0.0),
                            0.0)                                 # [8,4,4,nq,177]
            m[f"dph{s}"] = dphc.reshape(128, nq * 177).astype(bf16)
            q0, t0, r0 = 2 * h0 - 8, h0 - 3, h0 - 1
            qr = np.arange(S["nt2"]) + q0
            m2m = np.broadcast_to(((qr >= 0) & (qr < 64))[None, :],
                                  (128, S["nt2"]))
            tr = np.arange(S["nt3"]) + t0
            m3m = np.broadcast_to(((tr >= 0) & (tr < FH))[None, :],
                                  (128, S["nt3"]))
            rr = np.arange(S["nn1"]) + r0
            mn1m = np.broadcast_to(((rr >= 0) & (rr < FH))[None, :],
                                   (128, S["nn1"]))
            m[f"masks{s}"] = np.concatenate(
                [m2m, m3m, mn1m], axis=1).astype(bf16)
            xseg = np.zeros((2, 128, S["nt3"], 92), np.float32)
            lo2, hi2 = max(0, t0), min(FH, t0 + S["nt3"])
            if hi2 > lo2:
                xseg[:, :, lo2 - t0:hi2 - t0, 2:90] = \
                    x_img[cam, :, lo2:hi2, :].reshape(2, 128, hi2 - lo2, FW)
            m[f"xseg{s}"] = xseg.reshape(2, 128, S["nt3"] * 92).astype(bf16)
        maps.append(m)
    return maps


# ---------------------------------------------------------------- launch B
def build_launch_b(sizes):
    """Per chunk k: [128pix x 80ch] stationary feat tile x host-built
    [128pix x sizes[k] voxel-slot] depth-weight matrix -> [80, nv] window
    sums. W and out use packed (variable-size) layouts; W loads in a few
    batched DMAs, out in one."""
    nc = bacc.Bacc("TRN2", target_bir_lowering=False, debug=False,
                   num_devices=NCORES)
    NCH = len(sizes)
    offs = np.concatenate([[0], np.cumsum(sizes)]).astype(int)
    S = int(offs[-1])
    wmat = nc.dram_tensor("wmat", [128, S], dt.bfloat16,
                          kind="ExternalInput").ap()
    feats = nc.dram_tensor("feats", [128, NCH, CIMG], dt.bfloat16,
                           kind="ExternalInput").ap()
    owin = nc.dram_tensor("owin", [CIMG, S], dt.bfloat16,
                          kind="ExternalOutput").ap()
    NB = 4                                   # W DMA batches
    bnd = [int(round(NCH * i / NB)) for i in range(NB + 1)]
    with tile.TileContext(nc) as tc:
        with tc.tile_pool(name="const", bufs=1) as cpool, \
             tc.tile_pool(name="ps", bufs=4, space="PSUM") as pp:
            ft = cpool.tile([128, NCH, CIMG], dt.bfloat16, name="ft")
            nc.sync.dma_start(out=ft[:], in_=feats)
            wt = cpool.tile([128, S], dt.bfloat16, name="wt")
            for b in range(NB):
                lo, hi = offs[bnd[b]], offs[bnd[b + 1]]
                if hi > lo:
                    nc.sync.dma_start(out=wt[:, lo:hi], in_=wmat[:, lo:hi])
            ot = cpool.tile([CIMG, S], dt.bfloat16, name="ot")
            for k in range(NCH):
                nv, o0 = int(sizes[k]), int(offs[k])
                ps = pp.tile([CIMG, 512], dt.float32, tag="ps", name="ps")
                nc.tensor.matmul(ps[:, 0:nv], ft[:, k, :], wt[:, o0:o0 + nv],
                                 start=True, stop=True)
                if k % 2 == 0:
                    nc.scalar.activation(ot[:, o0:o0 + nv], ps[:, 0:nv],
                                         mybir.ActivationFunctionType.Copy)
                else:
                    nc.vector.tensor_copy(ot[:, o0:o0 + nv], ps[:, 0:nv])
            nc.sync.dma_start(out=owin, in_=ot[:])
    nc.compile()
    return nc


# ---------------------------------------------------------------- launch C
C_OUT_ROWS = 23              # ds2-out rows per core (8*23 = 184 >= 180)


def build_launch_c():
    nc = bacc.Bacc("TRN2", target_bir_lowering=False, debug=False,
                   num_devices=NCORES)
    NR1 = C_OUT_ROWS + 2                         # ds1-out rows incl halo (25)
    NRP = 2 * NR1 + 1                            # pooled rows needed (51)
    slab = nc.dram_tensor("slab", [CIMG, NRP, 362], dt.bfloat16,
                          kind="ExternalInput").ap()
    m1 = nc.dram_tensor("m1", [128, NR1], dt.bfloat16, kind="ExternalInput").ap()
    wd1 = nc.dram_tensor("wd1", [CIMG, 9, CIMG], dt.bfloat16,
                         kind="ExternalInput").ap()
    wd2 = nc.dram_tensor("wd2", [CIMG, 9, CIMG], dt.bfloat16,
                         kind="ExternalInput").ap()
    sb1 = nc.dram_tensor("sb1", [CIMG, 2], dt.float32, kind="ExternalInput").ap()
    sb2 = nc.dram_tensor("sb2", [CIMG, 2], dt.float32, kind="ExternalInput").ap()
    yout = nc.dram_tensor("yout", [CIMG, C_OUT_ROWS, 180], dt.float32,
                          kind="ExternalOutput").ap()
    RELU = mybir.ActivationFunctionType.Relu
    with tile.TileContext(nc) as tc:
        with tc.tile_pool(name="const", bufs=1) as cpool,              tc.tile_pool(name="work", bufs=2) as wp,              tc.tile_pool(name="big", bufs=1) as bp,              tc.tile_pool(name="ps", bufs=3, space="PSUM") as pp:
            # weights/consts first so ds1 can start on the first slab chunk
            w1 = cpool.tile([CIMG, 9, CIMG], dt.bfloat16, name="w1")
            nc.sync.dma_start(out=w1[:], in_=wd1)
            sb1t = cpool.tile([CIMG, 2], dt.float32, name="sb1t")
            nc.sync.dma_start(out=sb1t[:], in_=sb1)
            m1t = wp.tile([128, NR1], dt.bfloat16, name="m1t")
            nc.sync.dma_start(out=m1t[:], in_=m1)
            slabt = bp.tile([CIMG, NRP, 362], dt.bfloat16, name="slabt")
            for rr in range(0, NRP, 9):
                nrr = min(9, NRP - rr)
                nc.sync.dma_start(out=slabt[:, rr:rr + nrr, :],
                                  in_=slab[:, rr:rr + nrr, :])
            w2 = cpool.tile([CIMG, 9, CIMG], dt.bfloat16, name="w2")
            nc.sync.dma_start(out=w2[:], in_=wd2)
            sb2t = cpool.tile([CIMG, 2], dt.float32, name="sb2t")
            nc.sync.dma_start(out=sb2t[:], in_=sb2)
            h1 = bp.tile([CIMG, NR1, 182], dt.bfloat16, name="h1")
            nc.vector.memset(h1[:, :, 0:1], 0.0)
            nc.vector.memset(h1[:, :, 181:182], 0.0)
            # ds1: stride-2 3x3; out row t reads slab rows 2t..2t+2 (slab row 0
            # = pooled row 2o0-3, so out row t (global o0-1+t) reads
            # 2(o0-1+t)-1..+1 - (2o0-3) = 2t..2t+2); col c reads 2c..2c+2
            RP = 2
            for t0 in range(0, NR1, RP):
                nr = min(RP, NR1 - t0)
                ps = pp.tile([CIMG, nr, 180], dt.float32, tag="ps1", name="ps")
                gi = 0
                for ky in range(3):
                    for kx in range(3):
                        rhs = bass.AP(slabt.tensor,
                                      slabt.offset + (2 * t0 + ky) * 362 + kx,
                                      [slabt.ap[0], [2 * 362, nr], [2, 180]])
                        nc.tensor.matmul(ps[:], w1[:, ky * 3 + kx, :], rhs,
                                         start=(gi == 0), stop=(gi == 8))
                        gi += 1
                ev = wp.tile([CIMG, nr, 180], dt.bfloat16, tag="ev", name="ev")
                nc.scalar.activation(ev[:], ps[:], RELU, bias=sb1t[:, 1:2],
                                     scale=sb1t[:, 0:1])
                mbb = bass.AP(m1t.tensor, m1t.offset + t0,
                              [[m1t.ap[0][0], CIMG], [1, nr], [0, 180]])
                nc.vector.tensor_tensor(out=h1[:, t0:t0 + nr, 1:181],
                                        in0=ev[:], in1=mbb,
                                        op=mybir.AluOpType.mult)
            # ds2: 3x3 pad 1: out row o reads h1 rows o..o+2, col c: c..c+2
            yo = bp.tile([CIMG, C_OUT_ROWS, 180], dt.float32, name="yo")
            for o0 in range(0, C_OUT_ROWS, RP):
                nr = min(RP, C_OUT_ROWS - o0)
                ps = pp.tile([CIMG, nr, 180], dt.float32, tag="ps2", name="ps")
                gi = 0
                for ky in range(3):
                    for kx in range(3):
                        rhs = bass.AP(h1.tensor,
                                      h1.offset + (o0 + ky) * 182 + kx,
                                      [h1.ap[0], [182, nr], [1, 180]])
                        nc.tensor.matmul(ps[:], w2[:, ky * 3 + kx, :], rhs,
                                         start=(gi == 0), stop=(gi == 8))
                        gi += 1
                nc.scalar.activation(yo[:, o0:o0 + nr, :], ps[:], RELU,
                                     bias=sb2t[:, 1:2], scale=sb2t[:, 0:1])
                if (o0 // RP) % 3 == 2 or o0 + nr >= C_OUT_ROWS:
                    lo = (o0 // (3 * RP)) * 3 * RP
                    nc.sync.dma_start(out=yout[:, lo:o0 + nr, :],
                                      in_=yo[:, lo:o0 + nr, :])
    nc.compile()
    return nc


_CACHE = {}


def run_launch_a(inputs):
    if "A" not in _CACHE:
        _CACHE["A"] = build_launch_a()
    nc = _CACHE["A"]
    maps = _prep_a_inputs(inputs)
    res = run_bass_kernel_spmd(nc, maps, list(range(NCORES)))
    depth = np.zeros((NPIX, DD), np.float32)
    feat = np.zeros((NPIX, CIMG), np.float32)
    for c in range(NCORES):
        r = res.results[c]
        for s, (cam, h0) in enumerate([SEG_A[c], SEG_B[c]]):
            S = SEGS[s]
            npix = S["nout"] * FW
            base = (cam * FH + h0) * FW
            a0, pcs = (0, 11) if s == 0 else (11, 6)
            dsg = r["out_depth"][:, a0:a0 + pcs].transpose(1, 0, 2)
            depth[base:base + npix] = dsg.reshape(pcs * 128, DD)[:npix]
            fsg = r["out_feat"][:, a0:a0 + pcs].transpose(1, 0, 2)
            feat[base:base + npix] = fsg.reshape(pcs * 128, CIMG)[:npix]
    return depth, feat


def _build_chunks(flat, kept, depth_rows):
    """Group points by (camera, column-block); per group build the
    [pix, voxel-slot] depth-weight matrix over the group's voxel union.
    Splits column blocks whose union exceeds the PSUM window (512)."""
    fl = flat.reshape(N, DD, FH, FW)
    kp = kept.reshape(N, DD, FH, FW)
    chunks = []                      # (pix_ids, Wdense[npix, nv], vox_ids)

    def add_group(n, w0, w1):
        nw = w1 - w0
        f = fl[n, :, :, w0:w1]                       # [DD, FH, nw]
        k = kp[n, :, :, w0:w1]
        vids = np.unique(f[k])
        if len(vids) > 512 and nw > 1:
            mid = w0 + nw // 2
            add_group(n, w0, mid)
            add_group(n, mid, w1)
            return
        nv = max(len(vids), 1)
        # pixel local idx = (w - w0) * FH + h; point (d, h, w)
        slot = np.searchsorted(vids, f[k]) if len(vids) else np.zeros(0, np.int64)
        dd, hh, ww = np.nonzero(k)
        pix_loc = ww * FH + hh
        pixcol = n * FH * FW + hh * FW + (ww + w0)
        dep = depth_rows[pixcol, dd]
        Wd = np.bincount(pix_loc * nv + slot, weights=dep,
                         minlength=nw * FH * nv).reshape(nw * FH, nv)
        pix_ids = (n * FH * FW + np.arange(FH)[None, :] * FW
                   + (w0 + np.arange(nw))[:, None]).reshape(-1)
        chunks.append((pix_ids, Wd, vids))

    for n in range(N):
        for w0 in range(0, FW, 4):
            add_group(n, w0, w0 + 4)
    return chunks


def _prep_b_inputs(chunks, featflat_bf):
    """Balance chunks across cores by window size; build per-core maps with
    the packed per-slot layout (chunk k size = max over cores, desc-sorted)."""
    order = sorted(range(len(chunks)), key=lambda i: -chunks[i][1].shape[1])
    load = np.zeros(NCORES, np.int64)
    per_core = [[] for _ in range(NCORES)]
    for i in order:
        c = int(np.argmin(load))
        per_core[c].append(i)
        load[c] += chunks[i][1].shape[1]
    NCH = max(len(p) for p in per_core)
    sizes = np.zeros(NCH, np.int64)
    for p in per_core:
        for k, i in enumerate(p):
            sizes[k] = max(sizes[k], chunks[i][1].shape[1])
    sizes = (sizes + 15) // 16 * 16
    offs = np.concatenate([[0], np.cumsum(sizes)]).astype(int)
    S = int(offs[-1])
    maps, scatter = [], []
    for c in range(NCORES):
        wm = np.zeros((128, S), bf16)
        ft = np.zeros((128, NCH, CIMG), bf16)
        sc = []
        for k, i in enumerate(per_core[c]):
            pix_ids, Wd, vids = chunks[i]
            npix, nv = Wd.shape
            wm[0:npix, offs[k]:offs[k] + nv] = Wd
            ft[0:npix, k, :] = featflat_bf[pix_ids]
            sc.append((int(offs[k]), vids))
        maps.append(dict(wmat=wm, feats=ft))
        scatter.append(sc)
    return maps, scatter, tuple(int(s) for s in sizes)


def _prep_c_inputs(inputs, pooled_t):
    """pooled_t: [CIMG, 360, 360] f32 -> per-core slabs + masks + weights."""
    NR1 = C_OUT_ROWS + 2
    NRP = 2 * NR1 + 1
    w1 = np.asarray(inputs["ds1_w"], np.float32)
    w2 = np.asarray(inputs["ds2_w"], np.float32)
    # wd1/wd2: [ic, tap, oc]
    wd1 = np.ascontiguousarray(w1.transpose(1, 2, 3, 0).reshape(CIMG, 9, CIMG))
    wd2 = np.ascontiguousarray(w2.transpose(1, 2, 3, 0).reshape(CIMG, 9, CIMG))
    sb1 = np.stack([np.asarray(inputs["ds1_s"], np.float32),
                    np.asarray(inputs["ds1_t"], np.float32)], 1)
    sb2 = np.stack([np.asarray(inputs["ds2_s"], np.float32),
                    np.asarray(inputs["ds2_t"], np.float32)], 1)
    shared = dict(wd1=wd1.astype(bf16), wd2=wd2.astype(bf16), sb1=sb1, sb2=sb2)
    maps = []
    pt_bf = pooled_t.astype(bf16)
    for c in range(NCORES):
        o0g = C_OUT_ROWS * c
        p0 = 2 * o0g - 3
        slab = np.zeros((CIMG, NRP, 362), bf16)
        lo, hi = max(0, p0), min(NX, p0 + NRP)
        if hi > lo:
            slab[:, lo - p0:hi - p0, 1:361] = pt_bf[:, lo:hi, :]
        t1g = np.arange(NR1) + (o0g - 1)
        m1 = np.broadcast_to(((t1g >= 0) & (t1g < 180))[None, :],
                             (128, NR1)).astype(bf16)
        maps.append(dict(shared, slab=slab, m1=np.ascontiguousarray(m1)))
    return maps


def kernel(**inputs):
    inputs = {k: np.asarray(v) for k, v in inputs.items()}
    flat, kept = _host_geometry(inputs["cam2lidar_rots"],
                                inputs["cam2lidar_trans"], inputs["intrins"],
                                inputs["post_rots"], inputs["post_trans"])
    depth_rows, feat_rows = run_launch_a(inputs)
    featflat_bf = feat_rows.astype(bf16)

    chunks = _build_chunks(flat, kept, depth_rows)
    bmaps, scatter, sizes = _prep_b_inputs(chunks, featflat_bf)
    key = ("B", sizes)
    if key not in _CACHE:
        _CACHE[key] = build_launch_b(sizes)
    res_b = run_bass_kernel_spmd(_CACHE[key], bmaps, list(range(NCORES)))

    allvox = np.concatenate([vids for c in range(NCORES)
                             for _, vids in scatter[c]])
    allval = np.concatenate(
        [res_b.results[c]["owin"][:, o0:o0 + len(vids)].T.astype(np.float32)
         for c in range(NCORES) for o0, vids in scatter[c]])
    o = np.argsort(allvox, kind="stable")
    allvox, allval = allvox[o], allval[o]
    starts = np.flatnonzero(np.r_[True, allvox[1:] != allvox[:-1]])
    pooled = np.zeros((NX * NX, CIMG), np.float32)
    pooled[allvox[starts]] = np.add.reduceat(allval, starts, axis=0)
    pooled_t = np.ascontiguousarray(
        pooled.reshape(NX, NX, CIMG).transpose(2, 0, 1))

    if "C" not in _CACHE:
        _CACHE["C"] = build_launch_c()
    cmaps = _prep_c_inputs(inputs, pooled_t)
    res_c = run_bass_kernel_spmd(_CACHE["C"], cmaps, list(range(NCORES)))
    out = np.zeros((1, CIMG, 180, 180), np.float32)
    for c in range(NCORES):
        o0g = C_OUT_ROWS * c
        nr = min(C_OUT_ROWS, 180 - o0g)
        if nr > 0:
            out[0, :, o0g:o0g + nr, :] = res_c.results[c]["yout"][:, 0:nr, :]
    return out



# revision 98
# speedup vs baseline: 1.0548x; 1.0106x over previous
"""DepthLSSTransform Trainium kernel: 3 SPMD launches over 8 NeuronCores.

Launch A: per-camera conv pipeline (dtransform + depthnet + softmax) on
          24-row bands (one 16-row + one 8-row segment per core).
Launch B: bev_pool segment-sum via one-hot matmuls over a host-built
          virtual-window schedule (sorted-by-voxel points).
Launch C: BEV downsample convs, spatially sharded.
Host: geometry/voxel indices, scheduling, gathers, folds (orchestration).
"""
import numpy as np
import ml_dtypes

import concourse.bass as bass
import concourse.tile as tile
from concourse import bacc, mybir
from concourse.bass_utils import run_bass_kernel_spmd

dt = mybir.dt
bf16 = ml_dtypes.bfloat16

# ---- problem constants (hardcoded per contract) ----
B, N = 1, 6
CIN, CIMG, DD = 256, 80, 59
FH, FW, IH, IW = 32, 88, 256, 704
XY0, DXY, NX = -54.0, 0.3, 360
Z0, DZ, NZ = -10.0, 20.0, 1
NPTS = N * DD * FH * FW
NPIX = N * FH * FW
NCORES = 8
QV = 4                      # chunks of 128 points per virtual window

# per-core segments: (camera, h0) for seg A (16 rows) and seg B (8 rows)
SEG_A = [(0, 0), (1, 0), (1, 16), (2, 16), (3, 0), (4, 0), (4, 16), (5, 16)]
SEG_B = [(0, 16), (0, 24), (2, 0), (2, 8), (3, 16), (3, 24), (5, 0), (5, 8)]
# band pixel ranges in global row order (row = n*32 + h)
ROWS_OF_CORE = [[(SEG_A[c][0] * FH + SEG_A[c][1] + r) for r in range(16)] +
                [(SEG_B[c][0] * FH + SEG_B[c][1] + r) for r in range(8)]
                for c in range(NCORES)]

# segment geometry: rows16 segment: d rows [8h0-34, 8h0+158) (192), dt2 out
# rows [2h0-8, 2h0+39) (47), dt3 [h0-3, h0+19) (22), dn1 [h0-1, h0+17) (18)
SEGS = [dict(nout=16, nd=192, nq=48, nt2=47, nt3=22, nn1=18),
        dict(nout=8, nd=128, nq=32, nt2=31, nt3=14, nn1=10)]


def _seg_ranges(h0, S):
    return dict(d0=8 * h0 - 34, q0=2 * h0 - 8, t0=h0 - 3, r0=h0 - 1, o0=h0)


# ---------------------------------------------------------------- launch A
def build_launch_a(debug=False, psum_bufs=3, work_bufs=3, stages=9):
    nc = bacc.Bacc("TRN2", target_bir_lowering=False, debug=False,
                   num_devices=NCORES)
    AP = {}

    def inp(name, shape, dtype=dt.bfloat16):
        AP[name] = nc.dram_tensor(name, shape, dtype, kind="ExternalInput").ap()
        return AP[name]

    # per segment inputs (s = 0: 16-row, 1: 8-row); flat free dims so DMAs
    # are single-descriptor-per-partition and tile deps stay precise
    for s, S in enumerate(SEGS):
        inp(f"dph{s}", [128, S["nq"] * 177])
        inp(f"masks{s}", [128, S["nt2"] + S["nt3"] + S["nn1"]])
        inp(f"xseg{s}", [2, 128, S["nt3"] * 92])        # x_img slice (padded)
    # packed f32 constants: [alpha, beta, s_dt2, t_dt2, s_dt3, t_dt3,
    #  s_dn1(2), t_dn1(2), s_dn2(2), t_dn2(2), b_dn3(139)] -> [128, 153]
    inp("consts", [128, 153], dt.float32)
    # conv weights (host-prepped layouts)
    inp("w_dt2", [4, 128, 32])                          # groups (dky,dmx)
    inp("w_dt3", [9, 128, 64])
    inp("w_dn1", [9, 3, 128, 256])                      # tap, icchunk(128,128,64pad) -> 256
    inp("w_dn2", [9, 2, 128, 256])
    inp("w_dn3", [2, 128, 139])

    DBG = {}
    dbg_specs = [] if not debug else [("dbg_t1", [128, SEGS[0]["nq"], 177], dt.bfloat16),
                        ("dbg_dt2o", [32, SEGS[0]["nt2"] + 1, 180], dt.bfloat16),
                        ("dbg_dtc", [64, SEGS[0]["nt3"], 92], dt.bfloat16),
                        ("dbg_n1o", [128, SEGS[0]["nn1"], 92], dt.bfloat16),
                        ("dbg_n2o", [128, SEGS[0]["nout"], 88], dt.bfloat16)]
    for nm, sh, dty in dbg_specs:
        DBG[nm] = nc.dram_tensor(nm, sh, dty, kind="ExternalOutput").ap()
    # chunk-major outputs: pixel (a*128+p) of segment s at [p, a0_s + a, :]
    out_depth = nc.dram_tensor("out_depth", [128, 17, DD], dt.float32,
                               kind="ExternalOutput").ap()
    out_feat = nc.dram_tensor("out_feat", [128, 17, CIMG], dt.bfloat16,
                              kind="ExternalOutput").ap()

    # HBM scratch, phase-major: [c32, a2, b2, q', x90] (q' = dt2-row // 2)
    scr = {}
    for s, S in enumerate(SEGS):
        scr[f"dt2o{s}"] = nc.dram_tensor(
            f"dt2o{s}", [32, 2, 2, (S["nt2"] + 1) // 2, 90], dt.bfloat16).ap()

    RELU = mybir.ActivationFunctionType.Relu
    with tile.TileContext(nc) as tc:
        with tc.tile_pool(name="const", bufs=1) as cpool, \
             tc.tile_pool(name="work", bufs=work_bufs) as wpool, \
             tc.tile_pool(name="big", bufs=1) as bpool, \
             tc.tile_pool(name="psum", bufs=2, space="PSUM") as ppool, \
             tc.tile_pool(name="psum2", bufs=4, space="PSUM") as ppool2:
            # ---- DMA issue order = consumption order (the SP queue and the
            # modeled DMA engines serialize; early-stage inputs must land first)
            cts = cpool.tile([128, 153], dt.float32, name="cts")
            nc.sync.dma_start(out=cts[:], in_=AP["consts"])
            # tiny activation right away so the act-table load happens while
            # the first dph chunk is still in flight
            warm = wpool.tile([128, 1], dt.float32, tag="warm", name="warm")
            nc.scalar.activation(warm[:], cts[:, 0:1], RELU)
            ct = {"dt1_alpha": cts[:, 0:1], "dt1_beta": cts[:, 1:2],
                  "s_dt2": cts[:, 2:3], "t_dt2": cts[:, 3:4],
                  "s_dt3": cts[:, 4:5], "t_dt3": cts[:, 5:6],
                  "s_dn1": cts[:, 6:8], "t_dn1": cts[:, 8:10],
                  "s_dn2": cts[:, 10:12], "t_dn2": cts[:, 12:14],
                  "b_dn3": cts[:, 14:153]}
            wt = {}

            def load_w(nm, pat):
                sh = list(AP[nm].shape)
                wt[nm] = cpool.tile([sh[-2], int(np.prod(sh[:-2])), sh[-1]],
                                    dt.bfloat16, tag=nm, name=f'wt_{nm}')
                nc.sync.dma_start(out=wt[nm][:], in_=AP[nm].rearrange(pat))

            # first dph chunk small so dt2 starts ASAP; host has already
            # applied dt1 (relu(alpha*d+beta), pads zeroed) into dph.
            # The big dn-weights are issued later (stage_wload) so they don't
            # sit ahead of the dt2->dt3 scratch roundtrip in the serial DMA
            # stream.
            QCHUNKS = {0: [8, 12, 14, 14], 1: [8, 12, 12]}
            dphs, malls = {}, {}

            def load_dph(s):
                S = SEGS[s]
                nq = S["nq"]
                dphs[s] = bpool.tile([128, nq * 177], dt.bfloat16,
                                     tag=f"dph{s}", name=f"dph{s}")
                qq = 0
                for nqq in QCHUNKS[s]:
                    nc.sync.dma_start(
                        out=dphs[s][:, qq * 177:(qq + nqq) * 177],
                        in_=AP[f"dph{s}"][:, qq * 177:(qq + nqq) * 177])
                    qq += nqq
                malls[s] = wpool.tile([128, S["nt2"] + S["nt3"] + S["nn1"]],
                                      dt.bfloat16, tag=f"msk{s}", name="mall")
                nc.sync.dma_start(out=malls[s][:], in_=AP[f"masks{s}"])

            load_w("w_dt2", "g p o -> p g o")
            load_dph(0)
            load_dph(1)
            load_w("w_dt3", "g p o -> p g o")

            def stage_wload():
                load_w("w_dn1", "t i p o -> p (t i) o")
                load_w("w_dn2", "t i p o -> p (t i) o")
                load_w("w_dn3", "g p o -> p g o")

            feat_sb = {}
            depth_sb = {}
            st = {s: {} for s in range(len(SEGS))}

            def stage_dt2(s):
                S = SEGS[s]
                nt2, t1, mall = S["nt2"], dphs[s], malls[s]
                Q2 = (nt2 + 1) // 2
                # phase-major layout [c32, a2, b2, q', x90]: row q=(2q'+a),
                # col c at (b=c%2, x=c//2+1); makes scr write + ph3 reads
                # fully contiguous per partition
                o2 = bpool.tile([32, 2, 2, Q2, 90], dt.bfloat16, tag=f"o2{s}",
                                name=f"o2{s}")
                st[s]["o2"] = o2
                o2f = o2.rearrange("p a b q x -> p (a b q) x")
                nc.vector.memset(o2f[:, :, 0:1], 0.0)          # x pad left
                nc.vector.memset(o2f[:, :, 89:90], 0.0)        # x pad right
                nc.vector.memset(o2[:, 1, :, Q2 - 1, :], 0.0)  # pad row q=nt2
                m2 = bass.AP(mall.tensor, mall.offset, [mall.ap[0], [1, nt2]])
                RPP2 = 2
                for q0 in range(0, nt2, RPP2):
                    nr = min(RPP2, nt2 - q0)
                    ps = ppool2.tile([32, nr, 176], dt.float32, tag="ps2",
                                     name="ps2")
                    gi = 0
                    for dky in range(2):
                        for dmx in range(2):
                            g = dky * 2 + dmx
                            rhs = bass.AP(
                                t1.tensor, t1.offset + (q0 + dky) * 177 + dmx,
                                [t1.ap[0], [177, nr], [1, 176]])
                            nc.tensor.matmul(ps[:], wt["w_dt2"][:, g, :], rhs,
                                             start=(gi == 0), stop=(gi == 3))
                            gi += 1
                    ev = wpool.tile([32, nr, 176], dt.bfloat16, tag=f"ev2{s}")
                    nc.scalar.activation(ev[:], ps[:], RELU,
                                         bias=ct["t_dt2"][0:32, 0:1],
                                         scale=ct["s_dt2"][0:32, 0:1])
                    mbb = bass.AP(m2.tensor, m2.offset + q0,
                                  [[m2.ap[0][0], 32], [1, nr], [0, 176]])
                    # rows (q0, q0+1) -> a=(0,1) at q'=q0//2; c -> (x, b)
                    o2dst = bass.AP(o2.tensor,
                                    o2.offset + (q0 // 2) * 90 + 1,
                                    [[o2.ap[0][0], 32], [2 * Q2 * 90, nr],
                                     [1, 88], [Q2 * 90, 2]])
                    nc.vector.tensor_tensor(out=o2dst, in0=ev[:], in1=mbb,
                                            op=mybir.AluOpType.mult)

            def scr_write(s):
                # on the idle GPSIMD (SWDGE) queue: its sem wait must not
                # head-of-line-block the streaming SP DMA queue
                nc.gpsimd.dma_start(out=scr[f"dt2o{s}"], in_=st[s]["o2"][:])

            def stage_dt3(s):
                S = SEGS[s]
                nt2, nt3, mall = S["nt2"], S["nt3"], malls[s]
                Q2 = (nt2 + 1) // 2
                nry3 = nt3 + 2
                ph3 = bpool.tile([128, nry3 * 90], dt.bfloat16, tag=f"ph3{s}",
                                 name=f"ph3{s}")
                sd2 = scr[f"dt2o{s}"]
                # one DMA: partition (g, c) <- scr[(c, g)] nested dims
                pap3 = bass.AP(sd2.tensor, sd2.offset,
                               [[Q2 * 90, 4], [4 * Q2 * 90, 32],
                                [1, nry3 * 90]])
                nc.gpsimd.dma_start(out=ph3[:], in_=pap3)
                # concat input tile: [64 dt3 | pad] plus x_img tiles
                dtc = bpool.tile([64, nt3, 92], dt.bfloat16, tag=f"dtc{s}",
                                 name=f"dtc{s}")
                st[s]["dtc"] = dtc
                nc.vector.memset(dtc[:, :, 0:2], 0.0)
                nc.vector.memset(dtc[:, :, 90:92], 0.0)
                m3 = bass.AP(mall.tensor, mall.offset + nt2,
                             [mall.ap[0], [1, nt3]])
                RPP3 = 4
                for t0 in range(0, nt3, RPP3):
                    nr = min(RPP3, nt3 - t0)
                    ps = ppool.tile([64, nr, 88], dt.float32, tag=f"ps{s}")
                    gi = 0
                    for dky in range(3):
                        for dmx in range(3):
                            g = dky * 3 + dmx
                            rhs = bass.AP(ph3.tensor,
                                          ph3.offset + (t0 + dky) * 90 + dmx,
                                          [ph3.ap[0], [90, nr], [1, 88]])
                            nc.tensor.matmul(ps[:], wt["w_dt3"][:, g, :], rhs,
                                             start=(gi == 0), stop=(gi == 8))
                            gi += 1
                    ev = wpool.tile([64, nr, 88], dt.bfloat16, tag=f"ev3{s}")
                    nc.scalar.activation(ev[:], ps[:], RELU,
                                         bias=ct["t_dt3"][0:64, 0:1],
                                         scale=ct["s_dt3"][0:64, 0:1])
                    mbb = bass.AP(m3.tensor, m3.offset + t0,
                                  [m3.ap[0], [1, nr], [0, 88]])
                    nc.vector.tensor_tensor(out=dtc[:, t0:t0 + nr, 2:90],
                                            in0=ev[:], in1=mbb[0:64],
                                            op=mybir.AluOpType.mult)

            def stage_xload(s):
                S = SEGS[s]
                xs = []
                for g in range(2):
                    xt = bpool.tile([128, S["nt3"] * 92], dt.bfloat16,
                                    tag=f"x{g}_{s}", name=f"xseg_t{g}")
                    nc.sync.dma_start(out=xt[:], in_=AP[f"xseg{s}"][g])
                    xs.append(xt)
                st[s]["xs"] = xs

            def stage_dn1(s):
                S = SEGS[s]
                nt2, nt3, nn1 = S["nt2"], S["nt3"], S["nn1"]
                mall, dtc, xs = malls[s], st[s]["dtc"], st[s]["xs"]
                mn1 = bass.AP(mall.tensor, mall.offset + nt2 + nt3,
                              [mall.ap[0], [1, nn1]])
                n1o = []
                for g in range(2):
                    t = bpool.tile([128, nn1, 92], dt.bfloat16,
                                   tag=f"n1o{g}_{s}", name=f"n1o{g}_{s}")
                    nc.vector.memset(t[:, :, 0:2], 0.0)
                    nc.vector.memset(t[:, :, 90:92], 0.0)
                    n1o.append(t)
                st[s]["n1o"] = n1o
                RPP = 5
                for ocg in range(2):
                    for r0 in range(0, nn1, RPP):
                        nr = min(RPP, nn1 - r0)
                        ps = ppool.tile([128, nr, 88], dt.float32, tag=f"ps{s}")
                        gi = 0
                        for ky in range(3):
                            for kx in range(3):
                                tap = ky * 3 + kx
                                for icc, srcT in enumerate((xs[0], xs[1], dtc)):
                                    kk = 128 if icc < 2 else 64
                                    rhs = bass.AP(
                                        srcT.tensor,
                                        srcT.offset + (r0 + ky + 1) * 92 + kx + 1,
                                        [srcT.ap[0], [92, nr], [1, 88]])
                                    lhs = wt["w_dn1"][0:kk, tap * 3 + icc,
                                                      ocg * 128:(ocg + 1) * 128]
                                    nc.tensor.matmul(ps[:], lhs, rhs,
                                                     start=(gi == 0),
                                                     stop=(gi == 26))
                                    gi += 1
                        ev = wpool.tile([128, nr, 88], dt.bfloat16, tag=f"evn1{s}")
                        nc.scalar.activation(ev[:], ps[:], RELU,
                                             bias=ct["t_dn1"][:, ocg:ocg + 1],
                                             scale=ct["s_dn1"][:, ocg:ocg + 1])
                        mbb = bass.AP(mn1.tensor, mn1.offset + r0,
                                      [mn1.ap[0], [1, nr], [0, 88]])
                        nc.vector.tensor_tensor(
                            out=n1o[ocg][:, r0:r0 + nr, 2:90],
                            in0=ev[:], in1=mbb, op=mybir.AluOpType.mult)

            def stage_dn2(s):
                S = SEGS[s]
                nout, n1o = S["nout"], st[s]["n1o"]
                RPP = 5
                n2o = []
                for g in range(2):
                    n2o.append(bpool.tile([128, nout, 88], dt.bfloat16,
                                          tag=f"n2o{g}_{s}", name=f"n2o{g}_{s}"))
                st[s]["n2o"] = n2o
                dn3 = stage_dn3(s)
                next(dn3)                        # prime: allocates out tiles
                for r0 in range(0, nout, RPP):
                    nr = min(RPP, nout - r0)
                    for ocg in range(2):
                        ps = ppool.tile([128, nr, 88], dt.float32, tag=f"ps{s}")
                        gi = 0
                        for ky in range(3):
                            for kx in range(3):
                                tap = ky * 3 + kx
                                for icc in range(2):
                                    rhs = bass.AP(
                                        n1o[icc].tensor,
                                        n1o[icc].offset + (r0 + ky) * 92 + kx + 1,
                                        [n1o[icc].ap[0], [92, nr], [1, 88]])
                                    lhs = wt["w_dn2"][:, tap * 2 + icc,
                                                      ocg * 128:(ocg + 1) * 128]
                                    nc.tensor.matmul(ps[:], lhs, rhs,
                                                     start=(gi == 0),
                                                     stop=(gi == 17))
                                    gi += 1
                        ev = wpool.tile([128, nr, 88], dt.bfloat16, tag=f"evn2{s}")
                        nc.scalar.activation(ev[:], ps[:], RELU,
                                             bias=ct["t_dn2"][:, ocg:ocg + 1],
                                             scale=ct["s_dn2"][:, ocg:ocg + 1])
                        nc.vector.tensor_copy(n2o[ocg][:, r0:r0 + nr, :], ev[:])
                    try:
                        dn3.send(r0 + nr)        # emit dn3 chunks now ready
                    except StopIteration:
                        pass

            def stage_dn3(s):
                """Generator: receives the count of completed dn2 rows and
                emits dn3+softmax for pixel chunks whose rows are ready."""
                S = SEGS[s]
                nout, n2o = S["nout"], st[s]["n2o"]
                npix = nout * FW
                feat_sb[s] = bpool.tile([128, ((npix + 127) // 128) * CIMG],
                                        dt.bfloat16, tag=f"feat{s}", name=f"feat_sb{s}")
                depth_sb[s] = bpool.tile([128, ((npix + 127) // 128) * DD],
                                         dt.float32, tag=f"depth{s}", name=f"depth_sb{s}")
                n2f = [t.rearrange("p a b -> p (a b)") for t in n2o]
                a0 = 0 if s == 0 else 11
                pcs = (npix + 127) // 128
                rows_done = yield
                for pc in range(pcs):
                    if pc == pcs - 1:
                        # flush all-but-last chunk now so only the final
                        # chunk's output DMA sits in the tail
                        dsl = bass.AP(out_depth.tensor,
                                      out_depth.offset + a0 * DD,
                                      [[17 * DD, 128], [1, (pcs - 1) * DD]])
                        nc.sync.dma_start(
                            out=dsl, in_=depth_sb[s][:, 0:(pcs - 1) * DD])
                        fsl = bass.AP(out_feat.tensor,
                                      out_feat.offset + a0 * CIMG,
                                      [[17 * CIMG, 128], [1, (pcs - 1) * CIMG]])
                        nc.sync.dma_start(
                            out=fsl, in_=feat_sb[s][:, 0:(pcs - 1) * CIMG])
                    m = min(128, npix - pc * 128)
                    # rows needed by pixels [pc*128, pc*128+m)
                    need = (pc * 128 + m - 1) // FW + 1
                    while rows_done < need:
                        rows_done = yield
                    ps = ppool.tile([m, 139], dt.float32, tag=f"ps{s}")
                    for icc in range(2):
                        nc.tensor.matmul(ps[:], n2f[icc][:, pc * 128:pc * 128 + m],
                                         wt["w_dn3"][:, icc, :],
                                         start=(icc == 0), stop=(icc == 1))
                    # add bias via vector then softmax over first 59
                    lg = wpool.tile([m, 139], dt.float32, tag=f"lg{s}")
                    nc.vector.tensor_tensor(out=lg[:], in0=ps[:],
                                            in1=ct["b_dn3"][0:m],
                                            op=mybir.AluOpType.add)
                    mx = wpool.tile([m, 1], dt.float32, tag=f"mx{s}")
                    nc.vector.reduce_max(mx[:], lg[:, 0:DD],
                                         axis=mybir.AxisListType.X, negate=True)
                    ex = wpool.tile([m, DD], dt.float32, tag=f"ex{s}")
                    nc.scalar.activation(ex[:], lg[:, 0:DD],
                                         mybir.ActivationFunctionType.Exp,
                                         bias=mx[:, 0:1], scale=1.0)
                    sm = wpool.tile([m, 1], dt.float32, tag=f"sm{s}")
                    nc.vector.reduce_sum(sm[:], ex[:], axis=mybir.AxisListType.X)
                    rc = wpool.tile([m, 1], dt.float32, tag=f"rc{s}")
                    nc.vector.reciprocal(rc[:], sm[:])
                    nc.vector.tensor_scalar(
                        out=depth_sb[s][0:m, pc * DD:(pc + 1) * DD], in0=ex[:],
                        scalar1=rc[:, 0:1], scalar2=None,
                        op0=mybir.AluOpType.mult)
                    nc.vector.tensor_copy(
                        feat_sb[s][0:m, pc * CIMG:(pc + 1) * CIMG],
                        lg[:, DD:DD + CIMG])

                # final chunk's outputs
                dsl = bass.AP(out_depth.tensor,
                              out_depth.offset + (a0 + pcs - 1) * DD,
                              [[17 * DD, 128], [1, DD]])
                nc.sync.dma_start(out=dsl,
                                  in_=depth_sb[s][:, (pcs - 1) * DD:pcs * DD])
                fsl = bass.AP(out_feat.tensor,
                              out_feat.offset + (a0 + pcs - 1) * CIMG,
                              [[17 * CIMG, 128], [1, CIMG]])
                nc.sync.dma_start(out=fsl,
                                  in_=feat_sb[s][:, (pcs - 1) * CIMG:pcs * CIMG])

            # schedule: dt1 is folded into the host's dph prep; dt2(1)/dt3(0)
            # hide the scr roundtrips; dn3 is fused into dn2 so softmax
            # pipelines under matmuls
            stage_dt2(0)
            scr_write(0)
            stage_dt2(1)
            stage_dt3(0)
            stage_xload(0)
            scr_write(1)
            stage_dt3(1)
            stage_xload(1)
            stage_wload()
            stage_dn1(0)
            stage_dn1(1)
            stage_dn2(0)
            stage_dn2(1)
    nc.compile()
    return nc


# ------------------------------------------------------------ host helpers
def _host_geometry(rots, trans, intr, post_rots, post_trans):
    import jax
    import jax.numpy as jnp
    with jax.default_device(jax.devices("cpu")[0]):
        f32 = jnp.float32
        ds = jnp.arange(1.0, 60.0, 1.0, dtype=f32)
        xs = jnp.linspace(0.0, IW - 1.0, FW, dtype=f32)
        ys = jnp.linspace(0.0, IH - 1.0, FH, dtype=f32)
        dm = jnp.broadcast_to(ds[:, None, None], (DD, FH, FW))
        xm = jnp.broadcast_to(xs[None, None, :], (DD, FH, FW))
        ym = jnp.broadcast_to(ys[None, :, None], (DD, FH, FW))
        fr = jnp.stack([xm, ym, dm], -1)
        pts = fr[None, None] - jnp.asarray(post_trans)[:, :, None, None, None, :]
        pts = jnp.einsum("bnij,bndhwj->bndhwi",
                         jnp.linalg.inv(jnp.asarray(post_rots)), pts)
        pts = jnp.concatenate([pts[..., :2] * pts[..., 2:3], pts[..., 2:3]], -1)
        comb = jnp.einsum("bnij,bnjk->bnik", jnp.asarray(rots),
                          jnp.linalg.inv(jnp.asarray(intr)))
        pts = jnp.einsum("bnij,bndhwj->bndhwi", comb, pts) \
            + jnp.asarray(trans)[:, :, None, None, None, :]
        lo = jnp.array([XY0, XY0, Z0], dtype=f32)
        dxv = jnp.array([DXY, DXY, DZ], dtype=f32)
        g = ((pts - lo) / dxv).astype(jnp.int32).reshape(-1, 3)
        kept = ((g[:, 0] >= 0) & (g[:, 0] < NX) & (g[:, 1] >= 0) & (g[:, 1] < NX)
                & (g[:, 2] >= 0) & (g[:, 2] < NZ))
        flat = (g[:, 2] * NX + g[:, 0]) * NX + g[:, 1]
        return np.asarray(flat, np.int64), np.asarray(kept)


def _prep_a_inputs(inputs):
    """Build per-core input maps for launch A."""
    d = np.asarray(inputs["d"], np.float32).reshape(N, IH, IW)
    x_img = np.asarray(inputs["x_img"], np.float32)

    # dt1 folded affine: relu(alpha*d + beta), alpha = s*w, beta = s*b + t
    a1 = (inputs["dt1_s"] * inputs["dt1_w"][:, 0, 0, 0]).astype(np.float32)
    b1 = (inputs["dt1_s"] * inputs["dt1_b"] + inputs["dt1_t"]).astype(np.float32)
    cab = np.arange(128)
    dt1_alpha = a1[cab // 16][:, None]
    dt1_beta = b1[cab // 16][:, None]

    def wprep_dt2():
        w = np.asarray(inputs["dt2_w"], np.float32)      # [32,8,5,5]
        out = np.zeros((4, 128, 32), np.float32)
        for ky in range(5):
            for kx in range(5):
                a, dky = ky % 4, ky // 4
                bph, dmx = (kx + 2) % 4, (kx + 2) // 4
                g = dky * 2 + dmx
                rows = (np.arange(8)) * 16 + a * 4 + bph
                out[g, rows, :] = w[:, :, ky, kx].T
        return out.astype(bf16)

    def wprep_dt3():
        w = np.asarray(inputs["dt3_w"], np.float32)      # [64,32,5,5]
        out = np.zeros((9, 128, 64), np.float32)
        for ky in range(5):
            for kx in range(5):
                a, dky = ky % 2, ky // 2
                bph, dmx = kx % 2, (kx + 2) // 2 - 1
                g = dky * 3 + dmx
                rows = (a * 2 + bph) * 32 + np.arange(32)
                out[g, rows, :] = w[:, :, ky, kx].T
        return out.astype(bf16)

    def wprep_3x3(w, icc_sizes):
        O, I = w.shape[0], w.shape[1]
        nic = len(icc_sizes)
        out = np.zeros((9, nic, 128, O), np.float32)
        for ky in range(3):
            for kx in range(3):
                tap = ky * 3 + kx
                ic0 = 0
                for icc, sz in enumerate(icc_sizes):
                    out[tap, icc, 0:sz, :] = w[:, ic0:ic0 + sz, ky, kx].T
                    ic0 += sz
        return out.astype(bf16)

    # NOTE: dn1 input concat order is [dt3(64) | x_img(256)] in the reference;
    # our matmul chunks are (x0:128, x1:128, dt3:64) -> weight cols must match:
    w_dn1_full = np.asarray(inputs["dn1_w"], np.float32)
    w_dn1 = np.zeros((9, 3, 128, 256), np.float32)
    for ky in range(3):
        for kx in range(3):
            tap = ky * 3 + kx
            w_dn1[tap, 0, :, :] = w_dn1_full[:, 64:192, ky, kx].T
            w_dn1[tap, 1, :, :] = w_dn1_full[:, 192:320, ky, kx].T
            w_dn1[tap, 2, 0:64, :] = w_dn1_full[:, 0:64, ky, kx].T
    w_dn1 = w_dn1.astype(bf16)
    w_dn2 = wprep_3x3(np.asarray(inputs["dn2_w"], np.float32), [128, 128])
    w_dn3 = np.asarray(inputs["dn3_w"], np.float32)[:, :, 0, 0]  # [139, 256]
    w_dn3p = np.zeros((2, 128, 139), np.float32)
    w_dn3p[0] = w_dn3[:, 0:128].T
    w_dn3p[1] = w_dn3[:, 128:256].T

    def fold_bias(b, s, t):
        # conv bias b then bn scale/shift: relu(s*(x+b) + t) = relu(s*x + (s*b+t))
        return np.asarray(s, np.float32), np.asarray(s * b + t, np.float32)

    s2, t2 = fold_bias(inputs["dt2_b"], inputs["dt2_s"], inputs["dt2_t"])
    s3, t3 = fold_bias(inputs["dt3_b"], inputs["dt3_s"], inputs["dt3_t"])
    sn1, tn1 = fold_bias(inputs["dn1_b"], inputs["dn1_s"], inputs["dn1_t"])
    sn2, tn2 = fold_bias(inputs["dn2_b"], inputs["dn2_s"], inputs["dn2_t"])
    b_dn3 = np.broadcast_to(np.asarray(inputs["dn3_b"], np.float32)[None, :],
                            (128, 139)).copy()

    consts = np.zeros((128, 153), np.float32)
    consts[:, 0] = dt1_alpha[:, 0]
    consts[:, 1] = dt1_beta[:, 0]
    consts[:, 2] = np.tile(s2, 4)
    consts[:, 3] = np.tile(t2, 4)
    consts[:, 4] = np.tile(s3, 2)
    consts[:, 5] = np.tile(t3, 2)
    consts[:, 6:8] = sn1.reshape(2, 128).T
    consts[:, 8:10] = tn1.reshape(2, 128).T
    consts[:, 10:12] = sn2.reshape(2, 128).T
    consts[:, 12:14] = tn2.reshape(2, 128).T
    consts[:, 14:153] = b_dn3
    shared = dict(
        consts=consts,
        w_dt2=wprep_dt2(), w_dt3=wprep_dt3(), w_dn1=w_dn1, w_dn2=w_dn2,
        w_dn3=w_dn3p.astype(bf16),
    )

    maps = []
    for c in range(NCORES):
        m = dict(shared)
        for s, (cam, h0) in enumerate([SEG_A[c], SEG_B[c]]):
            S = SEGS[s]
            d0 = 8 * h0 - 34
            dseg = np.zeros((S["nd"], 712), np.float32)
            vseg = np.zeros((S["nd"], 712), bool)
            lo, hi = max(0, d0), min(IH, d0 + S["nd"])
            if hi > lo:
                dseg[lo - d0:hi - d0, 4:708] = d[cam, lo:hi]
                vseg[lo - d0:hi - d0, 4:708] = True
            nq = S["nq"]
            ph = dseg.reshape(nq, 4, 178, 4)[:, :, :177, :]     # ry a rx b
            ph = ph.transpose(1, 3, 0, 2)                        # a b ry rx
            vph = vseg.reshape(nq, 4, 178, 4)[:, :, :177, :].transpose(1, 3, 0, 2)
            # dt1 applied on host: relu(alpha*d + beta), zero at pads
            dphc = np.where(vph[None],
                            np.maximum(a1[:, None, None, None, None] * ph[None]
                                       + b1[:, None, None, None, None], 0.0),
                            0.0)                                 # [8,4,4,nq,177]
            m[f"dph{s}"] = dphc.reshape(128, nq * 177).astype(bf16)
            q0, t0, r0 = 2 * h0 - 8, h0 - 3, h0 - 1
            qr = np.arange(S["nt2"]) + q0
            m2m = np.broadcast_to(((qr >= 0) & (qr < 64))[None, :],
                                  (128, S["nt2"]))
            tr = np.arange(S["nt3"]) + t0
            m3m = np.broadcast_to(((tr >= 0) & (tr < FH))[None, :],
                                  (128, S["nt3"]))
            rr = np.arange(S["nn1"]) + r0
            mn1m = np.broadcast_to(((rr >= 0) & (rr < FH))[None, :],
                                   (128, S["nn1"]))
            m[f"masks{s}"] = np.concatenate(
                [m2m, m3m, mn1m], axis=1).astype(bf16)
            xseg = np.zeros((2, 128, S["nt3"], 92), np.float32)
            lo2, hi2 = max(0, t0), min(FH, t0 + S["nt3"])
            if hi2 > lo2:
                xseg[:, :, lo2 - t0:hi2 - t0, 2:90] = \
                    x_img[cam, :, lo2:hi2, :].reshape(2, 128, hi2 - lo2, FW)
            m[f"xseg{s}"] = xseg.reshape(2, 128, S["nt3"] * 92).astype(bf16)
        maps.append(m)
    return maps


# ---------------------------------------------------------------- launch B
def build_launch_b(sizes):
    """Per chunk k: [128pix x 80ch] stationary feat tile x host-built
    [128pix x sizes[k] voxel-slot] depth-weight matrix -> [80, nv] window
    sums. W and out use packed (variable-size) layouts; W loads in a few
    batched DMAs, out in one."""
    nc = bacc.Bacc("TRN2", target_bir_lowering=False, debug=False,
                   num_devices=NCORES)
    NCH = len(sizes)
    offs = np.concatenate([[0], np.cumsum(sizes)]).astype(int)
    S = int(offs[-1])
    wmat = nc.dram_tensor("wmat", [128, S], dt.bfloat16,
                          kind="ExternalInput").ap()
    feats = nc.dram_tensor("feats", [128, NCH, CIMG], dt.bfloat16,
                           kind="ExternalInput").ap()
    owin = nc.dram_tensor("owin", [CIMG, S], dt.bfloat16,
                          kind="ExternalOutput").ap()
    NB = 4                                   # W DMA batches
    bnd = [int(round(NCH * i / NB)) for i in range(NB + 1)]
    with tile.TileContext(nc) as tc:
        with tc.tile_pool(name="const", bufs=1) as cpool, \
             tc.tile_pool(name="ps", bufs=4, space="PSUM") as pp:
            ft = cpool.tile([128, NCH, CIMG], dt.bfloat16, name="ft")
            kf = min(3, NCH)
            nc.sync.dma_start(out=ft[:, 0:kf, :], in_=feats[:, 0:kf, :])
            wt = cpool.tile([128, S], dt.bfloat16, name="wt")
            nc.sync.dma_start(out=wt[:, 0:offs[bnd[1]]],
                              in_=wmat[:, 0:offs[bnd[1]]])
            if kf < NCH:
                nc.sync.dma_start(out=ft[:, kf:NCH, :], in_=feats[:, kf:NCH, :])
            for b in range(1, NB):
                lo, hi = offs[bnd[b]], offs[bnd[b + 1]]
                if hi > lo:
                    nc.sync.dma_start(out=wt[:, lo:hi], in_=wmat[:, lo:hi])
            ot = cpool.tile([CIMG, S], dt.bfloat16, name="ot")
            khalf = (NCH * 3) // 5
            for k in range(NCH):
                nv, o0 = int(sizes[k]), int(offs[k])
                ps = pp.tile([CIMG, 512], dt.float32, tag="ps", name="ps")
                nc.tensor.matmul(ps[:, 0:nv], ft[:, k, :], wt[:, o0:o0 + nv],
                                 start=True, stop=True)
                if k % 2 == 0:
                    nc.scalar.activation(ot[:, o0:o0 + nv], ps[:, 0:nv],
                                         mybir.ActivationFunctionType.Copy)
                else:
                    nc.vector.tensor_copy(ot[:, o0:o0 + nv], ps[:, 0:nv])
                if k == khalf:
                    # flush completed windows; only the tail rides the end
                    nc.sync.dma_start(out=owin[:, 0:int(offs[k + 1])],
                                      in_=ot[:, 0:int(offs[k + 1])])
            nc.sync.dma_start(out=owin[:, int(offs[khalf + 1]):S],
                              in_=ot[:, int(offs[khalf + 1]):S])
    nc.compile()
    return nc


# ---------------------------------------------------------------- launch C
C_OUT_ROWS = 23              # ds2-out rows per core (8*23 = 184 >= 180)


def build_launch_c():
    nc = bacc.Bacc("TRN2", target_bir_lowering=False, debug=False,
                   num_devices=NCORES)
    NR1 = C_OUT_ROWS + 2                         # ds1-out rows incl halo (25)
    NRP = 2 * NR1 + 1                            # pooled rows needed (51)
    slab = nc.dram_tensor("slab", [CIMG, NRP, 362], dt.bfloat16,
                          kind="ExternalInput").ap()
    m1 = nc.dram_tensor("m1", [128, NR1], dt.bfloat16, kind="ExternalInput").ap()
    wd1 = nc.dram_tensor("wd1", [CIMG, 9, CIMG], dt.bfloat16,
                         kind="ExternalInput").ap()
    wd2 = nc.dram_tensor("wd2", [CIMG, 9, CIMG], dt.bfloat16,
                         kind="ExternalInput").ap()
    sb1 = nc.dram_tensor("sb1", [CIMG, 2], dt.float32, kind="ExternalInput").ap()
    sb2 = nc.dram_tensor("sb2", [CIMG, 2], dt.float32, kind="ExternalInput").ap()
    yout = nc.dram_tensor("yout", [CIMG, C_OUT_ROWS, 180], dt.float32,
                          kind="ExternalOutput").ap()
    RELU = mybir.ActivationFunctionType.Relu
    with tile.TileContext(nc) as tc:
        with tc.tile_pool(name="const", bufs=1) as cpool,              tc.tile_pool(name="work", bufs=2) as wp,              tc.tile_pool(name="big", bufs=1) as bp,              tc.tile_pool(name="ps", bufs=3, space="PSUM") as pp:
            # weights/consts first so ds1 can start on the first slab chunk
            w1 = cpool.tile([CIMG, 9, CIMG], dt.bfloat16, name="w1")
            nc.sync.dma_start(out=w1[:], in_=wd1)
            sb1t = cpool.tile([CIMG, 2], dt.float32, name="sb1t")
            nc.sync.dma_start(out=sb1t[:], in_=sb1)
            m1t = wp.tile([128, NR1], dt.bfloat16, name="m1t")
            nc.sync.dma_start(out=m1t[:], in_=m1)
            slabt = bp.tile([CIMG, NRP, 362], dt.bfloat16, name="slabt")
            for rr in range(0, NRP, 9):
                nrr = min(9, NRP - rr)
                nc.sync.dma_start(out=slabt[:, rr:rr + nrr, :],
                                  in_=slab[:, rr:rr + nrr, :])
            w2 = cpool.tile([CIMG, 9, CIMG], dt.bfloat16, name="w2")
            nc.sync.dma_start(out=w2[:], in_=wd2)
            sb2t = cpool.tile([CIMG, 2], dt.float32, name="sb2t")
            nc.sync.dma_start(out=sb2t[:], in_=sb2)
            h1 = bp.tile([CIMG, NR1, 182], dt.bfloat16, name="h1")
            nc.vector.memset(h1[:, :, 0:1], 0.0)
            nc.vector.memset(h1[:, :, 181:182], 0.0)
            # ds1: stride-2 3x3; out row t reads slab rows 2t..2t+2 (slab row 0
            # = pooled row 2o0-3, so out row t (global o0-1+t) reads
            # 2(o0-1+t)-1..+1 - (2o0-3) = 2t..2t+2); col c reads 2c..2c+2
            RP = 2
            for t0 in range(0, NR1, RP):
                nr = min(RP, NR1 - t0)
                ps = pp.tile([CIMG, nr, 180], dt.float32, tag="ps1", name="ps")
                gi = 0
                for ky in range(3):
                    for kx in range(3):
                        rhs = bass.AP(slabt.tensor,
                                      slabt.offset + (2 * t0 + ky) * 362 + kx,
                                      [slabt.ap[0], [2 * 362, nr], [2, 180]])
                        nc.tensor.matmul(ps[:], w1[:, ky * 3 + kx, :], rhs,
                                         start=(gi == 0), stop=(gi == 8))
                        gi += 1
                ev = wp.tile([CIMG, nr, 180], dt.bfloat16, tag="ev", name="ev")
                nc.scalar.activation(ev[:], ps[:], RELU, bias=sb1t[:, 1:2],
                                     scale=sb1t[:, 0:1])
                mbb = bass.AP(m1t.tensor, m1t.offset + t0,
                              [[m1t.ap[0][0], CIMG], [1, nr], [0, 180]])
                nc.vector.tensor_tensor(out=h1[:, t0:t0 + nr, 1:181],
                                        in0=ev[:], in1=mbb,
                                        op=mybir.AluOpType.mult)
            # ds2: 3x3 pad 1: out row o reads h1 rows o..o+2, col c: c..c+2
            yo = bp.tile([CIMG, C_OUT_ROWS, 180], dt.float32, name="yo")
            for o0 in range(0, C_OUT_ROWS, RP):
                nr = min(RP, C_OUT_ROWS - o0)
                ps = pp.tile([CIMG, nr, 180], dt.float32, tag="ps2", name="ps")
                gi = 0
                for ky in range(3):
                    for kx in range(3):
                        rhs = bass.AP(h1.tensor,
                                      h1.offset + (o0 + ky) * 182 + kx,
                                      [h1.ap[0], [182, nr], [1, 180]])
                        nc.tensor.matmul(ps[:], w2[:, ky * 3 + kx, :], rhs,
                                         start=(gi == 0), stop=(gi == 8))
                        gi += 1
                nc.scalar.activation(yo[:, o0:o0 + nr, :], ps[:], RELU,
                                     bias=sb2t[:, 1:2], scale=sb2t[:, 0:1])
                if (o0 // RP) % 3 == 2 or o0 + nr >= C_OUT_ROWS:
                    lo = (o0 // (3 * RP)) * 3 * RP
                    nc.sync.dma_start(out=yout[:, lo:o0 + nr, :],
                                      in_=yo[:, lo:o0 + nr, :])
    nc.compile()
    return nc


_CACHE = {}


def run_launch_a(inputs):
    if "A" not in _CACHE:
        _CACHE["A"] = build_launch_a()
    nc = _CACHE["A"]
    maps = _prep_a_inputs(inputs)
    res = run_bass_kernel_spmd(nc, maps, list(range(NCORES)))
    depth = np.zeros((NPIX, DD), np.float32)
    feat = np.zeros((NPIX, CIMG), np.float32)
    for c in range(NCORES):
        r = res.results[c]
        for s, (cam, h0) in enumerate([SEG_A[c], SEG_B[c]]):
            S = SEGS[s]
            npix = S["nout"] * FW
            base = (cam * FH + h0) * FW
            a0, pcs = (0, 11) if s == 0 else (11, 6)
            dsg = r["out_depth"][:, a0:a0 + pcs].transpose(1, 0, 2)
            depth[base:base + npix] = dsg.reshape(pcs * 128, DD)[:npix]
            fsg = r["out_feat"][:, a0:a0 + pcs].transpose(1, 0, 2)
            feat[base:base + npix] = fsg.reshape(pcs * 128, CIMG)[:npix]
    return depth, feat


def _build_chunks(flat, kept, depth_rows):
    """Group points by (camera, column-block); per group build the
    [pix, voxel-slot] depth-weight matrix over the group's voxel union.
    Splits column blocks whose union exceeds the PSUM window (512)."""
    fl = flat.reshape(N, DD, FH, FW)
    kp = kept.reshape(N, DD, FH, FW)
    chunks = []                      # (pix_ids, Wdense[npix, nv], vox_ids)

    def add_group(n, w0, w1):
        nw = w1 - w0
        f = fl[n, :, :, w0:w1]                       # [DD, FH, nw]
        k = kp[n, :, :, w0:w1]
        vids = np.unique(f[k])
        if len(vids) > 512 and nw > 1:
            mid = w0 + nw // 2
            add_group(n, w0, mid)
            add_group(n, mid, w1)
            return
        nv = max(len(vids), 1)
        # pixel local idx = (w - w0) * FH + h; point (d, h, w)
        slot = np.searchsorted(vids, f[k]) if len(vids) else np.zeros(0, np.int64)
        dd, hh, ww = np.nonzero(k)
        pix_loc = ww * FH + hh
        pixcol = n * FH * FW + hh * FW + (ww + w0)
        dep = depth_rows[pixcol, dd]
        Wd = np.bincount(pix_loc * nv + slot, weights=dep,
                         minlength=nw * FH * nv).reshape(nw * FH, nv)
        pix_ids = (n * FH * FW + np.arange(FH)[None, :] * FW
                   + (w0 + np.arange(nw))[:, None]).reshape(-1)
        chunks.append((pix_ids, Wd, vids))

    for n in range(N):
        for w0 in range(0, FW, 4):
            add_group(n, w0, w0 + 4)
    return chunks


def _prep_b_inputs(chunks, featflat_bf):
    """Balance chunks across cores by window size; build per-core maps with
    the packed per-slot layout (chunk k size = max over cores, desc-sorted)."""
    order = sorted(range(len(chunks)), key=lambda i: -chunks[i][1].shape[1])
    load = np.zeros(NCORES, np.int64)
    per_core = [[] for _ in range(NCORES)]
    for i in order:
        c = int(np.argmin(load))
        per_core[c].append(i)
        load[c] += chunks[i][1].shape[1]
    NCH = max(len(p) for p in per_core)
    sizes = np.zeros(NCH, np.int64)
    for p in per_core:
        for k, i in enumerate(p):
            sizes[k] = max(sizes[k], chunks[i][1].shape[1])
    sizes = (sizes + 15) // 16 * 16
    offs = np.concatenate([[0], np.cumsum(sizes)]).astype(int)
    S = int(offs[-1])
    maps, scatter = [], []
    for c in range(NCORES):
        wm = np.zeros((128, S), bf16)
        ft = np.zeros((128, NCH, CIMG), bf16)
        sc = []
        for k, i in enumerate(per_core[c]):
            pix_ids, Wd, vids = chunks[i]
            npix, nv = Wd.shape
            wm[0:npix, offs[k]:offs[k] + nv] = Wd
            ft[0:npix, k, :] = featflat_bf[pix_ids]
            sc.append((int(offs[k]), vids))
        maps.append(dict(wmat=wm, feats=ft))
        scatter.append(sc)
    return maps, scatter, tuple(int(s) for s in sizes)


def _prep_c_inputs(inputs, pooled_t):
    """pooled_t: [CIMG, 360, 360] f32 -> per-core slabs + masks + weights."""
    NR1 = C_OUT_ROWS + 2
    NRP = 2 * NR1 + 1
    w1 = np.asarray(inputs["ds1_w"], np.float32)
    w2 = np.asarray(inputs["ds2_w"], np.float32)
    # wd1/wd2: [ic, tap, oc]
    wd1 = np.ascontiguousarray(w1.transpose(1, 2, 3, 0).reshape(CIMG, 9, CIMG))
    wd2 = np.ascontiguousarray(w2.transpose(1, 2, 3, 0).reshape(CIMG, 9, CIMG))
    sb1 = np.stack([np.asarray(inputs["ds1_s"], np.float32),
                    np.asarray(inputs["ds1_t"], np.float32)], 1)
    sb2 = np.stack([np.asarray(inputs["ds2_s"], np.float32),
                    np.asarray(inputs["ds2_t"], np.float32)], 1)
    shared = dict(wd1=wd1.astype(bf16), wd2=wd2.astype(bf16), sb1=sb1, sb2=sb2)
    maps = []
    pt_bf = pooled_t.astype(bf16)
    for c in range(NCORES):
        o0g = C_OUT_ROWS * c
        p0 = 2 * o0g - 3
        slab = np.zeros((CIMG, NRP, 362), bf16)
        lo, hi = max(0, p0), min(NX, p0 + NRP)
        if hi > lo:
            slab[:, lo - p0:hi - p0, 1:361] = pt_bf[:, lo:hi, :]
        t1g = np.arange(NR1) + (o0g - 1)
        m1 = np.broadcast_to(((t1g >= 0) & (t1g < 180))[None, :],
                             (128, NR1)).astype(bf16)
        maps.append(dict(shared, slab=slab, m1=np.ascontiguousarray(m1)))
    return maps


def kernel(**inputs):
    inputs = {k: np.asarray(v) for k, v in inputs.items()}
    flat, kept = _host_geometry(inputs["cam2lidar_rots"],
                                inputs["cam2lidar_trans"], inputs["intrins"],
                                inputs["post_rots"], inputs["post_trans"])
    depth_rows, feat_rows = run_launch_a(inputs)
    featflat_bf = feat_rows.astype(bf16)

    chunks = _build_chunks(flat, kept, depth_rows)
    bmaps, scatter, sizes = _prep_b_inputs(chunks, featflat_bf)
    key = ("B", sizes)
    if key not in _CACHE:
        _CACHE[key] = build_launch_b(sizes)
    res_b = run_bass_kernel_spmd(_CACHE[key], bmaps, list(range(NCORES)))

    allvox = np.concatenate([vids for c in range(NCORES)
                             for _, vids in scatter[c]])
    allval = np.concatenate(
        [res_b.results[c]["owin"][:, o0:o0 + len(vids)].T.astype(np.float32)
         for c in range(NCORES) for o0, vids in scatter[c]])
    o = np.argsort(allvox, kind="stable")
    allvox, allval = allvox[o], allval[o]
    starts = np.flatnonzero(np.r_[True, allvox[1:] != allvox[:-1]])
    pooled = np.zeros((NX * NX, CIMG), np.float32)
    pooled[allvox[starts]] = np.add.reduceat(allval, starts, axis=0)
    pooled_t = np.ascontiguousarray(
        pooled.reshape(NX, NX, CIMG).transpose(2, 0, 1))

    if "C" not in _CACHE:
        _CACHE["C"] = build_launch_c()
    cmaps = _prep_c_inputs(inputs, pooled_t)
    res_c = run_bass_kernel_spmd(_CACHE["C"], cmaps, list(range(NCORES)))
    out = np.zeros((1, CIMG, 180, 180), np.float32)
    for c in range(NCORES):
        o0g = C_OUT_ROWS * c
        nr = min(C_OUT_ROWS, 180 - o0g)
        if nr > 0:
            out[0, :, o0g:o0g + nr, :] = res_c.results[c]["yout"][:, 0:nr, :]
    return out



# revision 100
# speedup vs baseline: 1.0570x; 1.0021x over previous
"""DepthLSSTransform Trainium kernel: 3 SPMD launches over 8 NeuronCores.

Launch A: per-camera conv pipeline (dtransform + depthnet + softmax) on
          24-row bands (one 16-row + one 8-row segment per core).
Launch B: bev_pool segment-sum via one-hot matmuls over a host-built
          virtual-window schedule (sorted-by-voxel points).
Launch C: BEV downsample convs, spatially sharded.
Host: geometry/voxel indices, scheduling, gathers, folds (orchestration).
"""
import numpy as np
import ml_dtypes

import concourse.bass as bass
import concourse.tile as tile
from concourse import bacc, mybir
from concourse.bass_utils import run_bass_kernel_spmd

dt = mybir.dt
bf16 = ml_dtypes.bfloat16

# ---- problem constants (hardcoded per contract) ----
B, N = 1, 6
CIN, CIMG, DD = 256, 80, 59
FH, FW, IH, IW = 32, 88, 256, 704
XY0, DXY, NX = -54.0, 0.3, 360
Z0, DZ, NZ = -10.0, 20.0, 1
NPTS = N * DD * FH * FW
NPIX = N * FH * FW
NCORES = 8
QV = 4                      # chunks of 128 points per virtual window

# per-core segments: (camera, h0) for seg A (16 rows) and seg B (8 rows)
SEG_A = [(0, 0), (1, 0), (1, 16), (2, 16), (3, 0), (4, 0), (4, 16), (5, 16)]
SEG_B = [(0, 16), (0, 24), (2, 0), (2, 8), (3, 16), (3, 24), (5, 0), (5, 8)]
# band pixel ranges in global row order (row = n*32 + h)
ROWS_OF_CORE = [[(SEG_A[c][0] * FH + SEG_A[c][1] + r) for r in range(16)] +
                [(SEG_B[c][0] * FH + SEG_B[c][1] + r) for r in range(8)]
                for c in range(NCORES)]

# segment geometry: rows16 segment: d rows [8h0-34, 8h0+158) (192), dt2 out
# rows [2h0-8, 2h0+39) (47), dt3 [h0-3, h0+19) (22), dn1 [h0-1, h0+17) (18)
SEGS = [dict(nout=16, nd=192, nq=48, nt2=47, nt3=22, nn1=18),
        dict(nout=8, nd=128, nq=32, nt2=31, nt3=14, nn1=10)]


def _seg_ranges(h0, S):
    return dict(d0=8 * h0 - 34, q0=2 * h0 - 8, t0=h0 - 3, r0=h0 - 1, o0=h0)


# ---------------------------------------------------------------- launch A
def build_launch_a(debug=False, psum_bufs=3, work_bufs=3, stages=9):
    nc = bacc.Bacc("TRN2", target_bir_lowering=False, debug=False,
                   num_devices=NCORES)
    AP = {}

    def inp(name, shape, dtype=dt.bfloat16):
        AP[name] = nc.dram_tensor(name, shape, dtype, kind="ExternalInput").ap()
        return AP[name]

    # per segment inputs (s = 0: 16-row, 1: 8-row); flat free dims so DMAs
    # are single-descriptor-per-partition and tile deps stay precise
    for s, S in enumerate(SEGS):
        inp(f"dph{s}", [128, S["nq"] * 177])
        inp(f"masks{s}", [128, S["nt2"] + S["nt3"] + S["nn1"]])
        inp(f"xseg{s}", [2, 128, S["nt3"] * 92])        # x_img slice (padded)
    # packed f32 constants: [alpha, beta, s_dt2, t_dt2, s_dt3, t_dt3,
    #  s_dn1(2), t_dn1(2), s_dn2(2), t_dn2(2), b_dn3(139)] -> [128, 153]
    inp("consts", [128, 153], dt.float32)
    # conv weights (host-prepped layouts)
    inp("w_dt2", [4, 128, 32])                          # groups (dky,dmx)
    inp("w_dt3", [9, 128, 64])
    inp("w_dn1", [9, 3, 128, 256])                      # tap, icchunk(128,128,64pad) -> 256
    inp("w_dn2", [9, 2, 128, 256])
    inp("w_dn3", [2, 128, 139])

    DBG = {}
    dbg_specs = [] if not debug else [("dbg_t1", [128, SEGS[0]["nq"], 177], dt.bfloat16),
                        ("dbg_dt2o", [32, SEGS[0]["nt2"] + 1, 180], dt.bfloat16),
                        ("dbg_dtc", [64, SEGS[0]["nt3"], 92], dt.bfloat16),
                        ("dbg_n1o", [128, SEGS[0]["nn1"], 92], dt.bfloat16),
                        ("dbg_n2o", [128, SEGS[0]["nout"], 88], dt.bfloat16)]
    for nm, sh, dty in dbg_specs:
        DBG[nm] = nc.dram_tensor(nm, sh, dty, kind="ExternalOutput").ap()
    # chunk-major outputs: pixel (a*128+p) of segment s at [p, a0_s + a, :]
    out_depth = nc.dram_tensor("out_depth", [128, 17, DD], dt.float32,
                               kind="ExternalOutput").ap()
    out_feat = nc.dram_tensor("out_feat", [128, 17, CIMG], dt.bfloat16,
                              kind="ExternalOutput").ap()

    # HBM scratch, phase-major: [c32, a2, b2, q', x90] (q' = dt2-row // 2)
    scr = {}
    for s, S in enumerate(SEGS):
        scr[f"dt2o{s}"] = nc.dram_tensor(
            f"dt2o{s}", [32, 2, 2, (S["nt2"] + 1) // 2, 90], dt.bfloat16).ap()

    RELU = mybir.ActivationFunctionType.Relu
    with tile.TileContext(nc) as tc:
        with tc.tile_pool(name="const", bufs=1) as cpool, \
             tc.tile_pool(name="work", bufs=work_bufs) as wpool, \
             tc.tile_pool(name="big", bufs=1) as bpool, \
             tc.tile_pool(name="psum", bufs=2, space="PSUM") as ppool, \
             tc.tile_pool(name="psum2", bufs=4, space="PSUM") as ppool2:
            # ---- DMA issue order = consumption order (the SP queue and the
            # modeled DMA engines serialize; early-stage inputs must land first)
            cts = cpool.tile([128, 153], dt.float32, name="cts")
            nc.sync.dma_start(out=cts[:], in_=AP["consts"])
            # tiny activation right away so the act-table load happens while
            # the first dph chunk is still in flight
            warm = wpool.tile([128, 1], dt.float32, tag="warm", name="warm")
            nc.scalar.activation(warm[:], cts[:, 0:1], RELU)
            ct = {"dt1_alpha": cts[:, 0:1], "dt1_beta": cts[:, 1:2],
                  "s_dt2": cts[:, 2:3], "t_dt2": cts[:, 3:4],
                  "s_dt3": cts[:, 4:5], "t_dt3": cts[:, 5:6],
                  "s_dn1": cts[:, 6:8], "t_dn1": cts[:, 8:10],
                  "s_dn2": cts[:, 10:12], "t_dn2": cts[:, 12:14],
                  "b_dn3": cts[:, 14:153]}
            wt = {}

            def load_w(nm, pat):
                sh = list(AP[nm].shape)
                wt[nm] = cpool.tile([sh[-2], int(np.prod(sh[:-2])), sh[-1]],
                                    dt.bfloat16, tag=nm, name=f'wt_{nm}')
                nc.sync.dma_start(out=wt[nm][:], in_=AP[nm].rearrange(pat))

            # first dph chunk small so dt2 starts ASAP; host has already
            # applied dt1 (relu(alpha*d+beta), pads zeroed) into dph.
            # The big dn-weights are issued later (stage_wload) so they don't
            # sit ahead of the dt2->dt3 scratch roundtrip in the serial DMA
            # stream.
            QCHUNKS = {0: [8, 12, 14, 14], 1: [8, 12, 12]}
            dphs, malls = {}, {}

            def load_dph(s):
                S = SEGS[s]
                nq = S["nq"]
                dphs[s] = bpool.tile([128, nq * 177], dt.bfloat16,
                                     tag=f"dph{s}", name=f"dph{s}")
                qq = 0
                for nqq in QCHUNKS[s]:
                    nc.sync.dma_start(
                        out=dphs[s][:, qq * 177:(qq + nqq) * 177],
                        in_=AP[f"dph{s}"][:, qq * 177:(qq + nqq) * 177])
                    qq += nqq
                malls[s] = wpool.tile([128, S["nt2"] + S["nt3"] + S["nn1"]],
                                      dt.bfloat16, tag=f"msk{s}", name="mall")
                nc.sync.dma_start(out=malls[s][:], in_=AP[f"masks{s}"])

            load_w("w_dt2", "g p o -> p g o")
            load_dph(0)
            load_dph(1)
            load_w("w_dt3", "g p o -> p g o")

            def stage_wload():
                load_w("w_dn1", "t i p o -> p (t i) o")
                load_w("w_dn2", "t i p o -> p (t i) o")
                load_w("w_dn3", "g p o -> p g o")

            feat_sb = {}
            depth_sb = {}
            st = {s: {} for s in range(len(SEGS))}

            def stage_dt2(s):
                S = SEGS[s]
                nt2, t1, mall = S["nt2"], dphs[s], malls[s]
                Q2 = (nt2 + 1) // 2
                # phase-major layout [c32, a2, b2, q', x90]: row q=(2q'+a),
                # col c at (b=c%2, x=c//2+1); makes scr write + ph3 reads
                # fully contiguous per partition
                o2 = bpool.tile([32, 2, 2, Q2, 90], dt.bfloat16, tag=f"o2{s}",
                                name=f"o2{s}")
                st[s]["o2"] = o2
                o2f = o2.rearrange("p a b q x -> p (a b q) x")
                nc.vector.memset(o2f[:, :, 0:1], 0.0)          # x pad left
                nc.vector.memset(o2f[:, :, 89:90], 0.0)        # x pad right
                nc.vector.memset(o2[:, 1, :, Q2 - 1, :], 0.0)  # pad row q=nt2
                m2 = bass.AP(mall.tensor, mall.offset, [mall.ap[0], [1, nt2]])
                RPP2 = 2
                for q0 in range(0, nt2, RPP2):
                    nr = min(RPP2, nt2 - q0)
                    ps = ppool2.tile([32, nr, 176], dt.float32, tag="ps2",
                                     name="ps2")
                    gi = 0
                    for dky in range(2):
                        for dmx in range(2):
                            g = dky * 2 + dmx
                            rhs = bass.AP(
                                t1.tensor, t1.offset + (q0 + dky) * 177 + dmx,
                                [t1.ap[0], [177, nr], [1, 176]])
                            nc.tensor.matmul(ps[:], wt["w_dt2"][:, g, :], rhs,
                                             start=(gi == 0), stop=(gi == 3))
                            gi += 1
                    ev = wpool.tile([32, nr, 176], dt.bfloat16, tag=f"ev2{s}")
                    nc.scalar.activation(ev[:], ps[:], RELU,
                                         bias=ct["t_dt2"][0:32, 0:1],
                                         scale=ct["s_dt2"][0:32, 0:1])
                    mbb = bass.AP(m2.tensor, m2.offset + q0,
                                  [[m2.ap[0][0], 32], [1, nr], [0, 176]])
                    # rows (q0, q0+1) -> a=(0,1) at q'=q0//2; c -> (x, b)
                    o2dst = bass.AP(o2.tensor,
                                    o2.offset + (q0 // 2) * 90 + 1,
                                    [[o2.ap[0][0], 32], [2 * Q2 * 90, nr],
                                     [1, 88], [Q2 * 90, 2]])
                    nc.vector.tensor_tensor(out=o2dst, in0=ev[:], in1=mbb,
                                            op=mybir.AluOpType.mult)

            def scr_write(s):
                # on the idle GPSIMD (SWDGE) queue: its sem wait must not
                # head-of-line-block the streaming SP DMA queue
                nc.gpsimd.dma_start(out=scr[f"dt2o{s}"], in_=st[s]["o2"][:])

            def stage_dt3(s):
                S = SEGS[s]
                nt2, nt3, mall = S["nt2"], S["nt3"], malls[s]
                Q2 = (nt2 + 1) // 2
                nry3 = nt3 + 2
                ph3 = bpool.tile([128, nry3 * 90], dt.bfloat16, tag=f"ph3{s}",
                                 name=f"ph3{s}")
                sd2 = scr[f"dt2o{s}"]
                # one DMA: partition (g, c) <- scr[(c, g)] nested dims
                pap3 = bass.AP(sd2.tensor, sd2.offset,
                               [[Q2 * 90, 4], [4 * Q2 * 90, 32],
                                [1, nry3 * 90]])
                nc.gpsimd.dma_start(out=ph3[:], in_=pap3)
                # concat input tile: [64 dt3 | pad] plus x_img tiles
                dtc = bpool.tile([64, nt3, 92], dt.bfloat16, tag=f"dtc{s}",
                                 name=f"dtc{s}")
                st[s]["dtc"] = dtc
                nc.vector.memset(dtc[:, :, 0:2], 0.0)
                nc.vector.memset(dtc[:, :, 90:92], 0.0)
                m3 = bass.AP(mall.tensor, mall.offset + nt2,
                             [mall.ap[0], [1, nt3]])
                RPP3 = 4
                for t0 in range(0, nt3, RPP3):
                    nr = min(RPP3, nt3 - t0)
                    ps = ppool.tile([64, nr, 88], dt.float32, tag=f"ps{s}")
                    gi = 0
                    for dky in range(3):
                        for dmx in range(3):
                            g = dky * 3 + dmx
                            rhs = bass.AP(ph3.tensor,
                                          ph3.offset + (t0 + dky) * 90 + dmx,
                                          [ph3.ap[0], [90, nr], [1, 88]])
                            nc.tensor.matmul(ps[:], wt["w_dt3"][:, g, :], rhs,
                                             start=(gi == 0), stop=(gi == 8))
                            gi += 1
                    ev = wpool.tile([64, nr, 88], dt.bfloat16, tag=f"ev3{s}")
                    nc.scalar.activation(ev[:], ps[:], RELU,
                                         bias=ct["t_dt3"][0:64, 0:1],
                                         scale=ct["s_dt3"][0:64, 0:1])
                    mbb = bass.AP(m3.tensor, m3.offset + t0,
                                  [m3.ap[0], [1, nr], [0, 88]])
                    nc.vector.tensor_tensor(out=dtc[:, t0:t0 + nr, 2:90],
                                            in0=ev[:], in1=mbb[0:64],
                                            op=mybir.AluOpType.mult)

            def stage_xload(s):
                S = SEGS[s]
                xs = []
                for g in range(2):
                    xt = bpool.tile([128, S["nt3"] * 92], dt.bfloat16,
                                    tag=f"x{g}_{s}", name=f"xseg_t{g}")
                    nc.sync.dma_start(out=xt[:], in_=AP[f"xseg{s}"][g])
                    xs.append(xt)
                st[s]["xs"] = xs

            def stage_dn1(s):
                S = SEGS[s]
                nt2, nt3, nn1 = S["nt2"], S["nt3"], S["nn1"]
                mall, dtc, xs = malls[s], st[s]["dtc"], st[s]["xs"]
                mn1 = bass.AP(mall.tensor, mall.offset + nt2 + nt3,
                              [mall.ap[0], [1, nn1]])
                n1o = []
                for g in range(2):
                    t = bpool.tile([128, nn1, 92], dt.bfloat16,
                                   tag=f"n1o{g}_{s}", name=f"n1o{g}_{s}")
                    nc.vector.memset(t[:, :, 0:2], 0.0)
                    nc.vector.memset(t[:, :, 90:92], 0.0)
                    n1o.append(t)
                st[s]["n1o"] = n1o
                RPP = 5
                for ocg in range(2):
                    for r0 in range(0, nn1, RPP):
                        nr = min(RPP, nn1 - r0)
                        ps = ppool.tile([128, nr, 88], dt.float32, tag=f"ps{s}")
                        gi = 0
                        for ky in range(3):
                            for kx in range(3):
                                tap = ky * 3 + kx
                                for icc, srcT in enumerate((xs[0], xs[1], dtc)):
                                    kk = 128 if icc < 2 else 64
                                    rhs = bass.AP(
                                        srcT.tensor,
                                        srcT.offset + (r0 + ky + 1) * 92 + kx + 1,
                                        [srcT.ap[0], [92, nr], [1, 88]])
                                    lhs = wt["w_dn1"][0:kk, tap * 3 + icc,
                                                      ocg * 128:(ocg + 1) * 128]
                                    nc.tensor.matmul(ps[:], lhs, rhs,
                                                     start=(gi == 0),
                                                     stop=(gi == 26))
                                    gi += 1
                        ev = wpool.tile([128, nr, 88], dt.bfloat16, tag=f"evn1{s}")
                        nc.scalar.activation(ev[:], ps[:], RELU,
                                             bias=ct["t_dn1"][:, ocg:ocg + 1],
                                             scale=ct["s_dn1"][:, ocg:ocg + 1])
                        mbb = bass.AP(mn1.tensor, mn1.offset + r0,
                                      [mn1.ap[0], [1, nr], [0, 88]])
                        nc.vector.tensor_tensor(
                            out=n1o[ocg][:, r0:r0 + nr, 2:90],
                            in0=ev[:], in1=mbb, op=mybir.AluOpType.mult)

            def stage_dn2(s):
                S = SEGS[s]
                nout, n1o = S["nout"], st[s]["n1o"]
                RPP = 5
                n2o = []
                for g in range(2):
                    n2o.append(bpool.tile([128, nout, 88], dt.bfloat16,
                                          tag=f"n2o{g}_{s}", name=f"n2o{g}_{s}"))
                st[s]["n2o"] = n2o
                dn3 = stage_dn3(s)
                next(dn3)                        # prime: allocates out tiles
                for r0 in range(0, nout, RPP):
                    nr = min(RPP, nout - r0)
                    for ocg in range(2):
                        ps = ppool.tile([128, nr, 88], dt.float32, tag=f"ps{s}")
                        gi = 0
                        for ky in range(3):
                            for kx in range(3):
                                tap = ky * 3 + kx
                                for icc in range(2):
                                    rhs = bass.AP(
                                        n1o[icc].tensor,
                                        n1o[icc].offset + (r0 + ky) * 92 + kx + 1,
                                        [n1o[icc].ap[0], [92, nr], [1, 88]])
                                    lhs = wt["w_dn2"][:, tap * 2 + icc,
                                                      ocg * 128:(ocg + 1) * 128]
                                    nc.tensor.matmul(ps[:], lhs, rhs,
                                                     start=(gi == 0),
                                                     stop=(gi == 17))
                                    gi += 1
                        ev = wpool.tile([128, nr, 88], dt.bfloat16, tag=f"evn2{s}")
                        nc.scalar.activation(ev[:], ps[:], RELU,
                                             bias=ct["t_dn2"][:, ocg:ocg + 1],
                                             scale=ct["s_dn2"][:, ocg:ocg + 1])
                        nc.vector.tensor_copy(n2o[ocg][:, r0:r0 + nr, :], ev[:])
                    try:
                        dn3.send(r0 + nr)        # emit dn3 chunks now ready
                    except StopIteration:
                        pass

            def stage_dn3(s):
                """Generator: receives the count of completed dn2 rows and
                emits dn3+softmax for pixel chunks whose rows are ready."""
                S = SEGS[s]
                nout, n2o = S["nout"], st[s]["n2o"]
                npix = nout * FW
                feat_sb[s] = bpool.tile([128, ((npix + 127) // 128) * CIMG],
                                        dt.bfloat16, tag=f"feat{s}", name=f"feat_sb{s}")
                depth_sb[s] = bpool.tile([128, ((npix + 127) // 128) * DD],
                                         dt.float32, tag=f"depth{s}", name=f"depth_sb{s}")
                n2f = [t.rearrange("p a b -> p (a b)") for t in n2o]
                a0 = 0 if s == 0 else 11
                pcs = (npix + 127) // 128
                rows_done = yield
                for pc in range(pcs):
                    if pc == pcs - 1:
                        # flush all-but-last chunk now so only the final
                        # chunk's output DMA sits in the tail
                        dsl = bass.AP(out_depth.tensor,
                                      out_depth.offset + a0 * DD,
                                      [[17 * DD, 128], [1, (pcs - 1) * DD]])
                        nc.sync.dma_start(
                            out=dsl, in_=depth_sb[s][:, 0:(pcs - 1) * DD])
                        fsl = bass.AP(out_feat.tensor,
                                      out_feat.offset + a0 * CIMG,
                                      [[17 * CIMG, 128], [1, (pcs - 1) * CIMG]])
                        nc.sync.dma_start(
                            out=fsl, in_=feat_sb[s][:, 0:(pcs - 1) * CIMG])
                    m = min(128, npix - pc * 128)
                    # rows needed by pixels [pc*128, pc*128+m)
                    need = (pc * 128 + m - 1) // FW + 1
                    while rows_done < need:
                        rows_done = yield
                    ps = ppool.tile([m, 139], dt.float32, tag=f"ps{s}")
                    for icc in range(2):
                        nc.tensor.matmul(ps[:], n2f[icc][:, pc * 128:pc * 128 + m],
                                         wt["w_dn3"][:, icc, :],
                                         start=(icc == 0), stop=(icc == 1))
                    # add bias via vector then softmax over first 59
                    lg = wpool.tile([m, 139], dt.float32, tag=f"lg{s}")
                    nc.vector.tensor_tensor(out=lg[:], in0=ps[:],
                                            in1=ct["b_dn3"][0:m],
                                            op=mybir.AluOpType.add)
                    mx = wpool.tile([m, 1], dt.float32, tag=f"mx{s}")
                    nc.vector.reduce_max(mx[:], lg[:, 0:DD],
                                         axis=mybir.AxisListType.X, negate=True)
                    ex = wpool.tile([m, DD], dt.float32, tag=f"ex{s}")
                    nc.scalar.activation(ex[:], lg[:, 0:DD],
                                         mybir.ActivationFunctionType.Exp,
                                         bias=mx[:, 0:1], scale=1.0)
                    sm = wpool.tile([m, 1], dt.float32, tag=f"sm{s}")
                    nc.vector.reduce_sum(sm[:], ex[:], axis=mybir.AxisListType.X)
                    rc = wpool.tile([m, 1], dt.float32, tag=f"rc{s}")
                    nc.vector.reciprocal(rc[:], sm[:])
                    nc.vector.tensor_scalar(
                        out=depth_sb[s][0:m, pc * DD:(pc + 1) * DD], in0=ex[:],
                        scalar1=rc[:, 0:1], scalar2=None,
                        op0=mybir.AluOpType.mult)
                    nc.vector.tensor_copy(
                        feat_sb[s][0:m, pc * CIMG:(pc + 1) * CIMG],
                        lg[:, DD:DD + CIMG])

                # final chunk's outputs
                dsl = bass.AP(out_depth.tensor,
                              out_depth.offset + (a0 + pcs - 1) * DD,
                              [[17 * DD, 128], [1, DD]])
                nc.sync.dma_start(out=dsl,
                                  in_=depth_sb[s][:, (pcs - 1) * DD:pcs * DD])
                fsl = bass.AP(out_feat.tensor,
                              out_feat.offset + (a0 + pcs - 1) * CIMG,
                              [[17 * CIMG, 128], [1, CIMG]])
                nc.sync.dma_start(out=fsl,
                                  in_=feat_sb[s][:, (pcs - 1) * CIMG:pcs * CIMG])

            # schedule: dt1 is folded into the host's dph prep; dt2(1)/dt3(0)
            # hide the scr roundtrips; dn3 is fused into dn2 so softmax
            # pipelines under matmuls
            stage_dt2(0)
            scr_write(0)
            stage_dt2(1)
            stage_dt3(0)
            stage_xload(0)
            scr_write(1)
            stage_dt3(1)
            stage_xload(1)
            stage_wload()
            stage_dn1(0)
            stage_dn1(1)
            stage_dn2(0)
            stage_dn2(1)
    nc.compile()
    return nc


# ------------------------------------------------------------ host helpers
def _host_geometry(rots, trans, intr, post_rots, post_trans):
    import jax
    import jax.numpy as jnp
    with jax.default_device(jax.devices("cpu")[0]):
        f32 = jnp.float32
        ds = jnp.arange(1.0, 60.0, 1.0, dtype=f32)
        xs = jnp.linspace(0.0, IW - 1.0, FW, dtype=f32)
        ys = jnp.linspace(0.0, IH - 1.0, FH, dtype=f32)
        dm = jnp.broadcast_to(ds[:, None, None], (DD, FH, FW))
        xm = jnp.broadcast_to(xs[None, None, :], (DD, FH, FW))
        ym = jnp.broadcast_to(ys[None, :, None], (DD, FH, FW))
        fr = jnp.stack([xm, ym, dm], -1)
        pts = fr[None, None] - jnp.asarray(post_trans)[:, :, None, None, None, :]
        pts = jnp.einsum("bnij,bndhwj->bndhwi",
                         jnp.linalg.inv(jnp.asarray(post_rots)), pts)
        pts = jnp.concatenate([pts[..., :2] * pts[..., 2:3], pts[..., 2:3]], -1)
        comb = jnp.einsum("bnij,bnjk->bnik", jnp.asarray(rots),
                          jnp.linalg.inv(jnp.asarray(intr)))
        pts = jnp.einsum("bnij,bndhwj->bndhwi", comb, pts) \
            + jnp.asarray(trans)[:, :, None, None, None, :]
        lo = jnp.array([XY0, XY0, Z0], dtype=f32)
        dxv = jnp.array([DXY, DXY, DZ], dtype=f32)
        g = ((pts - lo) / dxv).astype(jnp.int32).reshape(-1, 3)
        kept = ((g[:, 0] >= 0) & (g[:, 0] < NX) & (g[:, 1] >= 0) & (g[:, 1] < NX)
                & (g[:, 2] >= 0) & (g[:, 2] < NZ))
        flat = (g[:, 2] * NX + g[:, 0]) * NX + g[:, 1]
        return np.asarray(flat, np.int64), np.asarray(kept)


def _prep_a_inputs(inputs):
    """Build per-core input maps for launch A."""
    d = np.asarray(inputs["d"], np.float32).reshape(N, IH, IW)
    x_img = np.asarray(inputs["x_img"], np.float32)

    # dt1 folded affine: relu(alpha*d + beta), alpha = s*w, beta = s*b + t
    a1 = (inputs["dt1_s"] * inputs["dt1_w"][:, 0, 0, 0]).astype(np.float32)
    b1 = (inputs["dt1_s"] * inputs["dt1_b"] + inputs["dt1_t"]).astype(np.float32)
    cab = np.arange(128)
    dt1_alpha = a1[cab // 16][:, None]
    dt1_beta = b1[cab // 16][:, None]

    def wprep_dt2():
        w = np.asarray(inputs["dt2_w"], np.float32)      # [32,8,5,5]
        out = np.zeros((4, 128, 32), np.float32)
        for ky in range(5):
            for kx in range(5):
                a, dky = ky % 4, ky // 4
                bph, dmx = (kx + 2) % 4, (kx + 2) // 4
                g = dky * 2 + dmx
                rows = (np.arange(8)) * 16 + a * 4 + bph
                out[g, rows, :] = w[:, :, ky, kx].T
        return out.astype(bf16)

    def wprep_dt3():
        w = np.asarray(inputs["dt3_w"], np.float32)      # [64,32,5,5]
        out = np.zeros((9, 128, 64), np.float32)
        for ky in range(5):
            for kx in range(5):
                a, dky = ky % 2, ky // 2
                bph, dmx = kx % 2, (kx + 2) // 2 - 1
                g = dky * 3 + dmx
                rows = (a * 2 + bph) * 32 + np.arange(32)
                out[g, rows, :] = w[:, :, ky, kx].T
        return out.astype(bf16)

    def wprep_3x3(w, icc_sizes):
        O, I = w.shape[0], w.shape[1]
        nic = len(icc_sizes)
        out = np.zeros((9, nic, 128, O), np.float32)
        for ky in range(3):
            for kx in range(3):
                tap = ky * 3 + kx
                ic0 = 0
                for icc, sz in enumerate(icc_sizes):
                    out[tap, icc, 0:sz, :] = w[:, ic0:ic0 + sz, ky, kx].T
                    ic0 += sz
        return out.astype(bf16)

    # NOTE: dn1 input concat order is [dt3(64) | x_img(256)] in the reference;
    # our matmul chunks are (x0:128, x1:128, dt3:64) -> weight cols must match:
    w_dn1_full = np.asarray(inputs["dn1_w"], np.float32)
    w_dn1 = np.zeros((9, 3, 128, 256), np.float32)
    for ky in range(3):
        for kx in range(3):
            tap = ky * 3 + kx
            w_dn1[tap, 0, :, :] = w_dn1_full[:, 64:192, ky, kx].T
            w_dn1[tap, 1, :, :] = w_dn1_full[:, 192:320, ky, kx].T
            w_dn1[tap, 2, 0:64, :] = w_dn1_full[:, 0:64, ky, kx].T
    w_dn1 = w_dn1.astype(bf16)
    w_dn2 = wprep_3x3(np.asarray(inputs["dn2_w"], np.float32), [128, 128])
    w_dn3 = np.asarray(inputs["dn3_w"], np.float32)[:, :, 0, 0]  # [139, 256]
    w_dn3p = np.zeros((2, 128, 139), np.float32)
    w_dn3p[0] = w_dn3[:, 0:128].T
    w_dn3p[1] = w_dn3[:, 128:256].T

    def fold_bias(b, s, t):
        # conv bias b then bn scale/shift: relu(s*(x+b) + t) = relu(s*x + (s*b+t))
        return np.asarray(s, np.float32), np.asarray(s * b + t, np.float32)

    s2, t2 = fold_bias(inputs["dt2_b"], inputs["dt2_s"], inputs["dt2_t"])
    s3, t3 = fold_bias(inputs["dt3_b"], inputs["dt3_s"], inputs["dt3_t"])
    sn1, tn1 = fold_bias(inputs["dn1_b"], inputs["dn1_s"], inputs["dn1_t"])
    sn2, tn2 = fold_bias(inputs["dn2_b"], inputs["dn2_s"], inputs["dn2_t"])
    b_dn3 = np.broadcast_to(np.asarray(inputs["dn3_b"], np.float32)[None, :],
                            (128, 139)).copy()

    consts = np.zeros((128, 153), np.float32)
    consts[:, 0] = dt1_alpha[:, 0]
    consts[:, 1] = dt1_beta[:, 0]
    consts[:, 2] = np.tile(s2, 4)
    consts[:, 3] = np.tile(t2, 4)
    consts[:, 4] = np.tile(s3, 2)
    consts[:, 5] = np.tile(t3, 2)
    consts[:, 6:8] = sn1.reshape(2, 128).T
    consts[:, 8:10] = tn1.reshape(2, 128).T
    consts[:, 10:12] = sn2.reshape(2, 128).T
    consts[:, 12:14] = tn2.reshape(2, 128).T
    consts[:, 14:153] = b_dn3
    shared = dict(
        consts=consts,
        w_dt2=wprep_dt2(), w_dt3=wprep_dt3(), w_dn1=w_dn1, w_dn2=w_dn2,
        w_dn3=w_dn3p.astype(bf16),
    )

    maps = []
    for c in range(NCORES):
        m = dict(shared)
        for s, (cam, h0) in enumerate([SEG_A[c], SEG_B[c]]):
            S = SEGS[s]
            d0 = 8 * h0 - 34
            dseg = np.zeros((S["nd"], 712), np.float32)
            vseg = np.zeros((S["nd"], 712), bool)
            lo, hi = max(0, d0), min(IH, d0 + S["nd"])
            if hi > lo:
                dseg[lo - d0:hi - d0, 4:708] = d[cam, lo:hi]
                vseg[lo - d0:hi - d0, 4:708] = True
            nq = S["nq"]
            ph = dseg.reshape(nq, 4, 178, 4)[:, :, :177, :]     # ry a rx b
            ph = ph.transpose(1, 3, 0, 2)                        # a b ry rx
            vph = vseg.reshape(nq, 4, 178, 4)[:, :, :177, :].transpose(1, 3, 0, 2)
            # dt1 applied on host: relu(alpha*d + beta), zero at pads
            dphc = np.where(vph[None],
                            np.maximum(a1[:, None, None, None, None] * ph[None]
                                       + b1[:, None, None, None, None], 0.0),
                            0.0)                                 # [8,4,4,nq,177]
            m[f"dph{s}"] = dphc.reshape(128, nq * 177).astype(bf16)
            q0, t0, r0 = 2 * h0 - 8, h0 - 3, h0 - 1
            qr = np.arange(S["nt2"]) + q0
            m2m = np.broadcast_to(((qr >= 0) & (qr < 64))[None, :],
                                  (128, S["nt2"]))
            tr = np.arange(S["nt3"]) + t0
            m3m = np.broadcast_to(((tr >= 0) & (tr < FH))[None, :],
                                  (128, S["nt3"]))
            rr = np.arange(S["nn1"]) + r0
            mn1m = np.broadcast_to(((rr >= 0) & (rr < FH))[None, :],
                                   (128, S["nn1"]))
            m[f"masks{s}"] = np.concatenate(
                [m2m, m3m, mn1m], axis=1).astype(bf16)
            xseg = np.zeros((2, 128, S["nt3"], 92), np.float32)
            lo2, hi2 = max(0, t0), min(FH, t0 + S["nt3"])
            if hi2 > lo2:
                xseg[:, :, lo2 - t0:hi2 - t0, 2:90] = \
                    x_img[cam, :, lo2:hi2, :].reshape(2, 128, hi2 - lo2, FW)
            m[f"xseg{s}"] = xseg.reshape(2, 128, S["nt3"] * 92).astype(bf16)
        maps.append(m)
    return maps


# ---------------------------------------------------------------- launch B
def build_launch_b(sizes):
    """Per chunk k: [128pix x 80ch] stationary feat tile x host-built
    [128pix x sizes[k] voxel-slot] depth-weight matrix -> [80, nv] window
    sums. W and out use packed (variable-size) layouts; W loads in a few
    batched DMAs, out in one."""
    nc = bacc.Bacc("TRN2", target_bir_lowering=False, debug=False,
                   num_devices=NCORES)
    NCH = len(sizes)
    offs = np.concatenate([[0], np.cumsum(sizes)]).astype(int)
    S = int(offs[-1])
    wmat = nc.dram_tensor("wmat", [128, S], dt.bfloat16,
                          kind="ExternalInput").ap()
    feats = nc.dram_tensor("feats", [128, NCH, CIMG], dt.bfloat16,
                           kind="ExternalInput").ap()
    owin = nc.dram_tensor("owin", [CIMG, S], dt.bfloat16,
                          kind="ExternalOutput").ap()
    NB = 4                                   # W DMA batches
    bnd = [int(round(NCH * i / NB)) for i in range(NB + 1)]
    with tile.TileContext(nc) as tc:
        with tc.tile_pool(name="const", bufs=1) as cpool, \
             tc.tile_pool(name="ps", bufs=4, space="PSUM") as pp:
            ft = cpool.tile([128, NCH, CIMG], dt.bfloat16, name="ft")
            kf = min(3, NCH)
            nc.sync.dma_start(out=ft[:, 0:kf, :], in_=feats[:, 0:kf, :])
            # act-table load happens under the W DMAs, not at first evac
            warm = cpool.tile([128, 1], dt.bfloat16, name="warm")
            nc.scalar.activation(warm[:], ft[:, 0, 0:1],
                                 mybir.ActivationFunctionType.Copy)
            wt = cpool.tile([128, S], dt.bfloat16, name="wt")
            nc.sync.dma_start(out=wt[:, 0:offs[bnd[1]]],
                              in_=wmat[:, 0:offs[bnd[1]]])
            if kf < NCH:
                nc.sync.dma_start(out=ft[:, kf:NCH, :], in_=feats[:, kf:NCH, :])
            for b in range(1, NB):
                lo, hi = offs[bnd[b]], offs[bnd[b + 1]]
                if hi > lo:
                    nc.sync.dma_start(out=wt[:, lo:hi], in_=wmat[:, lo:hi])
            ot = cpool.tile([CIMG, S], dt.bfloat16, name="ot")
            khalf = (NCH * 3) // 5
            for k in range(NCH):
                nv, o0 = int(sizes[k]), int(offs[k])
                ps = pp.tile([CIMG, 512], dt.float32, tag="ps", name="ps")
                nc.tensor.matmul(ps[:, 0:nv], ft[:, k, :], wt[:, o0:o0 + nv],
                                 start=True, stop=True)
                if k % 2 == 0:
                    nc.scalar.activation(ot[:, o0:o0 + nv], ps[:, 0:nv],
                                         mybir.ActivationFunctionType.Copy)
                else:
                    nc.vector.tensor_copy(ot[:, o0:o0 + nv], ps[:, 0:nv])
                if k == khalf:
                    # flush completed windows; only the tail rides the end
                    nc.sync.dma_start(out=owin[:, 0:int(offs[k + 1])],
                                      in_=ot[:, 0:int(offs[k + 1])])
            nc.sync.dma_start(out=owin[:, int(offs[khalf + 1]):S],
                              in_=ot[:, int(offs[khalf + 1]):S])
    nc.compile()
    return nc


# ---------------------------------------------------------------- launch C
C_OUT_ROWS = 23              # ds2-out rows per core (8*23 = 184 >= 180)


def build_launch_c():
    nc = bacc.Bacc("TRN2", target_bir_lowering=False, debug=False,
                   num_devices=NCORES)
    NR1 = C_OUT_ROWS + 2                         # ds1-out rows incl halo (25)
    NRP = 2 * NR1 + 1                            # pooled rows needed (51)
    slab = nc.dram_tensor("slab", [CIMG, NRP, 362], dt.bfloat16,
                          kind="ExternalInput").ap()
    m1 = nc.dram_tensor("m1", [128, NR1], dt.bfloat16, kind="ExternalInput").ap()
    wd1 = nc.dram_tensor("wd1", [CIMG, 9, CIMG], dt.bfloat16,
                         kind="ExternalInput").ap()
    wd2 = nc.dram_tensor("wd2", [CIMG, 9, CIMG], dt.bfloat16,
                         kind="ExternalInput").ap()
    sb1 = nc.dram_tensor("sb1", [CIMG, 2], dt.float32, kind="ExternalInput").ap()
    sb2 = nc.dram_tensor("sb2", [CIMG, 2], dt.float32, kind="ExternalInput").ap()
    yout = nc.dram_tensor("yout", [CIMG, C_OUT_ROWS, 180], dt.float32,
                          kind="ExternalOutput").ap()
    RELU = mybir.ActivationFunctionType.Relu
    with tile.TileContext(nc) as tc:
        with tc.tile_pool(name="const", bufs=1) as cpool,              tc.tile_pool(name="work", bufs=2) as wp,              tc.tile_pool(name="big", bufs=1) as bp,              tc.tile_pool(name="ps", bufs=3, space="PSUM") as pp:
            # weights/consts first so ds1 can start on the first slab chunk
            w1 = cpool.tile([CIMG, 9, CIMG], dt.bfloat16, name="w1")
            nc.sync.dma_start(out=w1[:], in_=wd1)
            sb1t = cpool.tile([CIMG, 2], dt.float32, name="sb1t")
            nc.sync.dma_start(out=sb1t[:], in_=sb1)
            warm = wp.tile([CIMG, 1], dt.float32, tag="warm", name="warm")
            nc.scalar.activation(warm[:], sb1t[:, 0:1], RELU)
            m1t = wp.tile([128, NR1], dt.bfloat16, name="m1t")
            nc.sync.dma_start(out=m1t[:], in_=m1)
            slabt = bp.tile([CIMG, NRP, 362], dt.bfloat16, name="slabt")
            for rr in range(0, NRP, 9):
                nrr = min(9, NRP - rr)
                nc.sync.dma_start(out=slabt[:, rr:rr + nrr, :],
                                  in_=slab[:, rr:rr + nrr, :])
            w2 = cpool.tile([CIMG, 9, CIMG], dt.bfloat16, name="w2")
            nc.sync.dma_start(out=w2[:], in_=wd2)
            sb2t = cpool.tile([CIMG, 2], dt.float32, name="sb2t")
            nc.sync.dma_start(out=sb2t[:], in_=sb2)
            h1 = bp.tile([CIMG, NR1, 182], dt.bfloat16, name="h1")
            nc.vector.memset(h1[:, :, 0:1], 0.0)
            nc.vector.memset(h1[:, :, 181:182], 0.0)
            # ds1: stride-2 3x3; out row t reads slab rows 2t..2t+2 (slab row 0
            # = pooled row 2o0-3, so out row t (global o0-1+t) reads
            # 2(o0-1+t)-1..+1 - (2o0-3) = 2t..2t+2); col c reads 2c..2c+2
            RP = 2
            for t0 in range(0, NR1, RP):
                nr = min(RP, NR1 - t0)
                ps = pp.tile([CIMG, nr, 180], dt.float32, tag="ps1", name="ps")
                gi = 0
                for ky in range(3):
                    for kx in range(3):
                        rhs = bass.AP(slabt.tensor,
                                      slabt.offset + (2 * t0 + ky) * 362 + kx,
                                      [slabt.ap[0], [2 * 362, nr], [2, 180]])
                        nc.tensor.matmul(ps[:], w1[:, ky * 3 + kx, :], rhs,
                                         start=(gi == 0), stop=(gi == 8))
                        gi += 1
                ev = wp.tile([CIMG, nr, 180], dt.bfloat16, tag="ev", name="ev")
                nc.scalar.activation(ev[:], ps[:], RELU, bias=sb1t[:, 1:2],
                                     scale=sb1t[:, 0:1])
                mbb = bass.AP(m1t.tensor, m1t.offset + t0,
                              [[m1t.ap[0][0], CIMG], [1, nr], [0, 180]])
                nc.vector.tensor_tensor(out=h1[:, t0:t0 + nr, 1:181],
                                        in0=ev[:], in1=mbb,
                                        op=mybir.AluOpType.mult)
            # ds2: 3x3 pad 1: out row o reads h1 rows o..o+2, col c: c..c+2
            yo = bp.tile([CIMG, C_OUT_ROWS, 180], dt.float32, name="yo")
            for o0 in range(0, C_OUT_ROWS, RP):
                nr = min(RP, C_OUT_ROWS - o0)
                ps = pp.tile([CIMG, nr, 180], dt.float32, tag="ps2", name="ps")
                gi = 0
                for ky in range(3):
                    for kx in range(3):
                        rhs = bass.AP(h1.tensor,
                                      h1.offset + (o0 + ky) * 182 + kx,
                                      [h1.ap[0], [182, nr], [1, 180]])
                        nc.tensor.matmul(ps[:], w2[:, ky * 3 + kx, :], rhs,
                                         start=(gi == 0), stop=(gi == 8))
                        gi += 1
                nc.scalar.activation(yo[:, o0:o0 + nr, :], ps[:], RELU,
                                     bias=sb2t[:, 1:2], scale=sb2t[:, 0:1])
                if (o0 // RP) % 3 == 2 or o0 + nr >= C_OUT_ROWS:
                    lo = (o0 // (3 * RP)) * 3 * RP
                    nc.sync.dma_start(out=yout[:, lo:o0 + nr, :],
                                      in_=yo[:, lo:o0 + nr, :])
    nc.compile()
    return nc


_CACHE = {}


def run_launch_a(inputs):
    if "A" not in _CACHE:
        _CACHE["A"] = build_launch_a()
    nc = _CACHE["A"]
    maps = _prep_a_inputs(inputs)
    res = run_bass_kernel_spmd(nc, maps, list(range(NCORES)))
    depth = np.zeros((NPIX, DD), np.float32)
    feat = np.zeros((NPIX, CIMG), np.float32)
    for c in range(NCORES):
        r = res.results[c]
        for s, (cam, h0) in enumerate([SEG_A[c], SEG_B[c]]):
            S = SEGS[s]
            npix = S["nout"] * FW
            base = (cam * FH + h0) * FW
            a0, pcs = (0, 11) if s == 0 else (11, 6)
            dsg = r["out_depth"][:, a0:a0 + pcs].transpose(1, 0, 2)
            depth[base:base + npix] = dsg.reshape(pcs * 128, DD)[:npix]
            fsg = r["out_feat"][:, a0:a0 + pcs].transpose(1, 0, 2)
            feat[base:base + npix] = fsg.reshape(pcs * 128, CIMG)[:npix]
    return depth, feat


def _build_chunks(flat, kept, depth_rows):
    """Group points by (camera, column-block); per group build the
    [pix, voxel-slot] depth-weight matrix over the group's voxel union.
    Splits column blocks whose union exceeds the PSUM window (512)."""
    fl = flat.reshape(N, DD, FH, FW)
    kp = kept.reshape(N, DD, FH, FW)
    chunks = []                      # (pix_ids, Wdense[npix, nv], vox_ids)

    def add_group(n, w0, w1):
        nw = w1 - w0
        f = fl[n, :, :, w0:w1]                       # [DD, FH, nw]
        k = kp[n, :, :, w0:w1]
        vids = np.unique(f[k])
        if len(vids) > 512 and nw > 1:
            mid = w0 + nw // 2
            add_group(n, w0, mid)
            add_group(n, mid, w1)
            return
        nv = max(len(vids), 1)
        # pixel local idx = (w - w0) * FH + h; point (d, h, w)
        slot = np.searchsorted(vids, f[k]) if len(vids) else np.zeros(0, np.int64)
        dd, hh, ww = np.nonzero(k)
        pix_loc = ww * FH + hh
        pixcol = n * FH * FW + hh * FW + (ww + w0)
        dep = depth_rows[pixcol, dd]
        Wd = np.bincount(pix_loc * nv + slot, weights=dep,
                         minlength=nw * FH * nv).reshape(nw * FH, nv)
        pix_ids = (n * FH * FW + np.arange(FH)[None, :] * FW
                   + (w0 + np.arange(nw))[:, None]).reshape(-1)
        chunks.append((pix_ids, Wd, vids))

    for n in range(N):
        for w0 in range(0, FW, 4):
            add_group(n, w0, w0 + 4)
    return chunks


def _prep_b_inputs(chunks, featflat_bf):
    """Balance chunks across cores by window size; build per-core maps with
    the packed per-slot layout (chunk k size = max over cores, desc-sorted)."""
    order = sorted(range(len(chunks)), key=lambda i: -chunks[i][1].shape[1])
    load = np.zeros(NCORES, np.int64)
    per_core = [[] for _ in range(NCORES)]
    for i in order:
        c = int(np.argmin(load))
        per_core[c].append(i)
        load[c] += chunks[i][1].shape[1]
    NCH = max(len(p) for p in per_core)
    sizes = np.zeros(NCH, np.int64)
    for p in per_core:
        for k, i in enumerate(p):
            sizes[k] = max(sizes[k], chunks[i][1].shape[1])
    sizes = (sizes + 15) // 16 * 16
    offs = np.concatenate([[0], np.cumsum(sizes)]).astype(int)
    S = int(offs[-1])
    maps, scatter = [], []
    for c in range(NCORES):
        wm = np.zeros((128, S), bf16)
        ft = np.zeros((128, NCH, CIMG), bf16)
        sc = []
        for k, i in enumerate(per_core[c]):
            pix_ids, Wd, vids = chunks[i]
            npix, nv = Wd.shape
            wm[0:npix, offs[k]:offs[k] + nv] = Wd
            ft[0:npix, k, :] = featflat_bf[pix_ids]
            sc.append((int(offs[k]), vids))
        maps.append(dict(wmat=wm, feats=ft))
        scatter.append(sc)
    return maps, scatter, tuple(int(s) for s in sizes)


def _prep_c_inputs(inputs, pooled_t):
    """pooled_t: [CIMG, 360, 360] f32 -> per-core slabs + masks + weights."""
    NR1 = C_OUT_ROWS + 2
    NRP = 2 * NR1 + 1
    w1 = np.asarray(inputs["ds1_w"], np.float32)
    w2 = np.asarray(inputs["ds2_w"], np.float32)
    # wd1/wd2: [ic, tap, oc]
    wd1 = np.ascontiguousarray(w1.transpose(1, 2, 3, 0).reshape(CIMG, 9, CIMG))
    wd2 = np.ascontiguousarray(w2.transpose(1, 2, 3, 0).reshape(CIMG, 9, CIMG))
    sb1 = np.stack([np.asarray(inputs["ds1_s"], np.float32),
                    np.asarray(inputs["ds1_t"], np.float32)], 1)
    sb2 = np.stack([np.asarray(inputs["ds2_s"], np.float32),
                    np.asarray(inputs["ds2_t"], np.float32)], 1)
    shared = dict(wd1=wd1.astype(bf16), wd2=wd2.astype(bf16), sb1=sb1, sb2=sb2)
    maps = []
    pt_bf = pooled_t.astype(bf16)
    for c in range(NCORES):
        o0g = C_OUT_ROWS * c
        p0 = 2 * o0g - 3
        slab = np.zeros((CIMG, NRP, 362), bf16)
        lo, hi = max(0, p0), min(NX, p0 + NRP)
        if hi > lo:
            slab[:, lo - p0:hi - p0, 1:361] = pt_bf[:, lo:hi, :]
        t1g = np.arange(NR1) + (o0g - 1)
        m1 = np.broadcast_to(((t1g >= 0) & (t1g < 180))[None, :],
                             (128, NR1)).astype(bf16)
        maps.append(dict(shared, slab=slab, m1=np.ascontiguousarray(m1)))
    return maps


def kernel(**inputs):
    inputs = {k: np.asarray(v) for k, v in inputs.items()}
    flat, kept = _host_geometry(inputs["cam2lidar_rots"],
                                inputs["cam2lidar_trans"], inputs["intrins"],
                                inputs["post_rots"], inputs["post_trans"])
    depth_rows, feat_rows = run_launch_a(inputs)
    featflat_bf = feat_rows.astype(bf16)

    chunks = _build_chunks(flat, kept, depth_rows)
    bmaps, scatter, sizes = _prep_b_inputs(chunks, featflat_bf)
    key = ("B", sizes)
    if key not in _CACHE:
        _CACHE[key] = build_launch_b(sizes)
    res_b = run_bass_kernel_spmd(_CACHE[key], bmaps, list(range(NCORES)))

    allvox = np.concatenate([vids for c in range(NCORES)
                             for _, vids in scatter[c]])
    allval = np.concatenate(
        [res_b.results[c]["owin"][:, o0:o0 + len(vids)].T.astype(np.float32)
         for c in range(NCORES) for o0, vids in scatter[c]])
    o = np.argsort(allvox, kind="stable")
    allvox, allval = allvox[o], allval[o]
    starts = np.flatnonzero(np.r_[True, allvox[1:] != allvox[:-1]])
    pooled = np.zeros((NX * NX, CIMG), np.float32)
    pooled[allvox[starts]] = np.add.reduceat(allval, starts, axis=0)
    pooled_t = np.ascontiguousarray(
        pooled.reshape(NX, NX, CIMG).transpose(2, 0, 1))

    if "C" not in _CACHE:
        _CACHE["C"] = build_launch_c()
    cmaps = _prep_c_inputs(inputs, pooled_t)
    res_c = run_bass_kernel_spmd(_CACHE["C"], cmaps, list(range(NCORES)))
    out = np.zeros((1, CIMG, 180, 180), np.float32)
    for c in range(NCORES):
        o0g = C_OUT_ROWS * c
        nr = min(C_OUT_ROWS, 180 - o0g)
        if nr > 0:
            out[0, :, o0g:o0g + nr, :] = res_c.results[c]["yout"][:, 0:nr, :]
    return out

